# revision 1
# baseline (speedup 1.0000x reference)
"""Trainium2 Bass kernel for hetero-GNN (2x ResGatedGraphConv + segment-mean pooling + MLP).

Sharding: destination-node range per core; each core processes the edges whose
dst falls in its range (edge lists sorted/bucketed by dst on host — index
marshalling only). All model arithmetic runs on device:
  - per-edge fused matmul: [x_src.T ; ea ; 1 ; x_dst.T] @ W_aug
      -> [q+k+2e+bias | v+e+bias]  (one constant-weight matmul)
  - sigmoid (ACT), gated message (DVE)
  - scatter-add via one-hot matmul accumulated in per-bucket PSUM
  - skip connection + relu, segment-mean pooling via one-hot matmul
  - cross-core AllReduce of pooled partials, 4-layer MLP head.
"""
import sys
import types
import numpy as np

NCORES = 8
G = 128
H = 64
F = 16
NC_N = 100000
NB_N = 200000
BUCKET = 128
GRP = 4  # 128-edge sub-tiles per matmul group
LAST_EXEC_NS = None


def _install_ntff_shim():
    if 'antenv.axon_hooks' in sys.modules:
        return
    try:
        mod = types.ModuleType('antenv.axon_hooks')
        _h = [None]
        mod.set_axon_ntff_profile_hook = lambda h: _h.__setitem__(0, h)
        mod.get_axon_ntff_profile_hook = lambda: _h[0]
        sys.modules['antenv.axon_hooks'] = mod
        import antenv
        antenv.axon_hooks = mod
        from trn_agent_boot.trn_boot import _ntff_profile_via_ctypes
        mod.set_axon_ntff_profile_hook(
            _ntff_profile_via_ctypes('/opt/axon/libaxon_pjrt.so'))
    except Exception:
        pass


def _prep_relation(x_src, x_dst, src, dst, ea, D):
    """Host marshalling for one relation: per-core padded edge streams."""
    nbuck = (D + BUCKET - 1) // BUCKET
    order = np.argsort(dst, kind="stable")
    src_s, dst_s, ea_s = src[order], dst[order], ea[order, 0]
    core_of = dst_s // D
    buck_of = (dst_s % D) // BUCKET
    counts = np.zeros((NCORES, nbuck), np.int64)
    np.add.at(counts, (core_of, buck_of), 1)
    subtiles = np.maximum((counts.max(axis=0) + 127) // 128, 1)   # [nbuck]
    ntot = int(subtiles.sum()) * 128
    starts = np.zeros(nbuck + 1, np.int64)
    starts[1:] = np.cumsum(subtiles) * 128
    xs = x_src.astype(np.float16)
    xd = x_dst.astype(np.float16)
    per_core = []
    core_bounds = np.searchsorted(core_of, np.arange(NCORES + 1))
    for m in range(NCORES):
        lo, hi = core_bounds[m], core_bounds[m + 1]
        c_src, c_dst, c_ea = src_s[lo:hi], dst_s[lo:hi], ea_s[lo:hi]
        c_buck = (c_dst % D) // BUCKET
        pos_in_bucket = np.arange(len(c_src)) - np.searchsorted(c_buck, c_buck)
        slot = starts[c_buck] + pos_in_bucket
        xt = np.zeros((35, ntot), np.float16)
        ld = np.full(ntot, -1.0, np.float32)
        xt[0:16, slot] = xs[c_src].T
        xt[16, slot] = c_ea.astype(np.float16)
        xt[17, slot] = 1.0
        xt[18:34, slot] = xd[c_dst].T
        ld[slot] = (c_dst % D) % BUCKET
        per_core.append({
            "xt": xt,
            "ld": np.ascontiguousarray(ld.reshape(-1, 128).T),  # [128, nsub]
        })
    return {"nbuck": nbuck, "subtiles": subtiles, "ntot": ntot,
            "per_core": per_core}


def kernel(**inputs):
    _install_ntff_shim()
    import concourse.bass as bass  # noqa: F401
    import concourse.bacc as bacc
    import concourse.mybir as mybir
    import concourse.tile as tile
    from concourse.bass_utils import run_bass_kernel_spmd

    F32 = mybir.dt.float32
    F16 = mybir.dt.float16
    AF = mybir.ActivationFunctionType
    OP = mybir.AluOpType

    ii = {k: np.asarray(v) for k, v in inputs.items()}
    Dc, Db = NC_N // NCORES, NB_N // NCORES

    rel_c = _prep_relation(ii["x_x"], ii["x_c"], ii["src_ac"].astype(np.int64),
                           ii["dst_ac"].astype(np.int64), ii["ea_ac"], Dc)
    rel_b = _prep_relation(ii["x_c"], ii["x_b"], ii["src_cb"].astype(np.int64),
                           ii["dst_cb"].astype(np.int64), ii["ea_cb"], Db)

    def phase_a(x_dst, D, m):
        sl = x_dst[m * D:(m + 1) * D]
        a = np.zeros((17, D), np.float16)
        a[0:16] = sl.T.astype(np.float16)
        a[16] = 1.0
        return a

    def batch_layout(batch, D, m):
        nbuck = (D + BUCKET - 1) // BUCKET
        sl = batch[m * D:(m + 1) * D].astype(np.float32)
        padded = np.full(nbuck * BUCKET, -1.0, np.float32)
        padded[:D] = sl
        return np.ascontiguousarray(padded.reshape(nbuck, BUCKET).T)  # [128, nbuck]

    cnt_c = np.bincount(ii["batch_c"].astype(np.int64), minlength=G).astype(np.float32)
    cnt_b = np.bincount(ii["batch_b"].astype(np.int64), minlength=G).astype(np.float32)
    recip = np.stack([1.0 / np.maximum(cnt_c, 1.0),
                      1.0 / np.maximum(cnt_b, 1.0)]).astype(np.float16)  # [2, G]

    def waug(rel):
        Wq, Wv, Wk = ii[f"Wq_{rel}"], ii[f"Wv_{rel}"], ii[f"Wk_{rel}"]
        We = ii[f"We_{rel}"][0]
        bq, bv, bk, be = (ii[f"bq_{rel}"], ii[f"bv_{rel}"],
                          ii[f"bk_{rel}"], ii[f"be_{rel}"])
        w = np.zeros((35, 128), np.float32)
        w[0:16, 0:64] = Wq; w[0:16, 64:128] = Wv
        w[16, 0:64] = 2 * We; w[16, 64:128] = We
        w[17, 0:64] = bq + bk + 2 * be; w[17, 64:128] = bv + be
        w[18:34, 0:64] = Wk
        return w.astype(np.float16)

    def wskip(rel):
        w = np.zeros((17, 64), np.float32)
        w[0:16] = ii[f"Wskip_{rel}"]
        w[16] = ii[f"bconv_{rel}"]
        return w.astype(np.float16)

    iota_row = np.tile(np.arange(BUCKET, dtype=np.float32), (128, 1))
    iota_g = np.tile(np.arange(G, dtype=np.float32), (128, 1))
    mlp_w = {
        "W1": ii["W1"].astype(np.float16), "W2": ii["W2"].astype(np.float16),
        "W3": ii["W3"].astype(np.float16), "Wout": ii["Wout"].astype(np.float16),
        "b1": ii["b1"].astype(np.float32).reshape(64, 1),
        "b2": ii["b2"].astype(np.float32).reshape(64, 1),
        "b3": ii["b3"].astype(np.float32).reshape(64, 1),
        "bout": ii["bout"].astype(np.float32).reshape(1, 1),
    }

    # ---------------- device program ----------------
    nc = bacc.Bacc("TRN2", target_bir_lowering=False, debug=False,
                   num_devices=NCORES)

    def din(name, arr0):
        return nc.dram_tensor(name, list(arr0.shape),
                              mybir.dt.from_np(arr0.dtype), kind="ExternalInput")

    h = {}
    h["xt_c"] = din("xt_c", rel_c["per_core"][0]["xt"])
    h["xt_b"] = din("xt_b", rel_b["per_core"][0]["xt"])
    h["ld_c"] = din("ld_c", rel_c["per_core"][0]["ld"])
    h["ld_b"] = din("ld_b", rel_b["per_core"][0]["ld"])
    h["pa_c"] = din("pa_c", phase_a(ii["x_c"], Dc, 0))
    h["pa_b"] = din("pa_b", phase_a(ii["x_b"], Db, 0))
    h["bt_c"] = din("bt_c", batch_layout(ii["batch_c"], Dc, 0))
    h["bt_b"] = din("bt_b", batch_layout(ii["batch_b"], Db, 0))
    h["waug_c"] = din("waug_c", waug("ac"))
    h["waug_b"] = din("waug_b", waug("cb"))
    h["wskip_c"] = din("wskip_c", wskip("ac"))
    h["wskip_b"] = din("wskip_b", wskip("cb"))
    h["iota"] = din("iota", iota_row)
    h["iotag"] = din("iotag", iota_g)
    h["recip"] = din("recip", recip)
    sel2 = np.zeros((2, 128), np.float16); sel2[0, 0:64] = 1; sel2[1, 64:128] = 1
    h["ones2"] = din("ones2", sel2)
    for k, v in mlp_w.items():
        h["mlp_" + k] = din("mlp_" + k, v)
    out_h = nc.dram_tensor("out", [1, G], F32, kind="ExternalOutput")

    with tile.TileContext(nc) as tc:
        with tc.tile_pool(name="const", bufs=1) as cp, \
             tc.tile_pool(name="acc", bufs=1) as accp, \
             tc.tile_pool(name="stream", bufs=3) as sp, \
             tc.tile_pool(name="work", bufs=3) as wp, \
             tc.tile_pool(name="psum", bufs=2, space="PSUM") as pp, \
             tc.tile_pool(name="psA", bufs=1, space="PSUM") as ppA, \
             tc.tile_pool(name="dram", bufs=1, space="DRAM") as dp:

            iota_t = cp.tile([128, BUCKET], F32, tag="iota_t")
            nc.sync.dma_start(iota_t[:], h["iota"].ap())
            iota4_t = cp.tile([128, GRP, BUCKET], F32, tag="iota4_t")
            for _j in range(GRP):
                nc.vector.tensor_copy(iota4_t[:, _j, :], iota_t[:])
            iotag_t = cp.tile([128, G], F32, tag="iotag_t")
            nc.sync.dma_start(iotag_t[:], h["iotag"].ap())

            pooled_ps = ppA.tile([128, G], F32, tag="pooled_ps")

            def relation(tag, rel, D, row_off):
                nbuck = rel["nbuck"]
                subtiles = rel["subtiles"]
                w_t = cp.tile([35, 128], F16, name=f"waug_{tag}", tag=f"waug_{tag}")
                nc.sync.dma_start(w_t[:], h[f"waug_{tag}"].ap())
                ws_t = cp.tile([17, 64], F16, name=f"wskip_{tag}", tag=f"wskip_{tag}")
                nc.sync.dma_start(ws_t[:], h[f"wskip_{tag}"].ap())

                agg = accp.tile([128, nbuck * 64], F32, name=f"agg_{tag}",
                                tag=f"agg_{tag}")
                pa_sb = accp.tile([17, D], F16, name=f"pa_{tag}", tag=f"pa_{tag}")
                nc.sync.dma_start(pa_sb[:], h[f"pa_{tag}"].ap())
                for b in range(nbuck):
                    w = min(BUCKET, D - b * BUCKET)
                    ps = pp.tile([128, 64], F32, name=f"skps_{tag}_{b}", tag="skps")
                    nc.tensor.matmul(ps[:w, :], pa_sb[:, b * BUCKET:b * BUCKET + w],
                                     ws_t[:], start=True, stop=True)
                    if w < BUCKET:
                        nc.vector.memset(agg[:, b * 64:(b + 1) * 64], 0.0)
                    nc.vector.tensor_copy(agg[:w, b * 64:(b + 1) * 64], ps[:w, :])

                xt_v = h[f"xt_{tag}"].ap()
                ld_v = h[f"ld_{tag}"].ap()
                sub0 = 0
                for b in range(nbuck):
                    nsub = int(subtiles[b])
                    bps = pp.tile([128, 64], F32, name=f"bps_{tag}_{b}", tag="bps")
                    s = 0
                    while s < nsub:
                        g = min(GRP, nsub - s)
                        e0 = (sub0 + s) * 128
                        xt_t = sp.tile([35, GRP * 128], F16, name=f"xt_{tag}_{b}_{s}",
                                       tag="xt")
                        nc.sync.dma_start(xt_t[:, :g * 128], xt_v[:, e0:e0 + g * 128])
                        ld_t = sp.tile([128, GRP], F32, name=f"ldt_{tag}_{b}_{s}",
                                       tag="ldt")
                        nc.sync.dma_start(ld_t[:, :g],
                                          ld_v[:, sub0 + s:sub0 + s + g])
                        sv = pp.tile([128, GRP * 128], F32, name=f"sv_{tag}_{b}_{s}",
                                     tag="sv")
                        for j in range(g):
                            nc.tensor.matmul(sv[:, j * 128:(j + 1) * 128],
                                             xt_t[:, j * 128:(j + 1) * 128],
                                             w_t[:], start=True, stop=True)
                        sv3 = sv[:].rearrange("p (a b) -> p a b", a=GRP)
                        gt = wp.tile([128, GRP, 64], F32, name=f"gt_{tag}_{b}_{s}",
                                     tag="gt")
                        nc.scalar.activation(gt[:, :g, :], sv3[:, :g, 0:64],
                                             AF.Sigmoid)
                        msg = wp.tile([128, GRP, 64], F16, name=f"msg_{tag}_{b}_{s}",
                                      tag="msg")
                        nc.vector.tensor_tensor(msg[:, :g, :], gt[:, :g, :],
                                                sv3[:, :g, 64:128], op=OP.mult)
                        oh4 = wp.tile([128, GRP, BUCKET], F16,
                                      name=f"oh_{tag}_{b}_{s}", tag="oh")
                        ld3 = ld_t[:, :g].rearrange("p (a o) -> p a o", o=1)
                        nc.vector.tensor_tensor(
                            oh4[:, :g, :], iota4_t[:, :g, :],
                            ld3.broadcast_to([128, g, BUCKET]),
                            op=OP.is_equal)
                        for j in range(g):
                            nc.tensor.matmul(bps[:], oh4[:, j, :], msg[:, j, :],
                                             start=(s + j == 0),
                                             stop=(s + j == nsub - 1),
                                             skip_group_check=True)
                        s += g
                    nc.vector.tensor_tensor(agg[:, b * 64:(b + 1) * 64],
                                            agg[:, b * 64:(b + 1) * 64], bps[:],
                                            op=OP.add)
                    sub0 += nsub

                h_sb = accp.tile([128, nbuck * 64], F16, name=f"h_{tag}",
                                 tag=f"h_{tag}")
                nc.scalar.activation(h_sb[:], agg[:], AF.Relu)
                bt_sb = accp.tile([128, nbuck], F32, name=f"bt_{tag}",
                                  tag=f"bt_{tag}")
                nc.sync.dma_start(bt_sb[:], h[f"bt_{tag}"].ap())
                for b in range(nbuck):
                    ohg = wp.tile([128, G], F16, name=f"ohg_{tag}_{b}", tag="ohg")
                    nc.vector.tensor_scalar(ohg[:], iotag_t[:], bt_sb[:, b:b + 1],
                                            None, OP.is_equal)
                    nc.tensor.matmul(pooled_ps[row_off:row_off + 64, :],
                                     h_sb[:, b * 64:(b + 1) * 64], ohg[:],
                                     start=(b == 0), stop=(b == nbuck - 1),
                                     skip_group_check=True)

            relation("c", rel_c, Dc, 0)
            relation("b", rel_b, Db, 64)

            pooled_sb = accp.tile([128, G], F32, tag="pooled_sb")
            nc.vector.tensor_copy(pooled_sb[:], pooled_ps[:])
            bounce_in = dp.tile([128, G], F32, tag="bounce_in")
            bounce_out = dp.tile([128, G], F32, tag="bounce_out")
            nc.sync.dma_start(bounce_in[:], pooled_sb[:])
            nc.gpsimd.collective_compute(
                "AllReduce", OP.add, replica_groups=[list(range(NCORES))],
                ins=[bounce_in.opt()], outs=[bounce_out.opt()])
            nc.sync.dma_start(pooled_sb[:], bounce_out[:])

            recip_sb = accp.tile([2, G], F16, tag="recip_sb")
            nc.sync.dma_start(recip_sb[:], h["recip"].ap())
            ones2_sb = accp.tile([2, 128], F16, tag="ones2_sb")
            nc.sync.dma_start(ones2_sb[:], h["ones2"].ap())
            rb_ps = ppA.tile([128, G], F32, tag="mlps")
            nc.tensor.matmul(rb_ps[:], ones2_sb[:], recip_sb[:],
                             start=True, stop=True)
            mean_sb = accp.tile([128, G], F16, tag="mean_sb")
            nc.vector.tensor_tensor(mean_sb[:], pooled_sb[:], rb_ps[:], op=OP.mult)

            mw, mb = {}, {}
            for k in ("W1", "W2", "W3", "Wout"):
                mw[k] = accp.tile(list(mlp_w[k].shape), F16, name=f"mw{k}",
                                  tag=f"mw{k}")
                nc.sync.dma_start(mw[k][:], h["mlp_" + k].ap())
            for k in ("b1", "b2", "b3", "bout"):
                mb[k] = accp.tile(list(mlp_w[k].shape), F32, name=f"mb{k}",
                                  tag=f"mb{k}")
                nc.sync.dma_start(mb[k][:], h["mlp_" + k].ap())

            hcur = mean_sb
            for li, (wk, bk) in enumerate((("W1", "b1"), ("W2", "b2"),
                                           ("W3", "b3"))):
                ps = ppA.tile([64, G], F32, name=f"mlp{li}", tag="mlps")
                nc.tensor.matmul(ps[:], mw[wk][:], hcur[:], start=True, stop=True)
                hn = accp.tile([64, G], F16, name=f"hn{li}", tag=f"hn{li}")
                nc.scalar.activation(hn[:], ps[:], AF.Relu, bias=mb[bk][:])
                hcur = hn
            ps_o = ppA.tile([1, G], F32, tag="mlps")
            nc.tensor.matmul(ps_o[:], mw["Wout"][:], hcur[:], start=True, stop=True)
            osb = accp.tile([1, G], F32, tag="osb")
            nc.scalar.activation(osb[:], ps_o[:], AF.Identity, bias=mb["bout"][:])
            nc.sync.dma_start(out_h.ap(), osb[:])

    nc.compile()

    in_maps = []
    for m in range(NCORES):
        in_maps.append({
            "xt_c": rel_c["per_core"][m]["xt"],
            "xt_b": rel_b["per_core"][m]["xt"],
            "ld_c": rel_c["per_core"][m]["ld"],
            "ld_b": rel_b["per_core"][m]["ld"],
            "pa_c": phase_a(ii["x_c"], Dc, m), "pa_b": phase_a(ii["x_b"], Db, m),
            "bt_c": batch_layout(ii["batch_c"], Dc, m),
            "bt_b": batch_layout(ii["batch_b"], Db, m),
            "waug_c": waug("ac"), "waug_b": waug("cb"),
            "wskip_c": wskip("ac"), "wskip_b": wskip("cb"),
            "iota": iota_row, "iotag": iota_g, "recip": recip,
            "ones2": sel2,
            **{"mlp_" + k: v for k, v in mlp_w.items()},
        })
    import os
    trace = bool(os.environ.get("KERNEL_TRACE"))
    res = run_bass_kernel_spmd(nc, in_maps, core_ids=list(range(NCORES)),
                               trace=trace)
    global LAST_EXEC_NS
    LAST_EXEC_NS = res.exec_time_ns
    return res.results[0]["out"].reshape(G).astype(np.float32)



# revision 28
# speedup vs baseline: 1.2963x; 1.2963x over previous
"""Trainium2 Bass kernel for hetero-GNN (2x ResGatedGraphConv + segment-mean pooling + MLP).

Sharding: destination-node range per core; each core processes the edges whose
dst falls in its range. Host does index marshalling only; all model arithmetic
runs on device.

Device strategy ("degree rounds"):
  - dst nodes are grouped into 128-slot buckets; 8 buckets form a PSUM group
    whose aggregate [128 slots, 8*64] lives in one PSUM bank.
  - edges of a bucket are packed into R identity rounds (the j-th edge of
    slot p sits at row p of round j) plus <=F flex rounds (leftovers, with a
    host-built fp8 one-hot scatter matrix).
  - a slab = one round of each of the 8 buckets = 8 subtiles of 128 edges.
    Per slab: 8 fused matmuls [xt.T @ W_aug] (row-tiled concurrent pairs),
    one sigmoid (ACT), one gated multiply (DVE), and ONE identity-stationary
    matmul that scatter-adds all 8 subtiles into the group aggregate.
  - skip connection is folded into the same PSUM accumulation (a dummy zero
    matmul opens the accumulation group for the whole bank).
  - relu + pooling via per-bucket one-hot matmul into a [G, 2H] transposed
    pooled PSUM; AllReduce across 8 cores; small MLP head on device.
"""
import os
import sys
import types
import numpy as np
import ml_dtypes

F8NP = ml_dtypes.float8_e4m3fn

NCORES = 8
G = 128
H = 64
F = 16
NC_N = 100000
NB_N = 200000
GRP = 8          # buckets per PSUM group
LAST_EXEC_NS = None


def _install_ntff_shim():
    if 'antenv.axon_hooks' in sys.modules:
        return
    try:
        mod = types.ModuleType('antenv.axon_hooks')
        _h = [None]
        mod.set_axon_ntff_profile_hook = lambda h: _h.__setitem__(0, h)
        mod.get_axon_ntff_profile_hook = lambda: _h[0]
        sys.modules['antenv.axon_hooks'] = mod
        import antenv
        antenv.axon_hooks = mod
        from trn_agent_boot.trn_boot import _ntff_profile_via_ctypes
        mod.set_axon_ntff_profile_hook(
            _ntff_profile_via_ctypes('/opt/axon/libaxon_pjrt.so'))
    except Exception:
        pass


def _waug(ii, rel):
    Wq, Wv, Wk = ii[f"Wq_{rel}"], ii[f"Wv_{rel}"], ii[f"Wk_{rel}"]
    We = ii[f"We_{rel}"][0]
    bq, bv, bk, be = (ii[f"bq_{rel}"], ii[f"bv_{rel}"],
                      ii[f"bk_{rel}"], ii[f"be_{rel}"])
    w = np.zeros((35, 128), np.float32)
    w[0:16, 0:64] = Wq
    w[0:16, 64:128] = Wv
    w[16, 0:64] = 2 * We
    w[16, 64:128] = We
    w[17, 0:64] = bq + bk + 2 * be
    w[17, 64:128] = bv + be
    w[18:34, 0:64] = Wk
    return w


def pack_relation(xs, xd, src, dst, ea, D, ii, rel, batch_dst):
    """Host marshalling for one relation.

    Returns common schedule + per-core device arrays."""
    E = len(src)
    nbuck = (D + 127) // 128
    order = np.argsort(dst, kind="stable")
    src_s, dst_s, ea_s = src[order], dst[order], ea[order]
    core = dst_s // D
    loc = dst_s % D
    buck = loc // 128
    slot = loc % 128
    lin = (core * nbuck + buck) * 128 + slot
    deg = np.bincount(lin, minlength=NCORES * nbuck * 128) \
            .reshape(NCORES, nbuck, 128)
    starts = np.searchsorted(dst_s, dst_s, side="left")
    rank = np.arange(E) - starts

    # common per-bucket-position R (identity rounds): minimize
    # R + w*max_core(F).  Flex subtiles cost more than identity rounds
    # (extra one-hot DMA + per-subtile scatter matmul), so weight them and
    # prefer the larger R on ties.
    maxd = int(deg.max())
    bestT = np.full(nbuck, np.inf)
    bestR = np.zeros(nbuck, np.int64)
    for R in range(0, maxd + 1):
        lo = np.maximum(deg - R, 0).sum(-1)            # [NCORES, nbuck]
        Fk = (-(-lo // 128)).max(0)                    # [nbuck]
        T = R + 1.3 * Fk
        upd = T <= bestT
        bestT[upd] = T[upd]
        bestR[upd] = R

    # group buckets (sorted by R desc) into chunks of GRP
    border = np.argsort(-bestR, kind="stable")
    ngroups = (nbuck + GRP - 1) // GRP
    groups = []
    bucket_group = np.zeros(nbuck, np.int64)   # bucket -> group
    bucket_pos = np.zeros(nbuck, np.int64)     # bucket -> index in group
    bucket_Rs = np.zeros(nbuck, np.int64)      # bucket -> group R*
    xt_col = 0
    oh_blk = 0
    for g in range(ngroups):
        bks = border[g * GRP:(g + 1) * GRP]
        n_b = len(bks)
        Rs = int(bestR[bks].max()) if n_b else 0
        # leftovers recomputed at group R*
        lo2 = np.maximum(deg[:, bks, :] - Rs, 0).sum(-1)   # [NCORES, n_b]
        Fk = (-(-lo2 // 128)).max(0)                       # [n_b]
        Fs = int(Fk.max()) if n_b else 0
        flex = []
        oh_idx = {}
        for f in range(Fs):
            present = [(i, 0) for i in range(n_b) if Fk[i] > f]
            present = [(i, oh_blk + j) for j, (i, _) in enumerate(present)]
            for i, ob in present:
                oh_idx[(i, f)] = ob
            oh_blk += len(present)
            flex.append(present)
        bucket_group[bks] = g
        bucket_pos[bks] = np.arange(n_b)
        bucket_Rs[bks] = Rs
        groups.append({
            "n_b": n_b, "R": Rs, "F": Fs, "bks": bks, "Fk": Fk,
            "flex": flex, "oh_idx": oh_idx, "xt_off": xt_col,
        })
        xt_col += (Rs + Fs) * 512
    XC = max(xt_col, 512)
    OC = max(oh_blk * 128, 128)

    # per-edge destination column in xt (per core arrays share the schedule)
    # xt layout: [64, nsub*128]; subtile (group g, slab s, pos i) at column
    # block (xt_off/512)*8 + s*8 + i  (xt_off counts 512-col slab units).
    g_of = bucket_group[buck]
    i_of = bucket_pos[buck]
    Rs_of = bucket_Rs[buck]
    xoff_of = np.array([gr["xt_off"] for gr in groups], np.int64)[g_of]
    suboff_of = xoff_of // 512 * 8
    is_id = rank < Rs_of
    col = np.zeros(E, np.int64)
    col[is_id] = ((suboff_of[is_id] + rank[is_id] * 8 + i_of[is_id]) * 128
                  + slot[is_id])
    # flex: position among the bucket's leftover edges (dst-sorted order)
    lx = ~is_id
    lin_lx = lin[lx] // 128      # (core,bucket) linear id of leftover edges
    first = np.searchsorted(lin_lx, lin_lx, side="left")
    fpos = np.arange(lx.sum()) - first
    f_of = fpos // 128
    row = fpos % 128
    col[lx] = ((suboff_of[lx] + (Rs_of[lx] + f_of) * 8 + i_of[lx]) * 128
               + row)
    # oh block index for flex edges
    ohmap = np.full((nbuck, 32), -1, np.int64)
    for gr in groups:
        for (i, f), ob in gr["oh_idx"].items():
            ohmap[gr["bks"][i], f] = ob
    oh_of = np.zeros(E, np.int64)
    oh_of[lx] = ohmap[buck[lx], f_of]
    assert (oh_of[lx] >= 0).all()
    flexrow = np.zeros(E, np.int64)
    flexrow[lx] = row

    xsT = xs.astype(np.float32)
    xdT = xd.astype(np.float32)
    per_core = []
    cb = np.searchsorted(core, np.arange(NCORES + 1))
    for m in range(NCORES):
        s0, s1 = cb[m], cb[m + 1]
        c_src, c_dst = src_s[s0:s1], dst_s[s0:s1]
        c_ea, c_col = ea_s[s0:s1], col[s0:s1]
        c_lx = lx[s0:s1]
        c_oh = oh_of[s0:s1]
        c_fr = flexrow[s0:s1]
        c_slot = slot[s0:s1]
        xt = np.zeros((64, XC * 2), np.float32)
        xt[0:16, c_col] = xsT[c_src].T
        xt[16, c_col] = c_ea
        xt[17, c_col] = 1.0
        xt[18:34, c_col] = xdT[c_dst].T
        # flex one-hots: edge at (oh block, row) -> slot
        oh = np.zeros((128, OC), np.float32)
        oh[c_fr[c_lx], c_oh[c_lx] * 128 + c_slot[c_lx]] = 1.0
        # pa (skip lhsT, [32, nbuck*128]) + ohg (pooling one-hot)
        PC = ngroups * GRP * 128
        GC = ngroups * GRP * 128
        pa = np.zeros((32, PC), np.float32)
        ohg = np.zeros((128, GC), np.float32)
        for g, gr in enumerate(groups):
            for i, k in enumerate(gr["bks"]):
                base = m * D + k * 128
                w = min(128, D - k * 128)
                nodes = np.arange(base, base + w)
                cblk = (g * GRP + i) * 128
                pa[0:16, cblk:cblk + w] = xdT[nodes].T
                pa[16, cblk:cblk + w] = 1.0
                bt = batch_dst[nodes]
                ohg[np.arange(w), cblk + bt] = 1.0
        f8 = np.float16 if os.environ.get("KF16") else F8NP
        per_core.append({
            "xt": xt.astype(f8),
            "oh": oh.astype(f8),
            "pa": pa.astype(np.float16),
            "ohg": ohg.astype(f8),
        })

    w2 = np.zeros((64, 128), np.float32)
    w2[0:35] = _waug(ii, rel)
    ws4 = np.zeros((32, 64), np.float32)
    ws4[0:16] = ii[f"Wskip_{rel}"]
    ws4[16] = ii[f"bconv_{rel}"]
    nslabs = sum(gr["R"] + gr["F"] for gr in groups)
    return {
        "groups": groups, "XC": XC, "OC": OC, "ngroups": ngroups,
        "per_core": per_core, "w2": w2.astype(np.float16),
        "ws4": ws4.astype(np.float16), "nslabs": nslabs, "D": D,
        "nbuck": nbuck,
    }


def pack_all(ii):
    Dc, Db = NC_N // NCORES, NB_N // NCORES
    rel_c = pack_relation(ii["x_x"], ii["x_c"], ii["src_ac"].astype(np.int64),
                          ii["dst_ac"].astype(np.int64),
                          np.asarray(ii["ea_ac"])[:, 0], Dc, ii, "ac",
                          ii["batch_c"].astype(np.int64))
    rel_b = pack_relation(ii["x_c"], ii["x_b"], ii["src_cb"].astype(np.int64),
                          ii["dst_cb"].astype(np.int64),
                          np.asarray(ii["ea_cb"])[:, 0], Db, ii, "cb",
                          ii["batch_b"].astype(np.int64))

    cnt_c = np.bincount(ii["batch_c"].astype(np.int64), minlength=G)
    cnt_b = np.bincount(ii["batch_b"].astype(np.int64), minlength=G)
    recip2 = np.zeros((G, 128), np.float32)
    recip2[:, 0:64] = (1.0 / np.maximum(cnt_c, 1))[:, None]
    recip2[:, 64:128] = (1.0 / np.maximum(cnt_b, 1))[:, None]

    mlp = {
        "W1": ii["W1"].astype(np.float16), "W2": ii["W2"].astype(np.float16),
        "W3": ii["W3"].astype(np.float16),
        "Wout": ii["Wout"].astype(np.float16),
        "b1": np.asarray(ii["b1"], np.float32).reshape(64, 1),
        "b2": np.asarray(ii["b2"], np.float32).reshape(64, 1),
        "b3": np.asarray(ii["b3"], np.float32).reshape(64, 1),
        "bout": np.asarray(ii["bout"], np.float32).reshape(1, 1),
    }
    f8 = np.float16 if os.environ.get("KF16") else F8NP
    ident8 = np.eye(128, dtype=f8)
    ident16 = np.eye(128, dtype=np.float16)
    zl = np.zeros((1, 128), np.float16)
    zr = np.zeros((1, 512), np.float16)
    return {"c": rel_c, "b": rel_b, "recip2": recip2.astype(np.float16),
            "mlp": mlp, "ident8": ident8, "ident16": ident16,
            "zl": zl, "zr": zr}


def emulate(ii):
    """Numpy emulation of the device program (for packing validation)."""
    pk = pack_all(ii)
    pooled = np.zeros((G, 128), np.float64)
    for tag in ("c", "b"):
        rl = pk[tag]
        w2 = rl["w2"].astype(np.float32)
        ws4 = rl["ws4"].astype(np.float32)
        for m in range(NCORES):
            pc = rl["per_core"][m]
            xt = pc["xt"].astype(np.float32)
            oh = pc["oh"].astype(np.float32)
            pa = pc["pa"].astype(np.float32)
            ohg = pc["ohg"].astype(np.float32)
            for g, gr in enumerate(rl["groups"]):
                n_b, Rs, Fs = gr["n_b"], gr["R"], gr["F"]
                agg = np.zeros((128, n_b, 64), np.float32)
                for i in range(n_b):
                    cblk = (g * GRP + i) * 128
                    lhs = pa[0:17, cblk:cblk + 128]
                    agg[:, i, :] += lhs.T @ ws4[0:17]
                for s in range(Rs + Fs):
                    c0 = gr["xt_off"] * 2 + s * 1024
                    blkx = xt[:, c0:c0 + 1024]
                    sv = np.zeros((128, 8, 128), np.float32)
                    for i in range(8):
                        sv[:, i, :] = (blkx[:, i * 128:(i + 1) * 128].T @ w2)
                    gt = (1.0 / (1.0 + np.exp(-sv[:, :, 0:64]))) \
                        .astype(np.float16).astype(np.float32)
                    msg = (gt * sv[:, :, 64:128]).astype(np.float16) \
                        .astype(np.float32)
                    if s < Rs:
                        agg += msg[:, :n_b, :]
                    else:
                        for (i, ob) in gr["flex"][s - Rs]:
                            ohb = oh[:, ob * 128:(ob + 1) * 128]
                            agg[:, i, :] += ohb.T @ msg[:, i, :]
                h = np.maximum(agg, 0.0).astype(np.float16).astype(np.float32)
                off = 0 if tag == "c" else 64
                for i in range(n_b):
                    ohgb = ohg[:, (g * GRP + i) * 128:(g * GRP + i + 1) * 128]
                    pooled[:, off:off + 64] += ohgb.T @ h[:, i, :]
    mean = pooled * pk["recip2"].astype(np.float64)
    hcur = mean.T.astype(np.float32)          # [2H, G]
    mlp = pk["mlp"]
    for wk, bk in (("W1", "b1"), ("W2", "b2"), ("W3", "b3")):
        hcur = np.maximum(mlp[wk].astype(np.float32).T @ hcur + mlp[bk], 0.0)
    out = mlp["Wout"].astype(np.float32).T @ hcur + mlp["bout"]
    return out.reshape(G)


def kernel(**inputs):
    _install_ntff_shim()
    import concourse.bass as bass  # noqa: F401
    import concourse.bacc as bacc
    import concourse.mybir as mybir
    import concourse.tile as tile
    from concourse.bass_utils import run_bass_kernel_spmd

    F32 = mybir.dt.float32
    F16 = mybir.dt.float16
    FP8 = F16 if os.environ.get("KF16") else mybir.dt.float8e4
    AF = mybir.ActivationFunctionType
    OP = mybir.AluOpType

    ii = {k: np.asarray(v) for k, v in inputs.items()}
    pk = pack_all(ii)

    nc = bacc.Bacc("TRN2", target_bir_lowering=False, debug=False,
                   num_devices=NCORES)

    def din(name, arr0):
        return nc.dram_tensor(name, list(arr0.shape),
                              mybir.dt.from_np(arr0.dtype),
                              kind="ExternalInput")

    h = {}
    for tag in ("c", "b"):
        rl = pk[tag]
        pc0 = rl["per_core"][0]
        h[f"xt_{tag}"] = din(f"xt_{tag}", pc0["xt"])
        h[f"oh_{tag}"] = din(f"oh_{tag}", pc0["oh"])
        h[f"pa_{tag}"] = din(f"pa_{tag}", pc0["pa"])
        h[f"ohg_{tag}"] = din(f"ohg_{tag}", pc0["ohg"])
        h[f"w2_{tag}"] = din(f"w2_{tag}", rl["w2"])
        h[f"ws4_{tag}"] = din(f"ws4_{tag}", rl["ws4"])
    h["i8"] = din("i8", pk["ident8"])
    h["i16"] = din("i16", pk["ident16"])
    h["recip2"] = din("recip2", pk["recip2"])
    h["zl"] = din("zl", pk["zl"])
    h["zr"] = din("zr", pk["zr"])
    for k, v in pk["mlp"].items():
        h["mlp_" + k] = din("mlp_" + k, v)
    out_h = nc.dram_tensor("out", [1, G], F32, kind="ExternalOutput")

    with tile.TileContext(nc) as tc:
        with tc.tile_pool(name="const", bufs=1) as cp, \
             tc.tile_pool(name="stream", bufs=3) as sp, \
             tc.tile_pool(name="work", bufs=3) as wp, \
             tc.tile_pool(name="svp", bufs=1, space="PSUM") as svp, \
             tc.tile_pool(name="aggp", bufs=2, space="PSUM") as aggp, \
             tc.tile_pool(name="poolp", bufs=1, space="PSUM") as poolp, \
             tc.tile_pool(name="dram", bufs=1, space="DRAM") as dp:

            i8_t = cp.tile([128, 128], FP8, tag="i8")
            nc.sync.dma_start(i8_t[:], h["i8"].ap())
            i16_t = cp.tile([128, 128], F16, tag="i16")
            nc.sync.dma_start(i16_t[:], h["i16"].ap())
            recip_t = cp.tile([128, 128], F16, tag="recip2")
            nc.sync.dma_start(recip_t[:], h["recip2"].ap())
            zl = cp.tile([1, 128], F16, tag="zl")
            nc.sync.dma_start(zl[:], h["zl"].ap())
            zr = cp.tile([1, 512], F16, tag="zr")
            nc.sync.dma_start(zr[:], h["zr"].ap())

            sv = svp.tile([128, 2, 8, 128], F32, tag="sv")
            pooled = poolp.tile([128, 128], F32, tag="pooled")

            slab_ctr = [0]

            def relation(tag, col_off):
                rl = pk[tag]
                w2_t = cp.tile([64, 128], F16, tag=f"w2{tag}")
                nc.sync.dma_start(w2_t[:], h[f"w2_{tag}"].ap())
                ws4_t = cp.tile([32, 64], F16, tag=f"ws4{tag}")
                nc.sync.dma_start(ws4_t[:], h[f"ws4_{tag}"].ap())
                xt_v = h[f"xt_{tag}"].ap()
                oh_v = h[f"oh_{tag}"].ap()
                pa_v = h[f"pa_{tag}"].ap()
                ohg_v = h[f"ohg_{tag}"].ap()
                first_pool = [True]
                ngroups = rl["ngroups"]
                pa_w = GRP * 128
                for g, gr in enumerate(rl["groups"]):
                    n_b, Rs, Fs = gr["n_b"], gr["R"], gr["F"]
                    if n_b == 0:
                        continue
                    pa_t = sp.tile([32, pa_w], F16, name=f"pa{tag}{g}",
                                   tag="pa")
                    nc.sync.dma_start(pa_t[:],
                                      pa_v[:, g * pa_w:(g + 1) * pa_w])
                    ohg_t = sp.tile([128, GRP * 128], FP8,
                                    name=f"ohg{tag}{g}", tag="ohg")
                    nc.sync.dma_start(
                        ohg_t[:, 0:n_b * 128],
                        ohg_v[:, g * GRP * 128:g * GRP * 128 + n_b * 128])
                    agg = aggp.tile([128, GRP, 64], F32, name=f"agg{tag}{g}",
                                    tag="agg")
                    nc.tensor.matmul(agg[:, 0:n_b, :], zl[:],
                                     zr[:, 0:n_b * 64], start=True,
                                     stop=False, skip_group_check=True)
                    for i in range(n_b):
                        nc.tensor.matmul(
                            agg[:, i, :],
                            pa_t[0:17, i * 128:i * 128 + 128],
                            ws4_t[0:17, :],
                            start=False, stop=False, skip_group_check=True)
                    # count scatters to mark the last with stop=True
                    nsc = Rs + sum(len(p) for p in gr["flex"])
                    sci = [0]

                    def sc_flags():
                        sci[0] += 1
                        return {"start": False, "stop": sci[0] == nsc,
                                "skip_group_check": True}

                    for s in range(Rs + Fs):
                        hp = slab_ctr[0] % 2
                        slab_ctr[0] += 1
                        c0 = gr["xt_off"] * 2 + s * 1024
                        xt_t = sp.tile([64, 1024], FP8,
                                       name=f"xt{tag}{g}_{s}", tag="xt")
                        nc.sync.dma_start(xt_t[:], xt_v[:, c0:c0 + 1024])
                        flex = None
                        if s >= Rs:
                            flex = gr["flex"][s - Rs]
                            ob0 = flex[0][1]
                            obn = len(flex)
                            oh_t = sp.tile([128, GRP * 128], FP8,
                                           name=f"oh{tag}{g}_{s}", tag="ohf")
                            nc.sync.dma_start(
                                oh_t[:, 0:obn * 128],
                                oh_v[:, ob0 * 128:(ob0 + obn) * 128])
                            present = set(i for i, _ in flex)
                        for i in range(8):
                            if flex is not None and i not in present:
                                continue
                            nc.tensor.matmul(
                                sv[:, hp, i, :],
                                xt_t[:, i * 128:(i + 1) * 128],
                                w2_t[:],
                                start=True, stop=True)
                        gt = wp.tile([128, 8, 64], F16,
                                     name=f"gt{tag}{g}_{s}", tag="gt")
                        nc.scalar.activation(gt[:], sv[:, hp, :, 0:64],
                                             AF.Sigmoid)
                        msg = wp.tile([128, 8, 64], F16,
                                      name=f"msg{tag}{g}_{s}", tag="msg")
                        nc.vector.tensor_tensor(msg[:], gt[:],
                                                sv[:, hp, :, 64:128],
                                                op=OP.mult)
                        if flex is None:
                            nc.tensor.matmul(agg[:, 0:n_b, :], i8_t[:],
                                             msg[:, 0:n_b, :], **sc_flags())
                        else:
                            for j, (i, _) in enumerate(flex):
                                nc.tensor.matmul(
                                    agg[:, i, :],
                                    oh_t[:, j * 128:(j + 1) * 128],
                                    msg[:, i, :], **sc_flags())
                    h_sb = wp.tile([128, GRP, 64], F16, name=f"h{tag}{g}",
                                   tag="hsb")
                    nc.scalar.activation(h_sb[:, 0:n_b, :], agg[:, 0:n_b, :],
                                         AF.Relu)
                    for i in range(n_b):
                        nc.tensor.matmul(
                            pooled[:, col_off:col_off + 64],
                            ohg_t[:, i * 128:(i + 1) * 128],
                            h_sb[:, i, :],
                            start=first_pool[0],
                            stop=(g == ngroups - 1 and i == n_b - 1),
                            skip_group_check=True)
                        first_pool[0] = False

            if os.environ.get("KONLYREL") != "b":
                relation("c", 0)
            if os.environ.get("KONLYREL") != "c":
                relation("b", 64)

            # --- head: AllReduce pooled, divide by counts, transpose, MLP ---
            pooled_sb = wp.tile([128, 128], F32, tag="pooled_sb")
            nc.vector.tensor_copy(pooled_sb[:], pooled[:])
            bounce_in = dp.tile([128, 128], F32, tag="bounce_in")
            bounce_out = dp.tile([128, 128], F32, tag="bounce_out")
            nc.sync.dma_start(bounce_in[:], pooled_sb[:])
            if not os.environ.get("KNOCOLL"):
                nc.gpsimd.collective_compute(
                    "AllReduce", OP.add,
                    replica_groups=[list(range(NCORES))],
                    ins=[bounce_in.opt()], outs=[bounce_out.opt()])
                nc.sync.dma_start(pooled_sb[:], bounce_out[:])
            mean16 = wp.tile([128, 128], F16, tag="mean16")
            nc.vector.tensor_tensor(mean16[:], pooled_sb[:], recip_t[:],
                                    op=OP.mult)
            tps = aggp.tile([128, 128], F16, tag="agg")
            nc.tensor.transpose(tps[:], mean16[:], i16_t[:])
            mean_sb = wp.tile([128, 128], F16, tag="mean_sb")
            nc.vector.tensor_copy(mean_sb[:], tps[:])

            mw, mb = {}, {}
            for k in ("W1", "W2", "W3", "Wout"):
                mw[k] = cp.tile(list(pk["mlp"][k].shape), F16, name=f"mw{k}",
                                tag=f"mw{k}")
                nc.sync.dma_start(mw[k][:], h["mlp_" + k].ap())
            for k in ("b1", "b2", "b3", "bout"):
                mb[k] = cp.tile(list(pk["mlp"][k].shape), F32, name=f"mb{k}",
                                tag=f"mb{k}")
                nc.sync.dma_start(mb[k][:], h["mlp_" + k].ap())

            hcur = mean_sb
            for li, (wk, bk) in enumerate((("W1", "b1"), ("W2", "b2"),
                                           ("W3", "b3"))):
                ps = aggp.tile([64, G], F32, name=f"mlp{li}", tag="agg")
                nc.tensor.matmul(ps[:], mw[wk][:], hcur[:],
                                 start=True, stop=True)
                hn = wp.tile([64, G], F16, name=f"hn{li}", tag=f"hn{li}")
                nc.scalar.activation(hn[:], ps[:], AF.Relu, bias=mb[bk][:])
                hcur = hn
            ps_o = aggp.tile([1, G], F32, tag="agg")
            nc.tensor.matmul(ps_o[:], mw["Wout"][:], hcur[:],
                             start=True, stop=True)
            osb = wp.tile([1, G], F32, tag="osb")
            nc.scalar.activation(osb[:], ps_o[:], AF.Identity,
                                 bias=mb["bout"][:])
            nc.sync.dma_start(out_h.ap(), osb[:])

    nc.compile()

    in_maps = []
    for m in range(NCORES):
        im = {}
        for tag in ("c", "b"):
            rl = pk[tag]
            pc = rl["per_core"][m]
            im[f"xt_{tag}"] = pc["xt"]
            im[f"oh_{tag}"] = pc["oh"]
            im[f"pa_{tag}"] = pc["pa"]
            im[f"ohg_{tag}"] = pc["ohg"]
            im[f"w2_{tag}"] = rl["w2"]
            im[f"ws4_{tag}"] = rl["ws4"]
        im["i8"] = pk["ident8"]
        im["i16"] = pk["ident16"]
        im["recip2"] = pk["recip2"]
        im["zl"] = pk["zl"]
        im["zr"] = pk["zr"]
        for k, v in pk["mlp"].items():
            im["mlp_" + k] = v
        in_maps.append(im)

    trace = bool(os.environ.get("KERNEL_TRACE"))
    res = run_bass_kernel_spmd(nc, in_maps, core_ids=list(range(NCORES)),
                               trace=trace)
    global LAST_EXEC_NS
    LAST_EXEC_NS = res.exec_time_ns
    return res.results[0]["out"].reshape(G).astype(np.float32)


# revision 32
# speedup vs baseline: 1.5130x; 1.1672x over previous
"""Trainium2 Bass kernel for hetero-GNN (2x ResGatedGraphConv + segment-mean pooling + MLP).

Sharding: destination-node range per core; each core processes the edges whose
dst falls in its range. Host does index marshalling only; all model arithmetic
runs on device.

Device strategy ("degree rounds"):
  - dst nodes are grouped into 128-slot buckets; 8 buckets form a PSUM group
    whose aggregate [128 slots, 8*64] lives in one PSUM bank.
  - edges of a bucket are packed into R identity rounds (the j-th edge of
    slot p sits at row p of round j) plus <=F flex rounds (leftovers, with a
    host-built fp8 one-hot scatter matrix).
  - a slab = one round of each of the 8 buckets = 8 subtiles of 128 edges.
    Per slab: 8 fused matmuls [xt.T @ W_aug] (row-tiled concurrent pairs),
    one sigmoid (ACT), one gated multiply (DVE), and ONE identity-stationary
    matmul that scatter-adds all 8 subtiles into the group aggregate.
  - skip connection is folded into the same PSUM accumulation (a dummy zero
    matmul opens the accumulation group for the whole bank).
  - relu + pooling via per-bucket one-hot matmul into a [G, 2H] transposed
    pooled PSUM; AllReduce across 8 cores; small MLP head on device.
"""
import os
import sys
import types
import numpy as np
import ml_dtypes

F8NP = ml_dtypes.float8_e4m3fn

NCORES = 8
G = 128
H = 64
F = 16
NC_N = 100000
NB_N = 200000
GRP = 8          # buckets per PSUM group
LAST_EXEC_NS = None


def _install_ntff_shim():
    if 'antenv.axon_hooks' in sys.modules:
        return
    try:
        mod = types.ModuleType('antenv.axon_hooks')
        _h = [None]
        mod.set_axon_ntff_profile_hook = lambda h: _h.__setitem__(0, h)
        mod.get_axon_ntff_profile_hook = lambda: _h[0]
        sys.modules['antenv.axon_hooks'] = mod
        import antenv
        antenv.axon_hooks = mod
        from trn_agent_boot.trn_boot import _ntff_profile_via_ctypes
        mod.set_axon_ntff_profile_hook(
            _ntff_profile_via_ctypes('/opt/axon/libaxon_pjrt.so'))
    except Exception:
        pass


def _waug(ii, rel):
    Wq, Wv, Wk = ii[f"Wq_{rel}"], ii[f"Wv_{rel}"], ii[f"Wk_{rel}"]
    We = ii[f"We_{rel}"][0]
    bq, bv, bk, be = (ii[f"bq_{rel}"], ii[f"bv_{rel}"],
                      ii[f"bk_{rel}"], ii[f"be_{rel}"])
    w = np.zeros((35, 128), np.float32)
    w[0:16, 0:64] = Wq
    w[0:16, 64:128] = Wv
    w[16, 0:64] = 2 * We
    w[16, 64:128] = We
    w[17, 0:64] = bq + bk + 2 * be
    w[17, 64:128] = bv + be
    w[18:34, 0:64] = Wk
    return w


def pack_relation(xs, xd, src, dst, ea, D, ii, rel, batch_dst):
    """Host marshalling for one relation.

    Returns common schedule + per-core device arrays."""
    E = len(src)
    nbuck = (D + 127) // 128
    order = np.argsort(dst, kind="stable")
    src_s, dst_s, ea_s = src[order], dst[order], ea[order]
    core = dst_s // D
    loc = dst_s % D
    buck = loc // 128
    slot = loc % 128
    lin = (core * nbuck + buck) * 128 + slot
    deg = np.bincount(lin, minlength=NCORES * nbuck * 128) \
            .reshape(NCORES, nbuck, 128)
    starts = np.searchsorted(dst_s, dst_s, side="left")
    rank = np.arange(E) - starts

    # common per-bucket-position R (identity rounds): minimize
    # R + w*max_core(F).  Flex subtiles cost more than identity rounds
    # (extra one-hot DMA + per-subtile scatter matmul), so weight them and
    # prefer the larger R on ties.
    maxd = int(deg.max())
    bestT = np.full(nbuck, np.inf)
    bestR = np.zeros(nbuck, np.int64)
    for R in range(0, maxd + 1):
        lo = np.maximum(deg - R, 0).sum(-1)            # [NCORES, nbuck]
        Fk = (-(-lo // 128)).max(0)                    # [nbuck]
        T = R + 1.3 * Fk
        upd = T <= bestT
        bestT[upd] = T[upd]
        bestR[upd] = R

    # group buckets (sorted by R desc) into chunks of GRP
    border = np.argsort(-bestR, kind="stable")
    ngroups = (nbuck + GRP - 1) // GRP
    groups = []
    bucket_group = np.zeros(nbuck, np.int64)   # bucket -> group
    bucket_pos = np.zeros(nbuck, np.int64)     # bucket -> index in group
    bucket_Rs = np.zeros(nbuck, np.int64)      # bucket -> group R*
    xt_col = 0
    oh_blk = 0
    for g in range(ngroups):
        bks = border[g * GRP:(g + 1) * GRP]
        n_b = len(bks)
        Rs = int(bestR[bks].max()) if n_b else 0
        # leftovers recomputed at group R*
        lo2 = np.maximum(deg[:, bks, :] - Rs, 0).sum(-1)   # [NCORES, n_b]
        Fk = (-(-lo2 // 128)).max(0)                       # [n_b]
        Fs = int(Fk.max()) if n_b else 0
        flex = []
        oh_idx = {}
        for f in range(Fs):
            present = [(i, 0) for i in range(n_b) if Fk[i] > f]
            present = [(i, oh_blk + j) for j, (i, _) in enumerate(present)]
            for i, ob in present:
                oh_idx[(i, f)] = ob
            oh_blk += len(present)
            flex.append(present)
        bucket_group[bks] = g
        bucket_pos[bks] = np.arange(n_b)
        bucket_Rs[bks] = Rs
        groups.append({
            "n_b": n_b, "R": Rs, "F": Fs, "bks": bks, "Fk": Fk,
            "flex": flex, "oh_idx": oh_idx, "xt_off": xt_col,
        })
        xt_col += (Rs + Fs) * 512
    XC = max(xt_col, 512)
    OC = max(oh_blk * 128, 128)

    # per-edge destination column in xt (per core arrays share the schedule)
    # xt layout: [64, nsub*128]; subtile (group g, slab s, pos i) at column
    # block (xt_off/512)*8 + s*8 + i  (xt_off counts 512-col slab units).
    g_of = bucket_group[buck]
    i_of = bucket_pos[buck]
    Rs_of = bucket_Rs[buck]
    xoff_of = np.array([gr["xt_off"] for gr in groups], np.int64)[g_of]
    suboff_of = xoff_of // 512 * 8
    is_id = rank < Rs_of
    col = np.zeros(E, np.int64)
    col[is_id] = ((suboff_of[is_id] + rank[is_id] * 8 + i_of[is_id]) * 128
                  + slot[is_id])
    # flex: position among the bucket's leftover edges (dst-sorted order)
    lx = ~is_id
    lin_lx = lin[lx] // 128      # (core,bucket) linear id of leftover edges
    first = np.searchsorted(lin_lx, lin_lx, side="left")
    fpos = np.arange(lx.sum()) - first
    f_of = fpos // 128
    row = fpos % 128
    col[lx] = ((suboff_of[lx] + (Rs_of[lx] + f_of) * 8 + i_of[lx]) * 128
               + row)
    # oh block index for flex edges
    ohmap = np.full((nbuck, 32), -1, np.int64)
    for gr in groups:
        for (i, f), ob in gr["oh_idx"].items():
            ohmap[gr["bks"][i], f] = ob
    oh_of = np.zeros(E, np.int64)
    oh_of[lx] = ohmap[buck[lx], f_of]
    assert (oh_of[lx] >= 0).all()
    flexrow = np.zeros(E, np.int64)
    flexrow[lx] = row

    xsT = xs.astype(np.float32)
    xdT = xd.astype(np.float32)
    per_core = []
    cb = np.searchsorted(core, np.arange(NCORES + 1))
    for m in range(NCORES):
        s0, s1 = cb[m], cb[m + 1]
        c_src, c_dst = src_s[s0:s1], dst_s[s0:s1]
        c_ea, c_col = ea_s[s0:s1], col[s0:s1]
        c_lx = lx[s0:s1]
        c_oh = oh_of[s0:s1]
        c_fr = flexrow[s0:s1]
        c_slot = slot[s0:s1]
        xt = np.zeros((128, XC * 2), np.float32)
        xt[0:16, c_col] = xsT[c_src].T
        xt[16, c_col] = c_ea
        xt[17, c_col] = 1.0
        xt[18:34, c_col] = xdT[c_dst].T
        # flex one-hots: edge at (oh block, row) -> slot
        oh = np.zeros((128, OC), np.float32)
        oh[c_fr[c_lx], c_oh[c_lx] * 128 + c_slot[c_lx]] = 1.0
        # pa (skip lhsT, [32, nbuck*128]) + ohg (pooling one-hot)
        PC = ngroups * GRP * 128
        GC = ngroups * GRP * 128
        pa = np.zeros((32, PC), np.float32)
        ohg = np.zeros((128, GC), np.float32)
        for g, gr in enumerate(groups):
            for i, k in enumerate(gr["bks"]):
                base = m * D + k * 128
                w = min(128, D - k * 128)
                nodes = np.arange(base, base + w)
                cblk = (g * GRP + i) * 128
                pa[0:16, cblk:cblk + w] = xdT[nodes].T
                pa[16, cblk:cblk + w] = 1.0
                bt = batch_dst[nodes]
                ohg[np.arange(w), cblk + bt] = 1.0
        f8 = np.float16 if os.environ.get("KF16") else F8NP
        per_core.append({
            "xt": xt.astype(f8),
            "oh": oh.astype(f8),
            "pa": pa.astype(np.float16),
            "ohg": ohg.astype(f8),
        })

    w2 = np.zeros((128, 128), np.float32)
    w2[0:35] = _waug(ii, rel)
    ws4 = np.zeros((32, 64), np.float32)
    ws4[0:16] = ii[f"Wskip_{rel}"]
    ws4[16] = ii[f"bconv_{rel}"]
    nslabs = sum(gr["R"] + gr["F"] for gr in groups)
    return {
        "groups": groups, "XC": XC, "OC": OC, "ngroups": ngroups,
        "per_core": per_core, "w2": w2.astype(np.float16),
        "ws4": ws4.astype(np.float16), "nslabs": nslabs, "D": D,
        "nbuck": nbuck,
    }


def pack_all(ii):
    Dc, Db = NC_N // NCORES, NB_N // NCORES
    rel_c = pack_relation(ii["x_x"], ii["x_c"], ii["src_ac"].astype(np.int64),
                          ii["dst_ac"].astype(np.int64),
                          np.asarray(ii["ea_ac"])[:, 0], Dc, ii, "ac",
                          ii["batch_c"].astype(np.int64))
    rel_b = pack_relation(ii["x_c"], ii["x_b"], ii["src_cb"].astype(np.int64),
                          ii["dst_cb"].astype(np.int64),
                          np.asarray(ii["ea_cb"])[:, 0], Db, ii, "cb",
                          ii["batch_b"].astype(np.int64))

    cnt_c = np.bincount(ii["batch_c"].astype(np.int64), minlength=G)
    cnt_b = np.bincount(ii["batch_b"].astype(np.int64), minlength=G)
    recip2 = np.zeros((G, 128), np.float32)
    recip2[:, 0:64] = (1.0 / np.maximum(cnt_c, 1))[:, None]
    recip2[:, 64:128] = (1.0 / np.maximum(cnt_b, 1))[:, None]

    mlp = {
        "W1": ii["W1"].astype(np.float16), "W2": ii["W2"].astype(np.float16),
        "W3": ii["W3"].astype(np.float16),
        "Wout": ii["Wout"].astype(np.float16),
        "b1": np.asarray(ii["b1"], np.float32).reshape(64, 1),
        "b2": np.asarray(ii["b2"], np.float32).reshape(64, 1),
        "b3": np.asarray(ii["b3"], np.float32).reshape(64, 1),
        "bout": np.asarray(ii["bout"], np.float32).reshape(1, 1),
    }
    f8 = np.float16 if os.environ.get("KF16") else F8NP
    ident8 = np.eye(128, dtype=f8)
    ident16 = np.eye(128, dtype=np.float16)
    zl = np.zeros((1, 128), np.float16)
    zr = np.zeros((1, 512), np.float16)
    return {"c": rel_c, "b": rel_b, "recip2": recip2.astype(np.float16),
            "mlp": mlp, "ident8": ident8, "ident16": ident16,
            "zl": zl, "zr": zr}


def emulate(ii):
    """Numpy emulation of the device program (for packing validation)."""
    pk = pack_all(ii)
    pooled = np.zeros((G, 128), np.float64)
    for tag in ("c", "b"):
        rl = pk[tag]
        w2 = rl["w2"].astype(np.float32)
        ws4 = rl["ws4"].astype(np.float32)
        for m in range(NCORES):
            pc = rl["per_core"][m]
            xt = pc["xt"].astype(np.float32)
            oh = pc["oh"].astype(np.float32)
            pa = pc["pa"].astype(np.float32)
            ohg = pc["ohg"].astype(np.float32)
            for g, gr in enumerate(rl["groups"]):
                n_b, Rs, Fs = gr["n_b"], gr["R"], gr["F"]
                agg = np.zeros((128, n_b, 64), np.float32)
                for i in range(n_b):
                    cblk = (g * GRP + i) * 128
                    lhs = pa[0:17, cblk:cblk + 128]
                    agg[:, i, :] += lhs.T @ ws4[0:17]
                for s in range(Rs + Fs):
                    c0 = gr["xt_off"] * 2 + s * 1024
                    blkx = xt[:, c0:c0 + 1024]
                    sv = np.zeros((128, 8, 128), np.float32)
                    for i in range(8):
                        sv[:, i, :] = (blkx[:, i * 128:(i + 1) * 128].T @ w2)
                    gt = (1.0 / (1.0 + np.exp(-sv[:, :, 0:64]))) \
                        .astype(np.float16).astype(np.float32)
                    msg = (gt * sv[:, :, 64:128]).astype(np.float16) \
                        .astype(np.float32)
                    if s < Rs:
                        agg += msg[:, :n_b, :]
                    else:
                        for (i, ob) in gr["flex"][s - Rs]:
                            ohb = oh[:, ob * 128:(ob + 1) * 128]
                            agg[:, i, :] += ohb.T @ msg[:, i, :]
                h = np.maximum(agg, 0.0).astype(np.float16).astype(np.float32)
                off = 0 if tag == "c" else 64
                for i in range(n_b):
                    ohgb = ohg[:, (g * GRP + i) * 128:(g * GRP + i + 1) * 128]
                    pooled[:, off:off + 64] += ohgb.T @ h[:, i, :]
    mean = pooled * pk["recip2"].astype(np.float64)
    hcur = mean.T.astype(np.float32)          # [2H, G]
    mlp = pk["mlp"]
    for wk, bk in (("W1", "b1"), ("W2", "b2"), ("W3", "b3")):
        hcur = np.maximum(mlp[wk].astype(np.float32).T @ hcur + mlp[bk], 0.0)
    out = mlp["Wout"].astype(np.float32).T @ hcur + mlp["bout"]
    return out.reshape(G)


def kernel(**inputs):
    _install_ntff_shim()
    import concourse.bass as bass  # noqa: F401
    import concourse.bacc as bacc
    import concourse.mybir as mybir
    import concourse.tile as tile
    from concourse.bass_utils import run_bass_kernel_spmd

    F32 = mybir.dt.float32
    F16 = mybir.dt.float16
    FP8 = F16 if os.environ.get("KF16") else mybir.dt.float8e4
    AF = mybir.ActivationFunctionType
    OP = mybir.AluOpType

    ii = {k: np.asarray(v) for k, v in inputs.items()}
    pk = pack_all(ii)

    nc = bacc.Bacc("TRN2", target_bir_lowering=False, debug=False,
                   num_devices=NCORES)

    def din(name, arr0):
        return nc.dram_tensor(name, list(arr0.shape),
                              mybir.dt.from_np(arr0.dtype),
                              kind="ExternalInput")

    h = {}
    for tag in ("c", "b"):
        rl = pk[tag]
        pc0 = rl["per_core"][0]
        h[f"xt_{tag}"] = din(f"xt_{tag}", pc0["xt"])
        h[f"oh_{tag}"] = din(f"oh_{tag}", pc0["oh"])
        h[f"pa_{tag}"] = din(f"pa_{tag}", pc0["pa"])
        h[f"ohg_{tag}"] = din(f"ohg_{tag}", pc0["ohg"])
        h[f"w2_{tag}"] = din(f"w2_{tag}", rl["w2"])
        h[f"ws4_{tag}"] = din(f"ws4_{tag}", rl["ws4"])
    h["i8"] = din("i8", pk["ident8"])
    h["i16"] = din("i16", pk["ident16"])
    h["recip2"] = din("recip2", pk["recip2"])
    h["zl"] = din("zl", pk["zl"])
    h["zr"] = din("zr", pk["zr"])
    for k, v in pk["mlp"].items():
        h["mlp_" + k] = din("mlp_" + k, v)
    out_h = nc.dram_tensor("out", [1, G], F32, kind="ExternalOutput")

    with tile.TileContext(nc) as tc:
        with tc.tile_pool(name="const", bufs=1) as cp, \
             tc.tile_pool(name="stream", bufs=3) as sp, \
             tc.tile_pool(name="work", bufs=3) as wp, \
             tc.tile_pool(name="svp", bufs=1, space="PSUM") as svp, \
             tc.tile_pool(name="aggp", bufs=2, space="PSUM") as aggp, \
             tc.tile_pool(name="poolp", bufs=1, space="PSUM") as poolp, \
             tc.tile_pool(name="dram", bufs=1, space="DRAM") as dp:

            i8_t = cp.tile([128, 128], FP8, tag="i8")
            nc.sync.dma_start(i8_t[:], h["i8"].ap())
            i16_t = cp.tile([128, 128], F16, tag="i16")
            nc.sync.dma_start(i16_t[:], h["i16"].ap())
            recip_t = cp.tile([128, 128], F16, tag="recip2")
            nc.sync.dma_start(recip_t[:], h["recip2"].ap())
            zl = cp.tile([1, 128], F16, tag="zl")
            nc.sync.dma_start(zl[:], h["zl"].ap())
            zr = cp.tile([1, 512], F16, tag="zr")
            nc.sync.dma_start(zr[:], h["zr"].ap())

            sv = svp.tile([128, 2, 8, 128], F32, tag="sv")
            pooled = poolp.tile([128, 128], F32, tag="pooled")

            slab_ctr = [0]

            def relation(tag, col_off):
                rl = pk[tag]
                w2_t = cp.tile([128, 128], F16, tag=f"w2{tag}")
                nc.sync.dma_start(w2_t[:], h[f"w2_{tag}"].ap())
                ws4_t = cp.tile([32, 64], F16, tag=f"ws4{tag}")
                nc.sync.dma_start(ws4_t[:], h[f"ws4_{tag}"].ap())
                xt_v = h[f"xt_{tag}"].ap()
                oh_v = h[f"oh_{tag}"].ap()
                pa_v = h[f"pa_{tag}"].ap()
                ohg_v = h[f"ohg_{tag}"].ap()
                first_pool = [True]
                ngroups = rl["ngroups"]
                pa_w = GRP * 128
                for g, gr in enumerate(rl["groups"]):
                    n_b, Rs, Fs = gr["n_b"], gr["R"], gr["F"]
                    if n_b == 0:
                        continue
                    pa_t = sp.tile([32, pa_w], F16, name=f"pa{tag}{g}",
                                   tag="pa")
                    nc.sync.dma_start(pa_t[:],
                                      pa_v[:, g * pa_w:(g + 1) * pa_w])
                    ohg_t = sp.tile([128, GRP * 128], FP8,
                                    name=f"ohg{tag}{g}", tag="ohg")
                    nc.sync.dma_start(
                        ohg_t[:, 0:n_b * 128],
                        ohg_v[:, g * GRP * 128:g * GRP * 128 + n_b * 128])
                    agg = aggp.tile([128, GRP, 64], F32, name=f"agg{tag}{g}",
                                    tag="agg")
                    nc.tensor.matmul(agg[:, 0:n_b, :], zl[:],
                                     zr[:, 0:n_b * 64], start=True,
                                     stop=False, skip_group_check=True)
                    for i in range(n_b):
                        nc.tensor.matmul(
                            agg[:, i, :],
                            pa_t[0:17, i * 128:i * 128 + 128],
                            ws4_t[0:17, :],
                            start=False, stop=False, skip_group_check=True)
                    # count scatters to mark the last with stop=True
                    nsc = Rs + sum(len(p) for p in gr["flex"])
                    sci = [0]

                    def sc_flags():
                        sci[0] += 1
                        return {"start": False, "stop": sci[0] == nsc,
                                "skip_group_check": True}

                    pend = [None]   # deferred scatter emitter (SW pipeline)

                    def flush():
                        if pend[0] is not None:
                            pend[0]()
                            pend[0] = None

                    for s in range(Rs + Fs):
                        hp = slab_ctr[0] % 2
                        slab_ctr[0] += 1
                        c0 = gr["xt_off"] * 2 + s * 1024
                        xt_t = sp.tile([128, 1024], FP8,
                                       name=f"xt{tag}{g}_{s}", tag="xt")
                        nc.sync.dma_start(xt_t[:], xt_v[:, c0:c0 + 1024])
                        flex = None
                        if s >= Rs:
                            flex = gr["flex"][s - Rs]
                            ob0 = flex[0][1]
                            obn = len(flex)
                            oh_t = sp.tile([128, GRP * 128], FP8,
                                           name=f"oh{tag}{g}_{s}", tag="ohf")
                            nc.sync.dma_start(
                                oh_t[:, 0:obn * 128],
                                oh_v[:, ob0 * 128:(ob0 + obn) * 128])
                            present = set(i for i, _ in flex)
                        for i in range(8):
                            if flex is not None and i not in present:
                                continue
                            nc.tensor.matmul(
                                sv[:, hp, i, :],
                                xt_t[:, i * 128:(i + 1) * 128],
                                w2_t[:],
                                start=True, stop=True)
                        flush()  # scatter of slab s-1 runs behind our MM1s
                        gt = wp.tile([128, 8, 64], F16,
                                     name=f"gt{tag}{g}_{s}", tag="gt")
                        nc.scalar.activation(gt[:], sv[:, hp, :, 0:64],
                                             AF.Sigmoid)
                        msg = wp.tile([128, 8, 64], F16,
                                      name=f"msg{tag}{g}_{s}", tag="msg")
                        nc.vector.tensor_tensor(msg[:], gt[:],
                                                sv[:, hp, :, 64:128],
                                                op=OP.mult)

                        def mk(flex, oh_t, msg):
                            def emit():
                                if flex is None:
                                    nc.tensor.matmul(agg[:, 0:n_b, :],
                                                     i8_t[:],
                                                     msg[:, 0:n_b, :],
                                                     **sc_flags())
                                else:
                                    for j, (i, _) in enumerate(flex):
                                        nc.tensor.matmul(
                                            agg[:, i, :],
                                            oh_t[:, j * 128:(j + 1) * 128],
                                            msg[:, i, :], **sc_flags())
                            return emit

                        pend[0] = mk(flex, oh_t if flex is not None else None,
                                     msg)
                    flush()
                    h_sb = wp.tile([128, GRP, 64], F16, name=f"h{tag}{g}",
                                   tag="hsb")
                    nc.scalar.activation(h_sb[:, 0:n_b, :], agg[:, 0:n_b, :],
                                         AF.Relu)
                    for i in range(n_b):
                        nc.tensor.matmul(
                            pooled[:, col_off:col_off + 64],
                            ohg_t[:, i * 128:(i + 1) * 128],
                            h_sb[:, i, :],
                            start=first_pool[0],
                            stop=(g == ngroups - 1 and i == n_b - 1),
                            skip_group_check=True)
                        first_pool[0] = False

            if os.environ.get("KONLYREL") != "b":
                relation("c", 0)
            if os.environ.get("KONLYREL") != "c":
                relation("b", 64)

            # --- head: AllReduce pooled, divide by counts, transpose, MLP ---
            pooled_sb = wp.tile([128, 128], F32, tag="pooled_sb")
            nc.vector.tensor_copy(pooled_sb[:], pooled[:])
            bounce_in = dp.tile([128, 128], F32, tag="bounce_in")
            bounce_out = dp.tile([128, 128], F32, tag="bounce_out")
            nc.sync.dma_start(bounce_in[:], pooled_sb[:])
            if not os.environ.get("KNOCOLL"):
                nc.gpsimd.collective_compute(
                    "AllReduce", OP.add,
                    replica_groups=[list(range(NCORES))],
                    ins=[bounce_in.opt()], outs=[bounce_out.opt()])
                nc.sync.dma_start(pooled_sb[:], bounce_out[:])
            mean16 = wp.tile([128, 128], F16, tag="mean16")
            nc.vector.tensor_tensor(mean16[:], pooled_sb[:], recip_t[:],
                                    op=OP.mult)
            tps = aggp.tile([128, 128], F16, tag="agg")
            nc.tensor.transpose(tps[:], mean16[:], i16_t[:])
            mean_sb = wp.tile([128, 128], F16, tag="mean_sb")
            nc.vector.tensor_copy(mean_sb[:], tps[:])

            mw, mb = {}, {}
            for k in ("W1", "W2", "W3", "Wout"):
                mw[k] = cp.tile(list(pk["mlp"][k].shape), F16, name=f"mw{k}",
                                tag=f"mw{k}")
                nc.sync.dma_start(mw[k][:], h["mlp_" + k].ap())
            for k in ("b1", "b2", "b3", "bout"):
                mb[k] = cp.tile(list(pk["mlp"][k].shape), F32, name=f"mb{k}",
                                tag=f"mb{k}")
                nc.sync.dma_start(mb[k][:], h["mlp_" + k].ap())

            hcur = mean_sb
            for li, (wk, bk) in enumerate((("W1", "b1"), ("W2", "b2"),
                                           ("W3", "b3"))):
                ps = aggp.tile([64, G], F32, name=f"mlp{li}", tag="agg")
                nc.tensor.matmul(ps[:], mw[wk][:], hcur[:],
                                 start=True, stop=True)
                hn = wp.tile([64, G], F16, name=f"hn{li}", tag=f"hn{li}")
                nc.scalar.activation(hn[:], ps[:], AF.Relu, bias=mb[bk][:])
                hcur = hn
            ps_o = aggp.tile([1, G], F32, tag="agg")
            nc.tensor.matmul(ps_o[:], mw["Wout"][:], hcur[:],
                             start=True, stop=True)
            osb = wp.tile([1, G], F32, tag="osb")
            nc.scalar.activation(osb[:], ps_o[:], AF.Identity,
                                 bias=mb["bout"][:])
            nc.sync.dma_start(out_h.ap(), osb[:])

    nc.compile()

    in_maps = []
    for m in range(NCORES):
        im = {}
        for tag in ("c", "b"):
            rl = pk[tag]
            pc = rl["per_core"][m]
            im[f"xt_{tag}"] = pc["xt"]
            im[f"oh_{tag}"] = pc["oh"]
            im[f"pa_{tag}"] = pc["pa"]
            im[f"ohg_{tag}"] = pc["ohg"]
            im[f"w2_{tag}"] = rl["w2"]
            im[f"ws4_{tag}"] = rl["ws4"]
        im["i8"] = pk["ident8"]
        im["i16"] = pk["ident16"]
        im["recip2"] = pk["recip2"]
        im["zl"] = pk["zl"]
        im["zr"] = pk["zr"]
        for k, v in pk["mlp"].items():
            im["mlp_" + k] = v
        in_maps.append(im)

    trace = bool(os.environ.get("KERNEL_TRACE"))
    res = run_bass_kernel_spmd(nc, in_maps, core_ids=list(range(NCORES)),
                               trace=trace)
    global LAST_EXEC_NS
    LAST_EXEC_NS = res.exec_time_ns
    return res.results[0]["out"].reshape(G).astype(np.float32)


# revision 35
# speedup vs baseline: 2.7650x; 1.8275x over previous
"""Trainium2 Bass kernel for hetero-GNN (2x ResGatedGraphConv + segment-mean pooling + MLP).

Sharding: destination-node range per core; each core processes the edges whose
dst falls in its range. Host does index marshalling only; all model arithmetic
runs on device.

Device strategy ("degree rounds"):
  - dst nodes are grouped into 128-slot buckets; 8 buckets form a PSUM group
    whose aggregate [128 slots, 8*64] lives in one PSUM bank.
  - edges of a bucket are packed into R identity rounds (the j-th edge of
    slot p sits at row p of round j) plus <=F flex rounds (leftovers, with a
    host-built fp8 one-hot scatter matrix).
  - a slab = one round of each of the 8 buckets = 8 subtiles of 128 edges.
    Per slab: 8 fused matmuls [xt.T @ W_aug] (row-tiled concurrent pairs),
    one sigmoid (ACT), one gated multiply (DVE), and ONE identity-stationary
    matmul that scatter-adds all 8 subtiles into the group aggregate.
  - skip connection is folded into the same PSUM accumulation (a dummy zero
    matmul opens the accumulation group for the whole bank).
  - relu + pooling via per-bucket one-hot matmul into a [G, 2H] transposed
    pooled PSUM; AllReduce across 8 cores; small MLP head on device.
"""
import os
import sys
import types
import numpy as np
import ml_dtypes

F8NP = ml_dtypes.float8_e4m3fn

NCORES = 8
G = 128
H = 64
F = 16
NC_N = 100000
NB_N = 200000
GRP = 8          # buckets per PSUM group
LAST_EXEC_NS = None


def _install_ntff_shim():
    if 'antenv.axon_hooks' in sys.modules:
        return
    try:
        mod = types.ModuleType('antenv.axon_hooks')
        _h = [None]
        mod.set_axon_ntff_profile_hook = lambda h: _h.__setitem__(0, h)
        mod.get_axon_ntff_profile_hook = lambda: _h[0]
        sys.modules['antenv.axon_hooks'] = mod
        import antenv
        antenv.axon_hooks = mod
        from trn_agent_boot.trn_boot import _ntff_profile_via_ctypes
        mod.set_axon_ntff_profile_hook(
            _ntff_profile_via_ctypes('/opt/axon/libaxon_pjrt.so'))
    except Exception:
        pass


def _waug(ii, rel):
    Wq, Wv, Wk = ii[f"Wq_{rel}"], ii[f"Wv_{rel}"], ii[f"Wk_{rel}"]
    We = ii[f"We_{rel}"][0]
    bq, bv, bk, be = (ii[f"bq_{rel}"], ii[f"bv_{rel}"],
                      ii[f"bk_{rel}"], ii[f"be_{rel}"])
    w = np.zeros((35, 128), np.float32)
    w[0:16, 0:64] = Wq
    w[0:16, 64:128] = Wv
    w[16, 0:64] = 2 * We
    w[16, 64:128] = We
    w[17, 0:64] = bq + bk + 2 * be
    w[17, 64:128] = bv + be
    w[18:34, 0:64] = Wk
    return w


def pack_relation(xs, xd, src, dst, ea, D, ii, rel, batch_dst):
    """Host marshalling for one relation.

    Returns common schedule + per-core device arrays."""
    E = len(src)
    nbuck = (D + 127) // 128
    order = np.argsort(dst, kind="stable")
    src_s, dst_s, ea_s = src[order], dst[order], ea[order]
    core = dst_s // D
    loc = dst_s % D
    buck = loc // 128
    slot = loc % 128
    lin = (core * nbuck + buck) * 128 + slot
    deg = np.bincount(lin, minlength=NCORES * nbuck * 128) \
            .reshape(NCORES, nbuck, 128)
    starts = np.searchsorted(dst_s, dst_s, side="left")
    rank = np.arange(E) - starts

    # common per-bucket-position R (identity rounds): minimize
    # R + w*max_core(F).  Flex subtiles cost more than identity rounds
    # (extra one-hot DMA + per-subtile scatter matmul), so weight them and
    # prefer the larger R on ties.
    maxd = int(deg.max())
    bestT = np.full(nbuck, np.inf)
    bestR = np.zeros(nbuck, np.int64)
    for R in range(0, maxd + 1):
        lo = np.maximum(deg - R, 0).sum(-1)            # [NCORES, nbuck]
        Fk = (-(-lo // 128)).max(0)                    # [nbuck]
        T = R + 1.3 * Fk
        upd = T <= bestT
        bestT[upd] = T[upd]
        bestR[upd] = R

    # group buckets (sorted by R desc) into chunks of GRP
    border = np.argsort(-bestR, kind="stable")
    ngroups = (nbuck + GRP - 1) // GRP
    groups = []
    bucket_group = np.zeros(nbuck, np.int64)   # bucket -> group
    bucket_pos = np.zeros(nbuck, np.int64)     # bucket -> index in group
    bucket_Rs = np.zeros(nbuck, np.int64)      # bucket -> group R*
    xt_col = 0
    oh_blk = 0
    for g in range(ngroups):
        bks = border[g * GRP:(g + 1) * GRP]
        n_b = len(bks)
        Rs = int(bestR[bks].max()) if n_b else 0
        # leftovers recomputed at group R*
        lo2 = np.maximum(deg[:, bks, :] - Rs, 0).sum(-1)   # [NCORES, n_b]
        Fk = (-(-lo2 // 128)).max(0)                       # [n_b]
        Fs = int(Fk.max()) if n_b else 0
        flex = []
        oh_idx = {}
        for f in range(Fs):
            present = [(i, 0) for i in range(n_b) if Fk[i] > f]
            present = [(i, oh_blk + j) for j, (i, _) in enumerate(present)]
            for i, ob in present:
                oh_idx[(i, f)] = ob
            oh_blk += len(present)
            flex.append(present)
        bucket_group[bks] = g
        bucket_pos[bks] = np.arange(n_b)
        bucket_Rs[bks] = Rs
        groups.append({
            "n_b": n_b, "R": Rs, "F": Fs, "bks": bks, "Fk": Fk,
            "flex": flex, "oh_idx": oh_idx, "xt_off": xt_col,
        })
        xt_col += (Rs + Fs) * 512
    XC = max(xt_col, 512)
    OC = max(oh_blk * 128, 128)

    # per-edge destination column in xt (per core arrays share the schedule)
    # xt layout: [64, nsub*128]; subtile (group g, slab s, pos i) at column
    # block (xt_off/512)*8 + s*8 + i  (xt_off counts 512-col slab units).
    g_of = bucket_group[buck]
    i_of = bucket_pos[buck]
    Rs_of = bucket_Rs[buck]
    xoff_of = np.array([gr["xt_off"] for gr in groups], np.int64)[g_of]
    suboff_of = xoff_of // 512 * 8
    is_id = rank < Rs_of
    col = np.zeros(E, np.int64)
    col[is_id] = ((suboff_of[is_id] + rank[is_id] * 8 + i_of[is_id]) * 128
                  + slot[is_id])
    # flex: position among the bucket's leftover edges (dst-sorted order)
    lx = ~is_id
    lin_lx = lin[lx] // 128      # (core,bucket) linear id of leftover edges
    first = np.searchsorted(lin_lx, lin_lx, side="left")
    fpos = np.arange(lx.sum()) - first
    f_of = fpos // 128
    row = fpos % 128
    col[lx] = ((suboff_of[lx] + (Rs_of[lx] + f_of) * 8 + i_of[lx]) * 128
               + row)
    # oh block index for flex edges
    ohmap = np.full((nbuck, 32), -1, np.int64)
    for gr in groups:
        for (i, f), ob in gr["oh_idx"].items():
            ohmap[gr["bks"][i], f] = ob
    oh_of = np.zeros(E, np.int64)
    oh_of[lx] = ohmap[buck[lx], f_of]
    assert (oh_of[lx] >= 0).all()
    flexrow = np.zeros(E, np.int64)
    flexrow[lx] = row

    xsT = xs.astype(np.float32)
    xdT = xd.astype(np.float32)
    per_core = []
    cb = np.searchsorted(core, np.arange(NCORES + 1))
    for m in range(NCORES):
        s0, s1 = cb[m], cb[m + 1]
        c_src, c_dst = src_s[s0:s1], dst_s[s0:s1]
        c_ea, c_col = ea_s[s0:s1], col[s0:s1]
        c_lx = lx[s0:s1]
        c_oh = oh_of[s0:s1]
        c_fr = flexrow[s0:s1]
        c_slot = slot[s0:s1]
        xt = np.zeros((128, XC * 2), np.float32)
        xt[0:16, c_col] = xsT[c_src].T
        xt[16, c_col] = c_ea
        xt[17, c_col] = 1.0
        xt[18:34, c_col] = xdT[c_dst].T
        # flex one-hots: edge at (oh block, row) -> slot
        oh = np.zeros((128, OC), np.float32)
        oh[c_fr[c_lx], c_oh[c_lx] * 128 + c_slot[c_lx]] = 1.0
        # pa (skip lhsT, [32, nbuck*128]) + ohg (pooling one-hot)
        PC = ngroups * GRP * 128
        GC = ngroups * GRP * 128
        pa = np.zeros((32, PC), np.float32)
        ohg = np.zeros((128, GC), np.float32)
        for g, gr in enumerate(groups):
            for i, k in enumerate(gr["bks"]):
                base = m * D + k * 128
                w = min(128, D - k * 128)
                nodes = np.arange(base, base + w)
                cblk = (g * GRP + i) * 128
                pa[0:16, cblk:cblk + w] = xdT[nodes].T
                pa[16, cblk:cblk + w] = 1.0
                bt = batch_dst[nodes]
                ohg[np.arange(w), cblk + bt] = 1.0
        f8 = np.float16 if os.environ.get("KF16") else F8NP
        per_core.append({
            "xt": xt.astype(f8),
            "oh": oh.astype(f8),
            "pa": pa.astype(np.float16),
            "ohg": ohg.astype(f8),
        })

    w2 = np.zeros((128, 128), np.float32)
    w2[0:35] = _waug(ii, rel)
    ws4 = np.zeros((32, 64), np.float32)
    ws4[0:16] = ii[f"Wskip_{rel}"]
    ws4[16] = ii[f"bconv_{rel}"]
    nslabs = sum(gr["R"] + gr["F"] for gr in groups)
    return {
        "groups": groups, "XC": XC, "OC": OC, "ngroups": ngroups,
        "per_core": per_core, "w2": w2.astype(np.float16),
        "ws4": ws4.astype(np.float16), "nslabs": nslabs, "D": D,
        "nbuck": nbuck,
    }


def pack_all(ii):
    Dc, Db = NC_N // NCORES, NB_N // NCORES
    rel_c = pack_relation(ii["x_x"], ii["x_c"], ii["src_ac"].astype(np.int64),
                          ii["dst_ac"].astype(np.int64),
                          np.asarray(ii["ea_ac"])[:, 0], Dc, ii, "ac",
                          ii["batch_c"].astype(np.int64))
    rel_b = pack_relation(ii["x_c"], ii["x_b"], ii["src_cb"].astype(np.int64),
                          ii["dst_cb"].astype(np.int64),
                          np.asarray(ii["ea_cb"])[:, 0], Db, ii, "cb",
                          ii["batch_b"].astype(np.int64))

    cnt_c = np.bincount(ii["batch_c"].astype(np.int64), minlength=G)
    cnt_b = np.bincount(ii["batch_b"].astype(np.int64), minlength=G)
    recip2 = np.zeros((G, 128), np.float32)
    recip2[:, 0:64] = (1.0 / np.maximum(cnt_c, 1))[:, None]
    recip2[:, 64:128] = (1.0 / np.maximum(cnt_b, 1))[:, None]

    mlp = {
        "W1": ii["W1"].astype(np.float16), "W2": ii["W2"].astype(np.float16),
        "W3": ii["W3"].astype(np.float16),
        "Wout": ii["Wout"].astype(np.float16),
        "b1": np.asarray(ii["b1"], np.float32).reshape(64, 1),
        "b2": np.asarray(ii["b2"], np.float32).reshape(64, 1),
        "b3": np.asarray(ii["b3"], np.float32).reshape(64, 1),
        "bout": np.asarray(ii["bout"], np.float32).reshape(1, 1),
    }
    f8 = np.float16 if os.environ.get("KF16") else F8NP
    ident8 = np.eye(128, dtype=f8)
    ident16 = np.eye(128, dtype=np.float16)
    zl = np.zeros((1, 128), np.float16)
    zr = np.zeros((1, 512), np.float16)
    return {"c": rel_c, "b": rel_b, "recip2": recip2.astype(np.float16),
            "mlp": mlp, "ident8": ident8, "ident16": ident16,
            "zl": zl, "zr": zr}


def emulate(ii):
    """Numpy emulation of the device program (for packing validation)."""
    pk = pack_all(ii)
    pooled = np.zeros((G, 128), np.float64)
    for tag in ("c", "b"):
        rl = pk[tag]
        w2 = rl["w2"].astype(np.float32)
        ws4 = rl["ws4"].astype(np.float32)
        for m in range(NCORES):
            pc = rl["per_core"][m]
            xt = pc["xt"].astype(np.float32)
            oh = pc["oh"].astype(np.float32)
            pa = pc["pa"].astype(np.float32)
            ohg = pc["ohg"].astype(np.float32)
            for g, gr in enumerate(rl["groups"]):
                n_b, Rs, Fs = gr["n_b"], gr["R"], gr["F"]
                agg = np.zeros((128, n_b, 64), np.float32)
                for i in range(n_b):
                    cblk = (g * GRP + i) * 128
                    lhs = pa[0:17, cblk:cblk + 128]
                    agg[:, i, :] += lhs.T @ ws4[0:17]
                for s in range(Rs + Fs):
                    c0 = gr["xt_off"] * 2 + s * 1024
                    blkx = xt[:, c0:c0 + 1024]
                    sv = np.zeros((128, 8, 128), np.float32)
                    for i in range(8):
                        sv[:, i, :] = (blkx[:, i * 128:(i + 1) * 128].T @ w2)
                    gt = (1.0 / (1.0 + np.exp(-sv[:, :, 0:64]))) \
                        .astype(np.float16).astype(np.float32)
                    msg = (gt * sv[:, :, 64:128]).astype(np.float16) \
                        .astype(np.float32)
                    if s < Rs:
                        agg += msg[:, :n_b, :]
                    else:
                        for (i, ob) in gr["flex"][s - Rs]:
                            ohb = oh[:, ob * 128:(ob + 1) * 128]
                            agg[:, i, :] += ohb.T @ msg[:, i, :]
                h = np.maximum(agg, 0.0).astype(np.float16).astype(np.float32)
                off = 0 if tag == "c" else 64
                for i in range(n_b):
                    ohgb = ohg[:, (g * GRP + i) * 128:(g * GRP + i + 1) * 128]
                    pooled[:, off:off + 64] += ohgb.T @ h[:, i, :]
    mean = pooled * pk["recip2"].astype(np.float64)
    hcur = mean.T.astype(np.float32)          # [2H, G]
    mlp = pk["mlp"]
    for wk, bk in (("W1", "b1"), ("W2", "b2"), ("W3", "b3")):
        hcur = np.maximum(mlp[wk].astype(np.float32).T @ hcur + mlp[bk], 0.0)
    out = mlp["Wout"].astype(np.float32).T @ hcur + mlp["bout"]
    return out.reshape(G)


def kernel(**inputs):
    _install_ntff_shim()
    import concourse.bass as bass  # noqa: F401
    import concourse.bacc as bacc
    import concourse.mybir as mybir
    import concourse.tile as tile
    from concourse.bass_utils import run_bass_kernel_spmd

    F32 = mybir.dt.float32
    F16 = mybir.dt.float16
    FP8 = F16 if os.environ.get("KF16") else mybir.dt.float8e4
    AF = mybir.ActivationFunctionType
    OP = mybir.AluOpType

    ii = {k: np.asarray(v) for k, v in inputs.items()}
    pk = pack_all(ii)

    nc = bacc.Bacc("TRN2", target_bir_lowering=False, debug=False,
                   num_devices=NCORES)

    def din(name, arr0):
        return nc.dram_tensor(name, list(arr0.shape),
                              mybir.dt.from_np(arr0.dtype),
                              kind="ExternalInput")

    h = {}
    for tag in ("c", "b"):
        rl = pk[tag]
        pc0 = rl["per_core"][0]
        h[f"xt_{tag}"] = din(f"xt_{tag}", pc0["xt"])
        h[f"oh_{tag}"] = din(f"oh_{tag}", pc0["oh"])
        h[f"pa_{tag}"] = din(f"pa_{tag}", pc0["pa"])
        h[f"ohg_{tag}"] = din(f"ohg_{tag}", pc0["ohg"])
        h[f"w2_{tag}"] = din(f"w2_{tag}", rl["w2"])
        h[f"ws4_{tag}"] = din(f"ws4_{tag}", rl["ws4"])
    h["i8"] = din("i8", pk["ident8"])
    h["i16"] = din("i16", pk["ident16"])
    h["recip2"] = din("recip2", pk["recip2"])
    h["zl"] = din("zl", pk["zl"])
    h["zr"] = din("zr", pk["zr"])
    for k, v in pk["mlp"].items():
        h["mlp_" + k] = din("mlp_" + k, v)
    out_h = nc.dram_tensor("out", [1, G], F32, kind="ExternalOutput")

    with tile.TileContext(nc) as tc:
        with tc.tile_pool(name="const", bufs=1) as cp, \
             tc.tile_pool(name="stream", bufs=3) as sp, \
             tc.tile_pool(name="work", bufs=3) as wp, \
             tc.tile_pool(name="svp", bufs=2, space="PSUM") as svp, \
             tc.tile_pool(name="aggp", bufs=2, space="PSUM") as aggp, \
             tc.tile_pool(name="poolp", bufs=1, space="PSUM") as poolp, \
             tc.tile_pool(name="dram", bufs=1, space="DRAM") as dp:

            i8_t = cp.tile([128, 128], FP8, tag="i8")
            nc.sync.dma_start(i8_t[:], h["i8"].ap())
            i16_t = cp.tile([128, 128], F16, tag="i16")
            nc.sync.dma_start(i16_t[:], h["i16"].ap())
            recip_t = cp.tile([128, 128], F16, tag="recip2")
            nc.sync.dma_start(recip_t[:], h["recip2"].ap())
            zl = cp.tile([1, 128], F16, tag="zl")
            nc.sync.dma_start(zl[:], h["zl"].ap())
            zr = cp.tile([1, 512], F16, tag="zr")
            nc.sync.dma_start(zr[:], h["zr"].ap())

            pooled = poolp.tile([128, 128], F32, tag="pooled")

            slab_ctr = [0]

            def relation(tag, col_off):
                rl = pk[tag]
                w2_t = cp.tile([128, 128], F16, tag=f"w2{tag}")
                nc.sync.dma_start(w2_t[:], h[f"w2_{tag}"].ap())
                ws4_t = cp.tile([32, 64], F16, tag=f"ws4{tag}")
                nc.sync.dma_start(ws4_t[:], h[f"ws4_{tag}"].ap())
                xt_v = h[f"xt_{tag}"].ap()
                oh_v = h[f"oh_{tag}"].ap()
                pa_v = h[f"pa_{tag}"].ap()
                ohg_v = h[f"ohg_{tag}"].ap()
                first_pool = [True]
                ngroups = rl["ngroups"]
                pa_w = GRP * 128
                for g, gr in enumerate(rl["groups"]):
                    n_b, Rs, Fs = gr["n_b"], gr["R"], gr["F"]
                    if n_b == 0:
                        continue
                    pa_t = sp.tile([32, pa_w], F16, name=f"pa{tag}{g}",
                                   tag="pa")
                    nc.sync.dma_start(pa_t[:],
                                      pa_v[:, g * pa_w:(g + 1) * pa_w])
                    ohg_t = sp.tile([128, GRP * 128], FP8,
                                    name=f"ohg{tag}{g}", tag="ohg")
                    nc.sync.dma_start(
                        ohg_t[:, 0:n_b * 128],
                        ohg_v[:, g * GRP * 128:g * GRP * 128 + n_b * 128])
                    agg = aggp.tile([128, GRP, 64], F32, name=f"agg{tag}{g}",
                                    tag="agg")
                    nc.tensor.matmul(agg[:, 0:n_b, :], zl[:],
                                     zr[:, 0:n_b * 64], start=True,
                                     stop=False, skip_group_check=True)
                    for i in range(n_b):
                        nc.tensor.matmul(
                            agg[:, i, :],
                            pa_t[0:17, i * 128:i * 128 + 128],
                            ws4_t[0:17, :],
                            start=False, stop=False, skip_group_check=True)
                    # count scatters to mark the last with stop=True
                    nsc = Rs + sum(len(p) for p in gr["flex"])
                    sci = [0]

                    def sc_flags():
                        sci[0] += 1
                        return {"start": False, "stop": sci[0] == nsc,
                                "skip_group_check": True}

                    pend = [None]   # deferred scatter emitter (SW pipeline)

                    def flush():
                        if pend[0] is not None:
                            pend[0]()
                            pend[0] = None

                    for s in range(Rs + Fs):
                        slab_ctr[0] += 1
                        c0 = gr["xt_off"] * 2 + s * 1024
                        xt_t = sp.tile([128, 1024], FP8,
                                       name=f"xt{tag}{g}_{s}", tag="xt")
                        nc.sync.dma_start(xt_t[:], xt_v[:, c0:c0 + 1024])
                        flex = None
                        if s >= Rs:
                            flex = gr["flex"][s - Rs]
                            ob0 = flex[0][1]
                            obn = len(flex)
                            oh_t = sp.tile([128, GRP * 128], FP8,
                                           name=f"oh{tag}{g}_{s}", tag="ohf")
                            nc.sync.dma_start(
                                oh_t[:, 0:obn * 128],
                                oh_v[:, ob0 * 128:(ob0 + obn) * 128])
                            present = set(i for i, _ in flex)
                        svk = svp.tile([128, 8, 128], F32,
                                       name=f"sv{tag}{g}_{s}", tag="sv")
                        for i in range(8):
                            if flex is not None and i not in present:
                                continue
                            nc.tensor.matmul(
                                svk[:, i, :],
                                xt_t[:, i * 128:(i + 1) * 128],
                                w2_t[:],
                                start=True, stop=True)
                        flush()  # scatter of slab s-1 runs behind our MM1s
                        gt = wp.tile([128, 8, 64], F16,
                                     name=f"gt{tag}{g}_{s}", tag="gt")
                        nc.scalar.activation(gt[:], svk[:, :, 0:64],
                                             AF.Sigmoid)
                        msg = wp.tile([128, 8, 64], F16,
                                      name=f"msg{tag}{g}_{s}", tag="msg")
                        nc.vector.tensor_tensor(msg[:], gt[:],
                                                svk[:, :, 64:128],
                                                op=OP.mult)

                        def mk(flex, oh_t, msg):
                            def emit():
                                if flex is None:
                                    nc.tensor.matmul(agg[:, 0:n_b, :],
                                                     i8_t[:],
                                                     msg[:, 0:n_b, :],
                                                     **sc_flags())
                                else:
                                    for j, (i, _) in enumerate(flex):
                                        nc.tensor.matmul(
                                            agg[:, i, :],
                                            oh_t[:, j * 128:(j + 1) * 128],
                                            msg[:, i, :], **sc_flags())
                            return emit

                        pend[0] = mk(flex, oh_t if flex is not None else None,
                                     msg)
                    flush()
                    h_sb = wp.tile([128, GRP, 64], F16, name=f"h{tag}{g}",
                                   tag="hsb")
                    nc.scalar.activation(h_sb[:, 0:n_b, :], agg[:, 0:n_b, :],
                                         AF.Relu)
                    for i in range(n_b):
                        nc.tensor.matmul(
                            pooled[:, col_off:col_off + 64],
                            ohg_t[:, i * 128:(i + 1) * 128],
                            h_sb[:, i, :],
                            start=first_pool[0],
                            stop=(g == ngroups - 1 and i == n_b - 1),
                            skip_group_check=True)
                        first_pool[0] = False

            if os.environ.get("KONLYREL") != "b":
                relation("c", 0)
            if os.environ.get("KONLYREL") != "c":
                relation("b", 64)

            # --- head: AllReduce pooled, divide by counts, transpose, MLP ---
            pooled_sb = wp.tile([128, 128], F32, tag="pooled_sb")
            nc.vector.tensor_copy(pooled_sb[:], pooled[:])
            bounce_in = dp.tile([128, 128], F32, tag="bounce_in")
            bounce_out = dp.tile([128, 128], F32, tag="bounce_out")
            nc.sync.dma_start(bounce_in[:], pooled_sb[:])
            if not os.environ.get("KNOCOLL"):
                nc.gpsimd.collective_compute(
                    "AllReduce", OP.add,
                    replica_groups=[list(range(NCORES))],
                    ins=[bounce_in.opt()], outs=[bounce_out.opt()])
                nc.sync.dma_start(pooled_sb[:], bounce_out[:])
            mean16 = wp.tile([128, 128], F16, tag="mean16")
            nc.vector.tensor_tensor(mean16[:], pooled_sb[:], recip_t[:],
                                    op=OP.mult)
            tps = aggp.tile([128, 128], F16, tag="agg")
            nc.tensor.transpose(tps[:], mean16[:], i16_t[:])
            mean_sb = wp.tile([128, 128], F16, tag="mean_sb")
            nc.vector.tensor_copy(mean_sb[:], tps[:])

            mw, mb = {}, {}
            for k in ("W1", "W2", "W3", "Wout"):
                mw[k] = cp.tile(list(pk["mlp"][k].shape), F16, name=f"mw{k}",
                                tag=f"mw{k}")
                nc.sync.dma_start(mw[k][:], h["mlp_" + k].ap())
            for k in ("b1", "b2", "b3", "bout"):
                mb[k] = cp.tile(list(pk["mlp"][k].shape), F32, name=f"mb{k}",
                                tag=f"mb{k}")
                nc.sync.dma_start(mb[k][:], h["mlp_" + k].ap())

            hcur = mean_sb
            for li, (wk, bk) in enumerate((("W1", "b1"), ("W2", "b2"),
                                           ("W3", "b3"))):
                ps = aggp.tile([64, G], F32, name=f"mlp{li}", tag="agg")
                nc.tensor.matmul(ps[:], mw[wk][:], hcur[:],
                                 start=True, stop=True)
                hn = wp.tile([64, G], F16, name=f"hn{li}", tag=f"hn{li}")
                nc.scalar.activation(hn[:], ps[:], AF.Relu, bias=mb[bk][:])
                hcur = hn
            ps_o = aggp.tile([1, G], F32, tag="agg")
            nc.tensor.matmul(ps_o[:], mw["Wout"][:], hcur[:],
                             start=True, stop=True)
            osb = wp.tile([1, G], F32, tag="osb")
            nc.scalar.activation(osb[:], ps_o[:], AF.Identity,
                                 bias=mb["bout"][:])
            nc.sync.dma_start(out_h.ap(), osb[:])

    nc.compile()

    in_maps = []
    for m in range(NCORES):
        im = {}
        for tag in ("c", "b"):
            rl = pk[tag]
            pc = rl["per_core"][m]
            im[f"xt_{tag}"] = pc["xt"]
            im[f"oh_{tag}"] = pc["oh"]
            im[f"pa_{tag}"] = pc["pa"]
            im[f"ohg_{tag}"] = pc["ohg"]
            im[f"w2_{tag}"] = rl["w2"]
            im[f"ws4_{tag}"] = rl["ws4"]
        im["i8"] = pk["ident8"]
        im["i16"] = pk["ident16"]
        im["recip2"] = pk["recip2"]
        im["zl"] = pk["zl"]
        im["zr"] = pk["zr"]
        for k, v in pk["mlp"].items():
            im["mlp_" + k] = v
        in_maps.append(im)

    trace = bool(os.environ.get("KERNEL_TRACE"))
    res = run_bass_kernel_spmd(nc, in_maps, core_ids=list(range(NCORES)),
                               trace=trace)
    global LAST_EXEC_NS
    LAST_EXEC_NS = res.exec_time_ns
    return res.results[0]["out"].reshape(G).astype(np.float32)


# revision 36
# speedup vs baseline: 2.9965x; 1.0837x over previous
"""Trainium2 Bass kernel for hetero-GNN (2x ResGatedGraphConv + segment-mean pooling + MLP).

Sharding: destination-node range per core; each core processes the edges whose
dst falls in its range. Host does index marshalling only; all model arithmetic
runs on device.

Device strategy ("degree rounds"):
  - dst nodes are grouped into 128-slot buckets; 8 buckets form a PSUM group
    whose aggregate [128 slots, 8*64] lives in one PSUM bank.
  - edges of a bucket are packed into R identity rounds (the j-th edge of
    slot p sits at row p of round j) plus <=F flex rounds (leftovers, with a
    host-built fp8 one-hot scatter matrix).
  - a slab = one round of each of the 8 buckets = 8 subtiles of 128 edges.
    Per slab: 8 fused matmuls [xt.T @ W_aug] (row-tiled concurrent pairs),
    one sigmoid (ACT), one gated multiply (DVE), and ONE identity-stationary
    matmul that scatter-adds all 8 subtiles into the group aggregate.
  - skip connection is folded into the same PSUM accumulation (a dummy zero
    matmul opens the accumulation group for the whole bank).
  - relu + pooling via per-bucket one-hot matmul into a [G, 2H] transposed
    pooled PSUM; AllReduce across 8 cores; small MLP head on device.
"""
import os
import sys
import types
import numpy as np
import ml_dtypes

F8NP = ml_dtypes.float8_e4m3fn

NCORES = 8
G = 128
H = 64
F = 16
NC_N = 100000
NB_N = 200000
GRP = 8          # buckets per PSUM group
LAST_EXEC_NS = None


def _install_ntff_shim():
    if 'antenv.axon_hooks' in sys.modules:
        return
    try:
        mod = types.ModuleType('antenv.axon_hooks')
        _h = [None]
        mod.set_axon_ntff_profile_hook = lambda h: _h.__setitem__(0, h)
        mod.get_axon_ntff_profile_hook = lambda: _h[0]
        sys.modules['antenv.axon_hooks'] = mod
        import antenv
        antenv.axon_hooks = mod
        from trn_agent_boot.trn_boot import _ntff_profile_via_ctypes
        mod.set_axon_ntff_profile_hook(
            _ntff_profile_via_ctypes('/opt/axon/libaxon_pjrt.so'))
    except Exception:
        pass


def _waug(ii, rel):
    Wq, Wv, Wk = ii[f"Wq_{rel}"], ii[f"Wv_{rel}"], ii[f"Wk_{rel}"]
    We = ii[f"We_{rel}"][0]
    bq, bv, bk, be = (ii[f"bq_{rel}"], ii[f"bv_{rel}"],
                      ii[f"bk_{rel}"], ii[f"be_{rel}"])
    w = np.zeros((35, 128), np.float32)
    w[0:16, 0:64] = Wq
    w[0:16, 64:128] = Wv
    w[16, 0:64] = 2 * We
    w[16, 64:128] = We
    w[17, 0:64] = bq + bk + 2 * be
    w[17, 64:128] = bv + be
    w[18:34, 0:64] = Wk
    return w


def pack_relation(xs, xd, src, dst, ea, D, ii, rel, batch_dst):
    """Host marshalling for one relation.

    Returns common schedule + per-core device arrays."""
    E = len(src)
    nbuck = (D + 127) // 128
    order = np.argsort(dst, kind="stable")
    src_s, dst_s, ea_s = src[order], dst[order], ea[order]
    core = dst_s // D
    loc = dst_s % D
    buck = loc // 128
    slot = loc % 128
    lin = (core * nbuck + buck) * 128 + slot
    deg = np.bincount(lin, minlength=NCORES * nbuck * 128) \
            .reshape(NCORES, nbuck, 128)
    starts = np.searchsorted(dst_s, dst_s, side="left")
    rank = np.arange(E) - starts

    # common per-bucket-position R (identity rounds): minimize
    # R + w*max_core(F).  Flex subtiles cost more than identity rounds
    # (extra one-hot DMA + per-subtile scatter matmul), so weight them and
    # prefer the larger R on ties.
    maxd = int(deg.max())
    bestT = np.full(nbuck, np.inf)
    bestR = np.zeros(nbuck, np.int64)
    for R in range(0, maxd + 1):
        lo = np.maximum(deg - R, 0).sum(-1)            # [NCORES, nbuck]
        Fk = (-(-lo // 128)).max(0)                    # [nbuck]
        T = R + 1.3 * Fk
        upd = T <= bestT
        bestT[upd] = T[upd]
        bestR[upd] = R

    # group buckets (sorted by R desc) into chunks of GRP
    border = np.argsort(-bestR, kind="stable")
    ngroups = (nbuck + GRP - 1) // GRP
    groups = []
    bucket_group = np.zeros(nbuck, np.int64)   # bucket -> group
    bucket_pos = np.zeros(nbuck, np.int64)     # bucket -> index in group
    bucket_Rs = np.zeros(nbuck, np.int64)      # bucket -> group R*
    xt_col = 0
    oh_blk = 0
    for g in range(ngroups):
        bks = border[g * GRP:(g + 1) * GRP]
        n_b = len(bks)
        Rs = int(bestR[bks].max()) if n_b else 0
        # leftovers recomputed at group R*
        lo2 = np.maximum(deg[:, bks, :] - Rs, 0).sum(-1)   # [NCORES, n_b]
        Fk = (-(-lo2 // 128)).max(0)                       # [n_b]
        Fs = int(Fk.max()) if n_b else 0
        flex = []
        oh_idx = {}
        for f in range(Fs):
            present = [(i, 0) for i in range(n_b) if Fk[i] > f]
            present = [(i, oh_blk + j) for j, (i, _) in enumerate(present)]
            for i, ob in present:
                oh_idx[(i, f)] = ob
            oh_blk += len(present)
            flex.append(present)
        bucket_group[bks] = g
        bucket_pos[bks] = np.arange(n_b)
        bucket_Rs[bks] = Rs
        groups.append({
            "n_b": n_b, "R": Rs, "F": Fs, "bks": bks, "Fk": Fk,
            "flex": flex, "oh_idx": oh_idx, "xt_off": xt_col,
        })
        xt_col += (Rs + Fs) * 512
    XC = max(xt_col, 512)
    OC = max(oh_blk * 128, 128)

    # per-edge destination column in xt (per core arrays share the schedule)
    # xt layout: [64, nsub*128]; subtile (group g, slab s, pos i) at column
    # block (xt_off/512)*8 + s*8 + i  (xt_off counts 512-col slab units).
    g_of = bucket_group[buck]
    i_of = bucket_pos[buck]
    Rs_of = bucket_Rs[buck]
    xoff_of = np.array([gr["xt_off"] for gr in groups], np.int64)[g_of]
    suboff_of = xoff_of // 512 * 8
    is_id = rank < Rs_of
    col = np.zeros(E, np.int64)
    col[is_id] = ((suboff_of[is_id] + rank[is_id] * 8 + i_of[is_id]) * 128
                  + slot[is_id])
    # flex: position among the bucket's leftover edges (dst-sorted order)
    lx = ~is_id
    lin_lx = lin[lx] // 128      # (core,bucket) linear id of leftover edges
    first = np.searchsorted(lin_lx, lin_lx, side="left")
    fpos = np.arange(lx.sum()) - first
    f_of = fpos // 128
    row = fpos % 128
    col[lx] = ((suboff_of[lx] + (Rs_of[lx] + f_of) * 8 + i_of[lx]) * 128
               + row)
    # oh block index for flex edges
    ohmap = np.full((nbuck, 32), -1, np.int64)
    for gr in groups:
        for (i, f), ob in gr["oh_idx"].items():
            ohmap[gr["bks"][i], f] = ob
    oh_of = np.zeros(E, np.int64)
    oh_of[lx] = ohmap[buck[lx], f_of]
    assert (oh_of[lx] >= 0).all()
    flexrow = np.zeros(E, np.int64)
    flexrow[lx] = row

    xsT = xs.astype(np.float32)
    xdT = xd.astype(np.float32)
    per_core = []
    cb = np.searchsorted(core, np.arange(NCORES + 1))
    for m in range(NCORES):
        s0, s1 = cb[m], cb[m + 1]
        c_src, c_dst = src_s[s0:s1], dst_s[s0:s1]
        c_ea, c_col = ea_s[s0:s1], col[s0:s1]
        c_lx = lx[s0:s1]
        c_oh = oh_of[s0:s1]
        c_fr = flexrow[s0:s1]
        c_slot = slot[s0:s1]
        xt = np.zeros((128, XC * 2), np.float32)
        xt[0:16, c_col] = xsT[c_src].T
        xt[16, c_col] = c_ea
        xt[17, c_col] = 1.0
        xt[18:34, c_col] = xdT[c_dst].T
        # flex one-hots: edge at (oh block, row) -> slot
        oh = np.zeros((128, OC), np.float32)
        oh[c_fr[c_lx], c_oh[c_lx] * 128 + c_slot[c_lx]] = 1.0
        # pa (skip lhsT, [32, nbuck*128]) + ohg (pooling one-hot)
        PC = ngroups * GRP * 128
        GC = ngroups * GRP * 128
        pa = np.zeros((32, PC), np.float32)
        ohg = np.zeros((128, GC), np.float32)
        for g, gr in enumerate(groups):
            for i, k in enumerate(gr["bks"]):
                base = m * D + k * 128
                w = min(128, D - k * 128)
                nodes = np.arange(base, base + w)
                cblk = (g * GRP + i) * 128
                pa[0:16, cblk:cblk + w] = xdT[nodes].T
                pa[16, cblk:cblk + w] = 1.0
                bt = batch_dst[nodes]
                ohg[np.arange(w), cblk + bt] = 1.0
        f8 = np.float16 if os.environ.get("KF16") else F8NP
        per_core.append({
            "xt": xt.astype(f8),
            "oh": oh.astype(f8),
            "pa": pa.astype(np.float16),
            "ohg": ohg.astype(f8),
        })

    w2 = np.zeros((128, 128), np.float32)
    w2[0:35] = _waug(ii, rel)
    ws4 = np.zeros((32, 64), np.float32)
    ws4[0:16] = ii[f"Wskip_{rel}"]
    ws4[16] = ii[f"bconv_{rel}"]
    nslabs = sum(gr["R"] + gr["F"] for gr in groups)
    return {
        "groups": groups, "XC": XC, "OC": OC, "ngroups": ngroups,
        "per_core": per_core, "w2": w2.astype(np.float16),
        "ws4": ws4.astype(np.float16), "nslabs": nslabs, "D": D,
        "nbuck": nbuck,
    }


def pack_all(ii):
    Dc, Db = NC_N // NCORES, NB_N // NCORES
    rel_c = pack_relation(ii["x_x"], ii["x_c"], ii["src_ac"].astype(np.int64),
                          ii["dst_ac"].astype(np.int64),
                          np.asarray(ii["ea_ac"])[:, 0], Dc, ii, "ac",
                          ii["batch_c"].astype(np.int64))
    rel_b = pack_relation(ii["x_c"], ii["x_b"], ii["src_cb"].astype(np.int64),
                          ii["dst_cb"].astype(np.int64),
                          np.asarray(ii["ea_cb"])[:, 0], Db, ii, "cb",
                          ii["batch_b"].astype(np.int64))

    cnt_c = np.bincount(ii["batch_c"].astype(np.int64), minlength=G)
    cnt_b = np.bincount(ii["batch_b"].astype(np.int64), minlength=G)
    recip2 = np.zeros((G, 128), np.float32)
    recip2[:, 0:64] = (1.0 / np.maximum(cnt_c, 1))[:, None]
    recip2[:, 64:128] = (1.0 / np.maximum(cnt_b, 1))[:, None]

    mlp = {
        "W1": ii["W1"].astype(np.float16), "W2": ii["W2"].astype(np.float16),
        "W3": ii["W3"].astype(np.float16),
        "Wout": ii["Wout"].astype(np.float16),
        "b1": np.asarray(ii["b1"], np.float32).reshape(64, 1),
        "b2": np.asarray(ii["b2"], np.float32).reshape(64, 1),
        "b3": np.asarray(ii["b3"], np.float32).reshape(64, 1),
        "bout": np.asarray(ii["bout"], np.float32).reshape(1, 1),
    }
    f8 = np.float16 if os.environ.get("KF16") else F8NP
    ident8 = np.eye(128, dtype=f8)
    ident16 = np.eye(128, dtype=np.float16)
    zl = np.zeros((1, 128), np.float16)
    zr = np.zeros((1, 512), np.float16)
    return {"c": rel_c, "b": rel_b, "recip2": recip2.astype(np.float16),
            "mlp": mlp, "ident8": ident8, "ident16": ident16,
            "zl": zl, "zr": zr}


def emulate(ii):
    """Numpy emulation of the device program (for packing validation)."""
    pk = pack_all(ii)
    pooled = np.zeros((G, 128), np.float64)
    for tag in ("c", "b"):
        rl = pk[tag]
        w2 = rl["w2"].astype(np.float32)
        ws4 = rl["ws4"].astype(np.float32)
        for m in range(NCORES):
            pc = rl["per_core"][m]
            xt = pc["xt"].astype(np.float32)
            oh = pc["oh"].astype(np.float32)
            pa = pc["pa"].astype(np.float32)
            ohg = pc["ohg"].astype(np.float32)
            for g, gr in enumerate(rl["groups"]):
                n_b, Rs, Fs = gr["n_b"], gr["R"], gr["F"]
                agg = np.zeros((128, n_b, 64), np.float32)
                for i in range(n_b):
                    cblk = (g * GRP + i) * 128
                    lhs = pa[0:17, cblk:cblk + 128]
                    agg[:, i, :] += lhs.T @ ws4[0:17]
                for s in range(Rs + Fs):
                    c0 = gr["xt_off"] * 2 + s * 1024
                    blkx = xt[:, c0:c0 + 1024]
                    sv = np.zeros((128, 8, 128), np.float32)
                    for i in range(8):
                        sv[:, i, :] = (blkx[:, i * 128:(i + 1) * 128].T @ w2)
                    gt = (1.0 / (1.0 + np.exp(-sv[:, :, 0:64]))) \
                        .astype(np.float16).astype(np.float32)
                    msg = (gt * sv[:, :, 64:128]).astype(np.float16) \
                        .astype(np.float32)
                    if s < Rs:
                        agg += msg[:, :n_b, :]
                    else:
                        for (i, ob) in gr["flex"][s - Rs]:
                            ohb = oh[:, ob * 128:(ob + 1) * 128]
                            agg[:, i, :] += ohb.T @ msg[:, i, :]
                h = np.maximum(agg, 0.0).astype(np.float16).astype(np.float32)
                off = 0 if tag == "c" else 64
                for i in range(n_b):
                    ohgb = ohg[:, (g * GRP + i) * 128:(g * GRP + i + 1) * 128]
                    pooled[:, off:off + 64] += ohgb.T @ h[:, i, :]
    mean = pooled * pk["recip2"].astype(np.float64)
    hcur = mean.T.astype(np.float32)          # [2H, G]
    mlp = pk["mlp"]
    for wk, bk in (("W1", "b1"), ("W2", "b2"), ("W3", "b3")):
        hcur = np.maximum(mlp[wk].astype(np.float32).T @ hcur + mlp[bk], 0.0)
    out = mlp["Wout"].astype(np.float32).T @ hcur + mlp["bout"]
    return out.reshape(G)


def kernel(**inputs):
    _install_ntff_shim()
    import concourse.bass as bass  # noqa: F401
    import concourse.bacc as bacc
    import concourse.mybir as mybir
    import concourse.tile as tile
    from concourse.bass_utils import run_bass_kernel_spmd

    F32 = mybir.dt.float32
    F16 = mybir.dt.float16
    FP8 = F16 if os.environ.get("KF16") else mybir.dt.float8e4
    AF = mybir.ActivationFunctionType
    OP = mybir.AluOpType

    ii = {k: np.asarray(v) for k, v in inputs.items()}
    pk = pack_all(ii)

    nc = bacc.Bacc("TRN2", target_bir_lowering=False, debug=False,
                   num_devices=NCORES)

    def din(name, arr0):
        return nc.dram_tensor(name, list(arr0.shape),
                              mybir.dt.from_np(arr0.dtype),
                              kind="ExternalInput")

    h = {}
    for tag in ("c", "b"):
        rl = pk[tag]
        pc0 = rl["per_core"][0]
        h[f"xt_{tag}"] = din(f"xt_{tag}", pc0["xt"])
        h[f"oh_{tag}"] = din(f"oh_{tag}", pc0["oh"])
        h[f"pa_{tag}"] = din(f"pa_{tag}", pc0["pa"])
        h[f"ohg_{tag}"] = din(f"ohg_{tag}", pc0["ohg"])
        h[f"w2_{tag}"] = din(f"w2_{tag}", rl["w2"])
        h[f"ws4_{tag}"] = din(f"ws4_{tag}", rl["ws4"])
    h["i8"] = din("i8", pk["ident8"])
    h["i16"] = din("i16", pk["ident16"])
    h["recip2"] = din("recip2", pk["recip2"])
    h["zl"] = din("zl", pk["zl"])
    h["zr"] = din("zr", pk["zr"])
    for k, v in pk["mlp"].items():
        h["mlp_" + k] = din("mlp_" + k, v)
    out_h = nc.dram_tensor("out", [1, G], F32, kind="ExternalOutput")

    with tile.TileContext(nc) as tc:
        with tc.tile_pool(name="const", bufs=1) as cp, \
             tc.tile_pool(name="stream", bufs=3) as sp, \
             tc.tile_pool(name="work", bufs=3) as wp, \
             tc.tile_pool(name="svp", bufs=2, space="PSUM") as svp, \
             tc.tile_pool(name="aggp", bufs=2, space="PSUM") as aggp, \
             tc.tile_pool(name="poolp", bufs=1, space="PSUM") as poolp, \
             tc.tile_pool(name="dram", bufs=1, space="DRAM") as dp:

            i8_t = cp.tile([128, 128], FP8, tag="i8")
            nc.sync.dma_start(i8_t[:], h["i8"].ap())
            i16_t = cp.tile([128, 128], F16, tag="i16")
            nc.sync.dma_start(i16_t[:], h["i16"].ap())
            recip_t = cp.tile([128, 128], F16, tag="recip2")
            nc.sync.dma_start(recip_t[:], h["recip2"].ap())
            zl = cp.tile([1, 128], F16, tag="zl")
            nc.sync.dma_start(zl[:], h["zl"].ap())
            zr = cp.tile([1, 512], F16, tag="zr")
            nc.sync.dma_start(zr[:], h["zr"].ap())

            pooled = poolp.tile([128, 128], F32, tag="pooled")

            slab_ctr = [0]

            def relation(tag, col_off):
                rl = pk[tag]
                w2_t = cp.tile([128, 128], F16, tag=f"w2{tag}")
                nc.sync.dma_start(w2_t[:], h[f"w2_{tag}"].ap())
                ws4_t = cp.tile([32, 64], F16, tag=f"ws4{tag}")
                nc.sync.dma_start(ws4_t[:], h[f"ws4_{tag}"].ap())
                xt_v = h[f"xt_{tag}"].ap()
                oh_v = h[f"oh_{tag}"].ap()
                pa_v = h[f"pa_{tag}"].ap()
                ohg_v = h[f"ohg_{tag}"].ap()
                first_pool = [True]
                ngroups = rl["ngroups"]
                pa_w = GRP * 128
                jobs = []
                for g, gr in enumerate(rl["groups"]):
                    if gr["n_b"] == 0:
                        continue
                    for s in range(gr["R"] + gr["F"]):
                        jobs.append((g, gr, s))
                last_g = jobs[-1][0]
                # software pipeline: scatter(slab k) emits after MM1s(k+1);
                # relu(group) emits with its last scatter; pooling(group)
                # defers one more slab so PE never waits on ACT.
                pend_sc = [None]
                pend_pool = []     # list of [due_idx, fn]
                cur = [0]

                def flush_sc():
                    if pend_sc[0] is not None:
                        pend_sc[0]()
                        pend_sc[0] = None

                def flush_pool():
                    while pend_pool and pend_pool[0][0] <= cur[0]:
                        pend_pool.pop(0)[1]()

                def mk_pool(g, gr, agg, ohg_t):
                    n_b = gr["n_b"]
                    h_sb = wp.tile([128, GRP, 64], F16, name=f"h{tag}{g}",
                                   tag="hsb")
                    nc.scalar.activation(h_sb[:, 0:n_b, :],
                                         agg[:, 0:n_b, :], AF.Relu)

                    def pool():
                        for i in range(n_b):
                            nc.tensor.matmul(
                                pooled[:, col_off:col_off + 64],
                                ohg_t[:, i * 128:(i + 1) * 128],
                                h_sb[:, i, :],
                                start=first_pool[0],
                                stop=(g == last_g and i == n_b - 1),
                                skip_group_check=True)
                            first_pool[0] = False
                    pend_pool.append([cur[0] + 1, pool])

                st = {}
                xt2 = [None, 0]
                for idx, (g, gr, s) in enumerate(jobs):
                    cur[0] = idx
                    n_b, Rs, Fs = gr["n_b"], gr["R"], gr["F"]
                    nsl = Rs + Fs
                    if s == 0:
                        pa_t = sp.tile([32, pa_w], F16, name=f"pa{tag}{g}",
                                       tag="pa")
                        nc.sync.dma_start(pa_t[:],
                                          pa_v[:, g * pa_w:(g + 1) * pa_w])
                        ohg_t = sp.tile([128, GRP * 128], FP8,
                                        name=f"ohg{tag}{g}", tag="ohg")
                        nc.sync.dma_start(
                            ohg_t[:, 0:n_b * 128],
                            ohg_v[:, g * GRP * 128:
                                  g * GRP * 128 + n_b * 128])
                        agg = aggp.tile([128, GRP, 64], F32,
                                        name=f"agg{tag}{g}", tag="agg")
                        nc.tensor.matmul(agg[:, 0:n_b, :], zl[:],
                                         zr[:, 0:n_b * 64], start=True,
                                         stop=False, skip_group_check=True)
                        for i in range(n_b):
                            nc.tensor.matmul(
                                agg[:, i, :],
                                pa_t[0:17, i * 128:i * 128 + 128],
                                ws4_t[0:17, :],
                                start=False, stop=False,
                                skip_group_check=True)
                        nsc = Rs + sum(len(p) for p in gr["flex"])
                        st[g] = {"agg": agg, "ohg_t": ohg_t, "sci": [0],
                                 "nsc": nsc}
                    sg = st[g]
                    agg, ohg_t = sg["agg"], sg["ohg_t"]

                    def sc_flags(sg=sg):
                        sg["sci"][0] += 1
                        return {"start": False,
                                "stop": sg["sci"][0] == sg["nsc"],
                                "skip_group_check": True}

                    # xt DMA batched over slab pairs (within the group)
                    if s % 2 == 0:
                        wcols = min(2, nsl - s) * 1024
                        c0 = gr["xt_off"] * 2 + s * 1024
                        xt2[0] = sp.tile([128, 2048], FP8,
                                         name=f"xt{tag}{g}_{s}", tag="xt")
                        nc.sync.dma_start(xt2[0][:, 0:wcols],
                                          xt_v[:, c0:c0 + wcols])
                    xtsl = xt2[0][:, (s % 2) * 1024:(s % 2) * 1024 + 1024]
                    flex = None
                    if s >= Rs:
                        flex = gr["flex"][s - Rs]
                        ob0 = flex[0][1]
                        obn = len(flex)
                        oh_t = sp.tile([128, GRP * 128], FP8,
                                       name=f"oh{tag}{g}_{s}", tag="ohf")
                        nc.sync.dma_start(
                            oh_t[:, 0:obn * 128],
                            oh_v[:, ob0 * 128:(ob0 + obn) * 128])
                        present = set(i for i, _ in flex)
                    svk = svp.tile([128, 8, 128], F32,
                                   name=f"sv{tag}{g}_{s}", tag="sv")
                    for i in range(8):
                        if flex is not None and i not in present:
                            continue
                        nc.tensor.matmul(
                            svk[:, i, :],
                            xtsl[:, i * 128:(i + 1) * 128],
                            w2_t[:],
                            start=True, stop=True)
                    flush_sc()   # scatter of slab idx-1 runs behind our MM1s
                    flush_pool()
                    gt = wp.tile([128, 8, 64], F16,
                                 name=f"gt{tag}{g}_{s}", tag="gt")
                    nc.scalar.activation(gt[:], svk[:, :, 0:64], AF.Sigmoid)
                    msg = wp.tile([128, 8, 64], F16,
                                  name=f"msg{tag}{g}_{s}", tag="msg")
                    nc.vector.tensor_tensor(msg[:], gt[:], svk[:, :, 64:128],
                                            op=OP.mult)

                    def mk(flex, oh_t, msg, g=g, gr=gr, agg=agg,
                           ohg_t=ohg_t, is_last=(s == nsl - 1),
                           sc_flags=sc_flags):
                        def emit():
                            if flex is None:
                                nc.tensor.matmul(agg[:, 0:gr["n_b"], :],
                                                 i8_t[:],
                                                 msg[:, 0:gr["n_b"], :],
                                                 **sc_flags())
                            else:
                                for j, (i, _) in enumerate(flex):
                                    nc.tensor.matmul(
                                        agg[:, i, :],
                                        oh_t[:, j * 128:(j + 1) * 128],
                                        msg[:, i, :], **sc_flags())
                            if is_last:
                                mk_pool(g, gr, agg, ohg_t)
                        return emit

                    pend_sc[0] = mk(flex,
                                    oh_t if flex is not None else None, msg)
                cur[0] += 1
                flush_sc()
                cur[0] += 1
                flush_pool()
                while pend_pool:
                    pend_pool.pop(0)[1]()

            if os.environ.get("KONLYREL") != "b":
                relation("c", 0)
            if os.environ.get("KONLYREL") != "c":
                relation("b", 64)

            # --- head: AllReduce pooled, divide by counts, transpose, MLP ---
            pooled_sb = wp.tile([128, 128], F32, tag="pooled_sb")
            nc.vector.tensor_copy(pooled_sb[:], pooled[:])
            bounce_in = dp.tile([128, 128], F32, tag="bounce_in")
            bounce_out = dp.tile([128, 128], F32, tag="bounce_out")
            nc.sync.dma_start(bounce_in[:], pooled_sb[:])
            if not os.environ.get("KNOCOLL"):
                nc.gpsimd.collective_compute(
                    "AllReduce", OP.add,
                    replica_groups=[list(range(NCORES))],
                    ins=[bounce_in.opt()], outs=[bounce_out.opt()])
                nc.sync.dma_start(pooled_sb[:], bounce_out[:])
            mean16 = wp.tile([128, 128], F16, tag="mean16")
            nc.vector.tensor_tensor(mean16[:], pooled_sb[:], recip_t[:],
                                    op=OP.mult)
            tps = aggp.tile([128, 128], F16, tag="agg")
            nc.tensor.transpose(tps[:], mean16[:], i16_t[:])
            mean_sb = wp.tile([128, 128], F16, tag="mean_sb")
            nc.vector.tensor_copy(mean_sb[:], tps[:])

            mw, mb = {}, {}
            for k in ("W1", "W2", "W3", "Wout"):
                mw[k] = cp.tile(list(pk["mlp"][k].shape), F16, name=f"mw{k}",
                                tag=f"mw{k}")
                nc.sync.dma_start(mw[k][:], h["mlp_" + k].ap())
            for k in ("b1", "b2", "b3", "bout"):
                mb[k] = cp.tile(list(pk["mlp"][k].shape), F32, name=f"mb{k}",
                                tag=f"mb{k}")
                nc.sync.dma_start(mb[k][:], h["mlp_" + k].ap())

            hcur = mean_sb
            for li, (wk, bk) in enumerate((("W1", "b1"), ("W2", "b2"),
                                           ("W3", "b3"))):
                ps = aggp.tile([64, G], F32, name=f"mlp{li}", tag="agg")
                nc.tensor.matmul(ps[:], mw[wk][:], hcur[:],
                                 start=True, stop=True)
                hn = wp.tile([64, G], F16, name=f"hn{li}", tag=f"hn{li}")
                nc.scalar.activation(hn[:], ps[:], AF.Relu, bias=mb[bk][:])
                hcur = hn
            ps_o = aggp.tile([1, G], F32, tag="agg")
            nc.tensor.matmul(ps_o[:], mw["Wout"][:], hcur[:],
                             start=True, stop=True)
            osb = wp.tile([1, G], F32, tag="osb")
            nc.scalar.activation(osb[:], ps_o[:], AF.Identity,
                                 bias=mb["bout"][:])
            nc.sync.dma_start(out_h.ap(), osb[:])

    nc.compile()

    in_maps = []
    for m in range(NCORES):
        im = {}
        for tag in ("c", "b"):
            rl = pk[tag]
            pc = rl["per_core"][m]
            im[f"xt_{tag}"] = pc["xt"]
            im[f"oh_{tag}"] = pc["oh"]
            im[f"pa_{tag}"] = pc["pa"]
            im[f"ohg_{tag}"] = pc["ohg"]
            im[f"w2_{tag}"] = rl["w2"]
            im[f"ws4_{tag}"] = rl["ws4"]
        im["i8"] = pk["ident8"]
        im["i16"] = pk["ident16"]
        im["recip2"] = pk["recip2"]
        im["zl"] = pk["zl"]
        im["zr"] = pk["zr"]
        for k, v in pk["mlp"].items():
            im["mlp_" + k] = v
        in_maps.append(im)

    trace = bool(os.environ.get("KERNEL_TRACE"))
    res = run_bass_kernel_spmd(nc, in_maps, core_ids=list(range(NCORES)),
                               trace=trace)
    global LAST_EXEC_NS
    LAST_EXEC_NS = res.exec_time_ns
    return res.results[0]["out"].reshape(G).astype(np.float32)


# revision 42
# speedup vs baseline: 3.0857x; 1.0297x over previous
"""Trainium2 Bass kernel for hetero-GNN (2x ResGatedGraphConv + segment-mean pooling + MLP).

Sharding: destination-node range per core; each core processes the edges whose
dst falls in its range. Host does index marshalling only; all model arithmetic
runs on device.

Device strategy ("degree rounds"):
  - dst nodes are grouped into 128-slot buckets; 8 buckets form a PSUM group
    whose aggregate [128 slots, 8*64] lives in one PSUM bank.
  - edges of a bucket are packed into R identity rounds (the j-th edge of
    slot p sits at row p of round j) plus <=F flex rounds (leftovers, with a
    host-built fp8 one-hot scatter matrix).
  - a slab = one round of each of the 8 buckets = 8 subtiles of 128 edges.
    Per slab: 8 fused matmuls [xt.T @ W_aug] (row-tiled concurrent pairs),
    one sigmoid (ACT), one gated multiply (DVE), and ONE identity-stationary
    matmul that scatter-adds all 8 subtiles into the group aggregate.
  - skip connection is folded into the same PSUM accumulation (a dummy zero
    matmul opens the accumulation group for the whole bank).
  - relu + pooling via per-bucket one-hot matmul into a [G, 2H] transposed
    pooled PSUM; AllReduce across 8 cores; small MLP head on device.
"""
import os
import sys
import types
import numpy as np
import ml_dtypes

F8NP = ml_dtypes.float8_e4m3fn

NCORES = 8
G = 128
H = 64
F = 16
NC_N = 100000
NB_N = 200000
GRP = 8          # buckets per PSUM group
LAST_EXEC_NS = None


def _install_ntff_shim():
    if 'antenv.axon_hooks' in sys.modules:
        return
    try:
        mod = types.ModuleType('antenv.axon_hooks')
        _h = [None]
        mod.set_axon_ntff_profile_hook = lambda h: _h.__setitem__(0, h)
        mod.get_axon_ntff_profile_hook = lambda: _h[0]
        sys.modules['antenv.axon_hooks'] = mod
        import antenv
        antenv.axon_hooks = mod
        from trn_agent_boot.trn_boot import _ntff_profile_via_ctypes
        mod.set_axon_ntff_profile_hook(
            _ntff_profile_via_ctypes('/opt/axon/libaxon_pjrt.so'))
    except Exception:
        pass


def _waug(ii, rel):
    Wq, Wv, Wk = ii[f"Wq_{rel}"], ii[f"Wv_{rel}"], ii[f"Wk_{rel}"]
    We = ii[f"We_{rel}"][0]
    bq, bv, bk, be = (ii[f"bq_{rel}"], ii[f"bv_{rel}"],
                      ii[f"bk_{rel}"], ii[f"be_{rel}"])
    w = np.zeros((35, 128), np.float32)
    w[0:16, 0:64] = Wq
    w[0:16, 64:128] = Wv
    w[16, 0:64] = 2 * We
    w[16, 64:128] = We
    w[17, 0:64] = bq + bk + 2 * be
    w[17, 64:128] = bv + be
    w[18:34, 0:64] = Wk
    return w


def pack_relation(xs, xd, src, dst, ea, D, ii, rel, batch_dst):
    """Host marshalling for one relation.

    Returns common schedule + per-core device arrays."""
    E = len(src)
    nbuck = (D + 127) // 128
    order = np.argsort(dst, kind="stable")
    src_s, dst_s, ea_s = src[order], dst[order], ea[order]
    core = dst_s // D
    loc = dst_s % D
    buck = loc // 128
    slot = loc % 128
    lin = (core * nbuck + buck) * 128 + slot
    deg = np.bincount(lin, minlength=NCORES * nbuck * 128) \
            .reshape(NCORES, nbuck, 128)
    starts = np.searchsorted(dst_s, dst_s, side="left")
    rank = np.arange(E) - starts

    # common per-bucket-position R (identity rounds): minimize
    # R + w*max_core(F).  Flex subtiles cost more than identity rounds
    # (extra one-hot DMA + per-subtile scatter matmul), so weight them and
    # prefer the larger R on ties.
    maxd = int(deg.max())
    bestT = np.full(nbuck, np.inf)
    bestR = np.zeros(nbuck, np.int64)
    for R in range(0, maxd + 1):
        lo = np.maximum(deg - R, 0).sum(-1)            # [NCORES, nbuck]
        Fk = (-(-lo // 128)).max(0)                    # [nbuck]
        T = R + 1.8 * Fk
        upd = T <= bestT
        bestT[upd] = T[upd]
        bestR[upd] = R

    # group buckets (sorted by R desc) into chunks of GRP
    border = np.argsort(-bestR, kind="stable")
    ngroups = (nbuck + GRP - 1) // GRP
    groups = []
    bucket_group = np.zeros(nbuck, np.int64)   # bucket -> group
    bucket_pos = np.zeros(nbuck, np.int64)     # bucket -> index in group
    bucket_Rs = np.zeros(nbuck, np.int64)      # bucket -> group R*
    xt_col = 0
    oh_blk = 0
    for g in range(ngroups):
        bks = border[g * GRP:(g + 1) * GRP]
        n_b = len(bks)
        Rs = int(bestR[bks].max()) if n_b else 0
        # leftovers recomputed at group R*; order buckets by flex count so
        # each flex slab's present subtiles form a prefix
        lo2 = np.maximum(deg[:, bks, :] - Rs, 0).sum(-1)   # [NCORES, n_b]
        Fk = (-(-lo2 // 128)).max(0)                       # [n_b]
        perm = np.argsort(-Fk, kind="stable")
        bks = bks[perm]
        Fk = Fk[perm]
        Fs = int(Fk.max()) if n_b else 0
        flex = []
        oh_idx = {}
        for f in range(Fs):
            present = [(i, 0) for i in range(n_b) if Fk[i] > f]
            present = [(i, oh_blk + j) for j, (i, _) in enumerate(present)]
            for i, ob in present:
                oh_idx[(i, f)] = ob
            oh_blk += len(present)
            flex.append(present)
        bucket_group[bks] = g
        bucket_pos[bks] = np.arange(n_b)
        bucket_Rs[bks] = Rs
        groups.append({
            "n_b": n_b, "R": Rs, "F": Fs, "bks": bks, "Fk": Fk,
            "flex": flex, "oh_idx": oh_idx, "xt_off": xt_col,
        })
        xt_col += (Rs + Fs) * 512
    XC = max(xt_col, 512)
    OC = max(oh_blk * 128, 128)

    # per-edge destination column in xt (per core arrays share the schedule)
    # xt layout: [64, nsub*128]; subtile (group g, slab s, pos i) at column
    # block (xt_off/512)*8 + s*8 + i  (xt_off counts 512-col slab units).
    g_of = bucket_group[buck]
    i_of = bucket_pos[buck]
    Rs_of = bucket_Rs[buck]
    xoff_of = np.array([gr["xt_off"] for gr in groups], np.int64)[g_of]
    suboff_of = xoff_of // 512 * 8
    is_id = rank < Rs_of
    col = np.zeros(E, np.int64)
    col[is_id] = ((suboff_of[is_id] + rank[is_id] * 8 + i_of[is_id]) * 128
                  + slot[is_id])
    # flex: position among the bucket's leftover edges (dst-sorted order)
    lx = ~is_id
    lin_lx = lin[lx] // 128      # (core,bucket) linear id of leftover edges
    first = np.searchsorted(lin_lx, lin_lx, side="left")
    fpos = np.arange(lx.sum()) - first
    f_of = fpos // 128
    row = fpos % 128
    col[lx] = ((suboff_of[lx] + (Rs_of[lx] + f_of) * 8 + i_of[lx]) * 128
               + row)
    # oh block index for flex edges
    ohmap = np.full((nbuck, 32), -1, np.int64)
    for gr in groups:
        for (i, f), ob in gr["oh_idx"].items():
            ohmap[gr["bks"][i], f] = ob
    oh_of = np.zeros(E, np.int64)
    oh_of[lx] = ohmap[buck[lx], f_of]
    assert (oh_of[lx] >= 0).all()
    flexrow = np.zeros(E, np.int64)
    flexrow[lx] = row

    xsT = xs.astype(np.float32)
    xdT = xd.astype(np.float32)
    per_core = []
    cb = np.searchsorted(core, np.arange(NCORES + 1))
    for m in range(NCORES):
        s0, s1 = cb[m], cb[m + 1]
        c_src, c_dst = src_s[s0:s1], dst_s[s0:s1]
        c_ea, c_col = ea_s[s0:s1], col[s0:s1]
        c_lx = lx[s0:s1]
        c_oh = oh_of[s0:s1]
        c_fr = flexrow[s0:s1]
        c_slot = slot[s0:s1]
        xt = np.zeros((128, XC * 2), np.float32)
        xt[0:16, c_col] = xsT[c_src].T
        xt[16, c_col] = c_ea
        xt[17, c_col] = 1.0
        xt[18:34, c_col] = xdT[c_dst].T
        # flex one-hots: edge at (oh block, row) -> slot
        oh = np.zeros((128, OC), np.float32)
        oh[c_fr[c_lx], c_oh[c_lx] * 128 + c_slot[c_lx]] = 1.0
        # pa (skip lhsT, [32, nbuck*128]) + ohg (pooling one-hot)
        PC = ngroups * GRP * 128
        GC = ngroups * GRP * 128
        pa = np.zeros((32, PC), np.float32)
        ohg = np.zeros((128, GC), np.float32)
        for g, gr in enumerate(groups):
            for i, k in enumerate(gr["bks"]):
                base = m * D + k * 128
                w = min(128, D - k * 128)
                nodes = np.arange(base, base + w)
                cblk = (g * GRP + i) * 128
                pa[0:16, cblk:cblk + w] = xdT[nodes].T
                pa[16, cblk:cblk + w] = 1.0
                bt = batch_dst[nodes]
                ohg[np.arange(w), cblk + bt] = 1.0
        f8 = np.float16 if os.environ.get("KF16") else F8NP
        per_core.append({
            "xt": xt.astype(f8),
            "oh": oh.astype(f8),
            "pa": pa.astype(np.float16),
            "ohg": ohg.astype(f8),
        })

    w2 = np.zeros((128, 128), np.float32)
    w2[0:35] = _waug(ii, rel)
    ws4 = np.zeros((32, 64), np.float32)
    ws4[0:16] = ii[f"Wskip_{rel}"]
    ws4[16] = ii[f"bconv_{rel}"]
    nslabs = sum(gr["R"] + gr["F"] for gr in groups)
    return {
        "groups": groups, "XC": XC, "OC": OC, "ngroups": ngroups,
        "per_core": per_core, "w2": w2.astype(np.float16),
        "ws4": ws4.astype(np.float16), "nslabs": nslabs, "D": D,
        "nbuck": nbuck,
    }


def pack_all(ii):
    Dc, Db = NC_N // NCORES, NB_N // NCORES
    rel_c = pack_relation(ii["x_x"], ii["x_c"], ii["src_ac"].astype(np.int64),
                          ii["dst_ac"].astype(np.int64),
                          np.asarray(ii["ea_ac"])[:, 0], Dc, ii, "ac",
                          ii["batch_c"].astype(np.int64))
    rel_b = pack_relation(ii["x_c"], ii["x_b"], ii["src_cb"].astype(np.int64),
                          ii["dst_cb"].astype(np.int64),
                          np.asarray(ii["ea_cb"])[:, 0], Db, ii, "cb",
                          ii["batch_b"].astype(np.int64))

    cnt_c = np.bincount(ii["batch_c"].astype(np.int64), minlength=G)
    cnt_b = np.bincount(ii["batch_b"].astype(np.int64), minlength=G)
    recip2 = np.zeros((G, 128), np.float32)
    recip2[:, 0:64] = (1.0 / np.maximum(cnt_c, 1))[:, None]
    recip2[:, 64:128] = (1.0 / np.maximum(cnt_b, 1))[:, None]

    mlp = {
        "W1": ii["W1"].astype(np.float16), "W2": ii["W2"].astype(np.float16),
        "W3": ii["W3"].astype(np.float16),
        "Wout": ii["Wout"].astype(np.float16),
        "b1": np.asarray(ii["b1"], np.float32).reshape(64, 1),
        "b2": np.asarray(ii["b2"], np.float32).reshape(64, 1),
        "b3": np.asarray(ii["b3"], np.float32).reshape(64, 1),
        "bout": np.asarray(ii["bout"], np.float32).reshape(1, 1),
    }
    f8 = np.float16 if os.environ.get("KF16") else F8NP
    ident8 = np.eye(128, dtype=f8)
    ident16 = np.eye(128, dtype=np.float16)
    zl = np.zeros((1, 128), np.float16)
    zr = np.zeros((1, 512), np.float16)
    return {"c": rel_c, "b": rel_b, "recip2": recip2.astype(np.float16),
            "mlp": mlp, "ident8": ident8, "ident16": ident16,
            "zl": zl, "zr": zr}


def emulate(ii):
    """Numpy emulation of the device program (for packing validation)."""
    pk = pack_all(ii)
    pooled = np.zeros((G, 128), np.float64)
    for tag in ("c", "b"):
        rl = pk[tag]
        w2 = rl["w2"].astype(np.float32)
        ws4 = rl["ws4"].astype(np.float32)
        for m in range(NCORES):
            pc = rl["per_core"][m]
            xt = pc["xt"].astype(np.float32)
            oh = pc["oh"].astype(np.float32)
            pa = pc["pa"].astype(np.float32)
            ohg = pc["ohg"].astype(np.float32)
            for g, gr in enumerate(rl["groups"]):
                n_b, Rs, Fs = gr["n_b"], gr["R"], gr["F"]
                agg = np.zeros((128, n_b, 64), np.float32)
                for i in range(n_b):
                    cblk = (g * GRP + i) * 128
                    lhs = pa[0:17, cblk:cblk + 128]
                    agg[:, i, :] += lhs.T @ ws4[0:17]
                for s in range(Rs + Fs):
                    c0 = gr["xt_off"] * 2 + s * 1024
                    blkx = xt[:, c0:c0 + 1024]
                    sv = np.zeros((128, 8, 128), np.float32)
                    for i in range(8):
                        sv[:, i, :] = (blkx[:, i * 128:(i + 1) * 128].T @ w2)
                    gt = (1.0 / (1.0 + np.exp(-sv[:, :, 0:64]))) \
                        .astype(np.float16).astype(np.float32)
                    msg = (gt * sv[:, :, 64:128]).astype(np.float16) \
                        .astype(np.float32)
                    if s < Rs:
                        agg += msg[:, :n_b, :]
                    else:
                        for (i, ob) in gr["flex"][s - Rs]:
                            ohb = oh[:, ob * 128:(ob + 1) * 128]
                            agg[:, i, :] += ohb.T @ msg[:, i, :]
                h = np.maximum(agg, 0.0).astype(np.float16).astype(np.float32)
                off = 0 if tag == "c" else 64
                for i in range(n_b):
                    ohgb = ohg[:, (g * GRP + i) * 128:(g * GRP + i + 1) * 128]
                    pooled[:, off:off + 64] += ohgb.T @ h[:, i, :]
    mean = pooled * pk["recip2"].astype(np.float64)
    hcur = mean.T.astype(np.float32)          # [2H, G]
    mlp = pk["mlp"]
    for wk, bk in (("W1", "b1"), ("W2", "b2"), ("W3", "b3")):
        hcur = np.maximum(mlp[wk].astype(np.float32).T @ hcur + mlp[bk], 0.0)
    out = mlp["Wout"].astype(np.float32).T @ hcur + mlp["bout"]
    return out.reshape(G)


def kernel(**inputs):
    _install_ntff_shim()
    import concourse.bass as bass  # noqa: F401
    import concourse.bacc as bacc
    import concourse.mybir as mybir
    import concourse.tile as tile
    from concourse.bass_utils import run_bass_kernel_spmd

    F32 = mybir.dt.float32
    F16 = mybir.dt.float16
    FP8 = F16 if os.environ.get("KF16") else mybir.dt.float8e4
    AF = mybir.ActivationFunctionType
    OP = mybir.AluOpType

    ii = {k: np.asarray(v) for k, v in inputs.items()}
    pk = pack_all(ii)

    nc = bacc.Bacc("TRN2", target_bir_lowering=False, debug=False,
                   num_devices=NCORES)

    def din(name, arr0):
        return nc.dram_tensor(name, list(arr0.shape),
                              mybir.dt.from_np(arr0.dtype),
                              kind="ExternalInput")

    h = {}
    for tag in ("c", "b"):
        rl = pk[tag]
        pc0 = rl["per_core"][0]
        h[f"xt_{tag}"] = din(f"xt_{tag}", pc0["xt"])
        h[f"oh_{tag}"] = din(f"oh_{tag}", pc0["oh"])
        h[f"pa_{tag}"] = din(f"pa_{tag}", pc0["pa"])
        h[f"ohg_{tag}"] = din(f"ohg_{tag}", pc0["ohg"])
        h[f"w2_{tag}"] = din(f"w2_{tag}", rl["w2"])
        h[f"ws4_{tag}"] = din(f"ws4_{tag}", rl["ws4"])
    h["i8"] = din("i8", pk["ident8"])
    h["i16"] = din("i16", pk["ident16"])
    h["recip2"] = din("recip2", pk["recip2"])
    h["zl"] = din("zl", pk["zl"])
    h["zr"] = din("zr", pk["zr"])
    for k, v in pk["mlp"].items():
        h["mlp_" + k] = din("mlp_" + k, v)
    out_h = nc.dram_tensor("out", [1, G], F32, kind="ExternalOutput")

    with tile.TileContext(nc) as tc:
        with tc.tile_pool(name="const", bufs=1) as cp, \
             tc.tile_pool(name="stream", bufs=3) as sp, \
             tc.tile_pool(name="work", bufs=3) as wp, \
             tc.tile_pool(name="svp", bufs=2, space="PSUM") as svp, \
             tc.tile_pool(name="aggp", bufs=2, space="PSUM") as aggp, \
             tc.tile_pool(name="poolp", bufs=1, space="PSUM") as poolp, \
             tc.tile_pool(name="dram", bufs=1, space="DRAM") as dp:

            i8_t = cp.tile([128, 128], FP8, tag="i8")
            nc.sync.dma_start(i8_t[:], h["i8"].ap())
            i16_t = cp.tile([128, 128], F16, tag="i16")
            nc.sync.dma_start(i16_t[:], h["i16"].ap())
            recip_t = cp.tile([128, 128], F16, tag="recip2")
            nc.sync.dma_start(recip_t[:], h["recip2"].ap())
            zl = cp.tile([1, 128], F16, tag="zl")
            nc.sync.dma_start(zl[:], h["zl"].ap())
            zr = cp.tile([1, 512], F16, tag="zr")
            nc.sync.dma_start(zr[:], h["zr"].ap())

            pooled = poolp.tile([128, 128], F32, tag="pooled")

            slab_ctr = [0]

            def relation(tag, col_off):
                rl = pk[tag]
                w2_t = cp.tile([128, 128], F16, tag=f"w2{tag}")
                nc.sync.dma_start(w2_t[:], h[f"w2_{tag}"].ap())
                ws4_t = cp.tile([32, 64], F16, tag=f"ws4{tag}")
                nc.sync.dma_start(ws4_t[:], h[f"ws4_{tag}"].ap())
                xt_v = h[f"xt_{tag}"].ap()
                oh_v = h[f"oh_{tag}"].ap()
                pa_v = h[f"pa_{tag}"].ap()
                ohg_v = h[f"ohg_{tag}"].ap()
                first_pool = [True]
                ngroups = rl["ngroups"]
                pa_w = GRP * 128
                jobs = []
                for g, gr in enumerate(rl["groups"]):
                    if gr["n_b"] == 0:
                        continue
                    for s in range(gr["R"] + gr["F"]):
                        jobs.append((g, gr, s))
                last_g = jobs[-1][0]
                # software pipeline: scatter(slab k) emits after MM1s(k+2)
                # (2-deep: the sigmoid+mult chain is longer than one slab);
                # relu(group) emits with its last scatter; pooling(group)
                # defers one more slab so PE never waits on ACT.
                pend_sc = []
                pend_pool = []     # list of [due_idx, fn]
                cur = [0]

                def flush_sc(depth=1):
                    while len(pend_sc) > depth:
                        pend_sc.pop(0)()

                def flush_pool():
                    while pend_pool and pend_pool[0][0] <= cur[0]:
                        pend_pool.pop(0)[1]()

                def mk_pool(g, gr, agg, ohg_t):
                    n_b = gr["n_b"]
                    h_sb = wp.tile([128, GRP, 64], F16, name=f"h{tag}{g}",
                                   tag="hsb")
                    nc.scalar.activation(h_sb[:, 0:n_b, :],
                                         agg[:, 0:n_b, :], AF.Relu)

                    def pool():
                        for i in range(n_b):
                            nc.tensor.matmul(
                                pooled[:, col_off:col_off + 64],
                                ohg_t[:, i * 128:(i + 1) * 128],
                                h_sb[:, i, :],
                                start=first_pool[0],
                                stop=(g == last_g and i == n_b - 1),
                                skip_group_check=True)
                            first_pool[0] = False
                    pend_pool.append([cur[0] + 1, pool])

                st = {}
                xt2 = [None, 0]
                for idx, (g, gr, s) in enumerate(jobs):
                    cur[0] = idx
                    n_b, Rs, Fs = gr["n_b"], gr["R"], gr["F"]
                    nsl = Rs + Fs
                    if s == 0:
                        pa_t = sp.tile([32, pa_w], F16, name=f"pa{tag}{g}",
                                       tag="pa")
                        nc.sync.dma_start(pa_t[:],
                                          pa_v[:, g * pa_w:(g + 1) * pa_w])
                        ohg_t = sp.tile([128, GRP * 128], FP8,
                                        name=f"ohg{tag}{g}", tag="ohg")
                        nc.sync.dma_start(
                            ohg_t[:, 0:n_b * 128],
                            ohg_v[:, g * GRP * 128:
                                  g * GRP * 128 + n_b * 128])
                        agg = aggp.tile([128, GRP, 64], F32,
                                        name=f"agg{tag}{g}", tag="agg")
                        nc.tensor.matmul(agg[:, 0:n_b, :], zl[:],
                                         zr[:, 0:n_b * 64], start=True,
                                         stop=False, skip_group_check=True)
                        for i in range(n_b):
                            nc.tensor.matmul(
                                agg[:, i, :],
                                pa_t[0:17, i * 128:i * 128 + 128],
                                ws4_t[0:17, :],
                                start=False, stop=False,
                                skip_group_check=True)
                        nsc = Rs + sum(len(p) for p in gr["flex"])
                        st[g] = {"agg": agg, "ohg_t": ohg_t, "sci": [0],
                                 "nsc": nsc}
                    sg = st[g]
                    agg, ohg_t = sg["agg"], sg["ohg_t"]

                    def sc_flags(sg=sg):
                        sg["sci"][0] += 1
                        return {"start": False,
                                "stop": sg["sci"][0] == sg["nsc"],
                                "skip_group_check": True}

                    # xt DMA batched over slab pairs (within the group)
                    if s % 2 == 0:
                        wcols = min(2, nsl - s) * 1024
                        c0 = gr["xt_off"] * 2 + s * 1024
                        xt2[0] = sp.tile([128, 2048], FP8,
                                         name=f"xt{tag}{g}_{s}", tag="xt")
                        nc.sync.dma_start(xt2[0][:, 0:wcols],
                                          xt_v[:, c0:c0 + wcols])
                    xtsl = xt2[0][:, (s % 2) * 1024:(s % 2) * 1024 + 1024]
                    flex = None
                    if s >= Rs:
                        flex = gr["flex"][s - Rs]
                        ob0 = flex[0][1]
                        obn = len(flex)
                        oh_t = sp.tile([128, GRP * 128], FP8,
                                       name=f"oh{tag}{g}_{s}", tag="ohf")
                        nc.sync.dma_start(
                            oh_t[:, 0:obn * 128],
                            oh_v[:, ob0 * 128:(ob0 + obn) * 128])
                        present = set(i for i, _ in flex)
                    svk = svp.tile([128, 8, 128], F32,
                                   name=f"sv{tag}{g}_{s}", tag="sv")
                    nact = 8 if flex is None else len(flex)
                    for i in range(8):
                        if flex is not None and i not in present:
                            continue
                        nc.tensor.matmul(
                            svk[:, i, :],
                            xtsl[:, i * 128:(i + 1) * 128],
                            w2_t[:],
                            start=True, stop=True)
                    flush_sc(1)  # scatter of slab idx-2 runs behind our MM1s
                    flush_pool()
                    gt = wp.tile([128, 8, 64], F16,
                                 name=f"gt{tag}{g}_{s}", tag="gt")
                    nc.scalar.activation(gt[:, 0:nact, :],
                                         svk[:, 0:nact, 0:64], AF.Sigmoid)
                    msg = wp.tile([128, 8, 64], F16,
                                  name=f"msg{tag}{g}_{s}", tag="msg")
                    nc.vector.tensor_tensor(msg[:, 0:nact, :],
                                            gt[:, 0:nact, :],
                                            svk[:, 0:nact, 64:128],
                                            op=OP.mult)

                    def mk(flex, oh_t, msg, g=g, gr=gr, agg=agg,
                           ohg_t=ohg_t, is_last=(s == nsl - 1),
                           sc_flags=sc_flags):
                        def emit():
                            if flex is None:
                                nc.tensor.matmul(agg[:, 0:gr["n_b"], :],
                                                 i8_t[:],
                                                 msg[:, 0:gr["n_b"], :],
                                                 **sc_flags())
                            else:
                                for j, (i, _) in enumerate(flex):
                                    nc.tensor.matmul(
                                        agg[:, i, :],
                                        oh_t[:, j * 128:(j + 1) * 128],
                                        msg[:, i, :], **sc_flags())
                            if is_last:
                                mk_pool(g, gr, agg, ohg_t)
                        return emit

                    pend_sc.append(mk(flex,
                                      oh_t if flex is not None else None,
                                      msg))
                cur[0] += 1
                flush_sc(0)
                cur[0] += 1
                flush_pool()
                while pend_pool:
                    pend_pool.pop(0)[1]()

            if os.environ.get("KONLYREL") != "b":
                relation("c", 0)
            if os.environ.get("KONLYREL") != "c":
                relation("b", 64)

            # --- head: AllReduce pooled, divide by counts, transpose, MLP ---
            pooled_sb = wp.tile([128, 128], F32, tag="pooled_sb")
            nc.vector.tensor_copy(pooled_sb[:], pooled[:])
            bounce_in = dp.tile([128, 128], F32, tag="bounce_in")
            bounce_out = dp.tile([128, 128], F32, tag="bounce_out")
            nc.sync.dma_start(bounce_in[:], pooled_sb[:])
            if not os.environ.get("KNOCOLL"):
                nc.gpsimd.collective_compute(
                    "AllReduce", OP.add,
                    replica_groups=[list(range(NCORES))],
                    ins=[bounce_in.opt()], outs=[bounce_out.opt()])
                nc.sync.dma_start(pooled_sb[:], bounce_out[:])
            mean16 = wp.tile([128, 128], F16, tag="mean16")
            nc.vector.tensor_tensor(mean16[:], pooled_sb[:], recip_t[:],
                                    op=OP.mult)
            tps = aggp.tile([128, 128], F16, tag="agg")
            nc.tensor.transpose(tps[:], mean16[:], i16_t[:])
            mean_sb = wp.tile([128, 128], F16, tag="mean_sb")
            nc.vector.tensor_copy(mean_sb[:], tps[:])

            mw, mb = {}, {}
            for k in ("W1", "W2", "W3", "Wout"):
                mw[k] = cp.tile(list(pk["mlp"][k].shape), F16, name=f"mw{k}",
                                tag=f"mw{k}")
                nc.sync.dma_start(mw[k][:], h["mlp_" + k].ap())
            for k in ("b1", "b2", "b3", "bout"):
                mb[k] = cp.tile(list(pk["mlp"][k].shape), F32, name=f"mb{k}",
                                tag=f"mb{k}")
                nc.sync.dma_start(mb[k][:], h["mlp_" + k].ap())

            hcur = mean_sb
            for li, (wk, bk) in enumerate((("W1", "b1"), ("W2", "b2"),
                                           ("W3", "b3"))):
                ps = aggp.tile([64, G], F32, name=f"mlp{li}", tag="agg")
                nc.tensor.matmul(ps[:], mw[wk][:], hcur[:],
                                 start=True, stop=True)
                hn = wp.tile([64, G], F16, name=f"hn{li}", tag=f"hn{li}")
                nc.scalar.activation(hn[:], ps[:], AF.Relu, bias=mb[bk][:])
                hcur = hn
            ps_o = aggp.tile([1, G], F32, tag="agg")
            nc.tensor.matmul(ps_o[:], mw["Wout"][:], hcur[:],
                             start=True, stop=True)
            osb = wp.tile([1, G], F32, tag="osb")
            nc.scalar.activation(osb[:], ps_o[:], AF.Identity,
                                 bias=mb["bout"][:])
            nc.sync.dma_start(out_h.ap(), osb[:])

    nc.compile()

    in_maps = []
    for m in range(NCORES):
        im = {}
        for tag in ("c", "b"):
            rl = pk[tag]
            pc = rl["per_core"][m]
            im[f"xt_{tag}"] = pc["xt"]
            im[f"oh_{tag}"] = pc["oh"]
            im[f"pa_{tag}"] = pc["pa"]
            im[f"ohg_{tag}"] = pc["ohg"]
            im[f"w2_{tag}"] = rl["w2"]
            im[f"ws4_{tag}"] = rl["ws4"]
        im["i8"] = pk["ident8"]
        im["i16"] = pk["ident16"]
        im["recip2"] = pk["recip2"]
        im["zl"] = pk["zl"]
        im["zr"] = pk["zr"]
        for k, v in pk["mlp"].items():
            im["mlp_" + k] = v
        in_maps.append(im)

    trace = bool(os.environ.get("KERNEL_TRACE"))
    res = run_bass_kernel_spmd(nc, in_maps, core_ids=list(range(NCORES)),
                               trace=trace)
    global LAST_EXEC_NS
    LAST_EXEC_NS = res.exec_time_ns
    return res.results[0]["out"].reshape(G).astype(np.float32)


# revision 49
# speedup vs baseline: 3.4982x; 1.1337x over previous
"""Trainium2 Bass kernel for hetero-GNN (2x ResGatedGraphConv + segment-mean pooling + MLP).

Sharding: destination-node range per core; each core processes the edges whose
dst falls in its range. Host does index marshalling only; all model arithmetic
runs on device.

Device strategy ("degree rounds"):
  - dst nodes are grouped into 128-slot buckets; 8 buckets form a PSUM group
    whose aggregate [128 slots, 8*64] lives in one PSUM bank.
  - edges of a bucket are packed into R identity rounds (the j-th edge of
    slot p sits at row p of round j) plus <=F flex rounds (leftovers, with a
    host-built fp8 one-hot scatter matrix).
  - a slab = one round of each of the 8 buckets = 8 subtiles of 128 edges.
    Per slab: 8 fused matmuls [xt.T @ W_aug] (row-tiled concurrent pairs),
    one sigmoid (ACT), one gated multiply (DVE), and ONE identity-stationary
    matmul that scatter-adds all 8 subtiles into the group aggregate.
  - skip connection is folded into the same PSUM accumulation (a dummy zero
    matmul opens the accumulation group for the whole bank).
  - relu + pooling via per-bucket one-hot matmul into a [G, 2H] transposed
    pooled PSUM; AllReduce across 8 cores; small MLP head on device.
"""
import os
import sys
import types
import numpy as np
import ml_dtypes

F8NP = ml_dtypes.float8_e4m3fn

NCORES = 8
G = 128
H = 64
F = 16
NC_N = 100000
NB_N = 200000
GRP = 8          # buckets per PSUM group
LAST_EXEC_NS = None


def _install_ntff_shim():
    if 'antenv.axon_hooks' in sys.modules:
        return
    try:
        mod = types.ModuleType('antenv.axon_hooks')
        _h = [None]
        mod.set_axon_ntff_profile_hook = lambda h: _h.__setitem__(0, h)
        mod.get_axon_ntff_profile_hook = lambda: _h[0]
        sys.modules['antenv.axon_hooks'] = mod
        import antenv
        antenv.axon_hooks = mod
        from trn_agent_boot.trn_boot import _ntff_profile_via_ctypes
        mod.set_axon_ntff_profile_hook(
            _ntff_profile_via_ctypes('/opt/axon/libaxon_pjrt.so'))
    except Exception:
        pass


def _waug(ii, rel):
    Wq, Wv, Wk = ii[f"Wq_{rel}"], ii[f"Wv_{rel}"], ii[f"Wk_{rel}"]
    We = ii[f"We_{rel}"][0]
    bq, bv, bk, be = (ii[f"bq_{rel}"], ii[f"bv_{rel}"],
                      ii[f"bk_{rel}"], ii[f"be_{rel}"])
    w = np.zeros((35, 128), np.float32)
    w[0:16, 0:64] = Wq
    w[0:16, 64:128] = Wv
    w[16, 0:64] = 2 * We
    w[16, 64:128] = We
    w[17, 0:64] = bq + bk + 2 * be
    w[17, 64:128] = bv + be
    w[18:34, 0:64] = Wk
    return w


def pack_relation(xs, xd, src, dst, ea, D, ii, rel, batch_dst):
    """Host marshalling for one relation.

    Returns common schedule + per-core device arrays."""
    E = len(src)
    nbuck = (D + 127) // 128
    order = np.argsort(dst, kind="stable")
    src_s, dst_s, ea_s = src[order], dst[order], ea[order]
    core = dst_s // D
    loc = dst_s % D
    buck = loc // 128
    slot = loc % 128
    lin = (core * nbuck + buck) * 128 + slot
    deg = np.bincount(lin, minlength=NCORES * nbuck * 128) \
            .reshape(NCORES, nbuck, 128)
    starts = np.searchsorted(dst_s, dst_s, side="left")
    rank = np.arange(E) - starts

    # common per-bucket-position R (identity rounds): minimize
    # R + w*max_core(F).  Flex subtiles cost more than identity rounds
    # (extra one-hot DMA + per-subtile scatter matmul), so weight them and
    # prefer the larger R on ties.
    maxd = int(deg.max())
    bestT = np.full(nbuck, np.inf)
    bestR = np.zeros(nbuck, np.int64)
    for R in range(0, maxd + 1):
        lo = np.maximum(deg - R, 0).sum(-1)            # [NCORES, nbuck]
        Fk = (-(-lo // 128)).max(0)                    # [nbuck]
        T = R + 1.8 * Fk
        upd = T <= bestT
        bestT[upd] = T[upd]
        bestR[upd] = R

    # group buckets (sorted by R desc) into chunks of GRP
    border = np.argsort(-bestR, kind="stable")
    ngroups = (nbuck + GRP - 1) // GRP
    groups = []
    bucket_group = np.zeros(nbuck, np.int64)   # bucket -> group
    bucket_pos = np.zeros(nbuck, np.int64)     # bucket -> index in group
    bucket_Rs = np.zeros(nbuck, np.int64)      # bucket -> group R*
    xt_col = 0
    oh_blk = 0
    for g in range(ngroups):
        bks = border[g * GRP:(g + 1) * GRP]
        n_b = len(bks)
        Rs = int(bestR[bks].max()) if n_b else 0
        # leftovers recomputed at group R*; order buckets by flex count so
        # each flex slab's present subtiles form a prefix
        lo2 = np.maximum(deg[:, bks, :] - Rs, 0).sum(-1)   # [NCORES, n_b]
        Fk = (-(-lo2 // 128)).max(0)                       # [n_b]
        perm = np.argsort(-Fk, kind="stable")
        bks = bks[perm]
        Fk = Fk[perm]
        Fs = int(Fk.max()) if n_b else 0
        flex = []
        oh_idx = {}
        for f in range(Fs):
            present = [(i, 0) for i in range(n_b) if Fk[i] > f]
            present = [(i, oh_blk + j) for j, (i, _) in enumerate(present)]
            for i, ob in present:
                oh_idx[(i, f)] = ob
            oh_blk += len(present)
            flex.append(present)
        bucket_group[bks] = g
        bucket_pos[bks] = np.arange(n_b)
        bucket_Rs[bks] = Rs
        groups.append({
            "n_b": n_b, "R": Rs, "F": Fs, "bks": bks, "Fk": Fk,
            "flex": flex, "oh_idx": oh_idx, "xt_off": xt_col,
        })
        xt_col += (Rs + Fs) * 512
    XC = max(xt_col, 512)
    OC = max(oh_blk * 128, 128)

    # per-edge destination column in xt (per core arrays share the schedule)
    # xt layout: [64, nsub*128]; subtile (group g, slab s, pos i) at column
    # block (xt_off/512)*8 + s*8 + i  (xt_off counts 512-col slab units).
    g_of = bucket_group[buck]
    i_of = bucket_pos[buck]
    Rs_of = bucket_Rs[buck]
    xoff_of = np.array([gr["xt_off"] for gr in groups], np.int64)[g_of]
    suboff_of = xoff_of // 512 * 8
    is_id = rank < Rs_of
    col = np.zeros(E, np.int64)
    col[is_id] = ((suboff_of[is_id] + rank[is_id] * 8 + i_of[is_id]) * 128
                  + slot[is_id])
    # flex: position among the bucket's leftover edges (dst-sorted order)
    lx = ~is_id
    lin_lx = lin[lx] // 128      # (core,bucket) linear id of leftover edges
    first = np.searchsorted(lin_lx, lin_lx, side="left")
    fpos = np.arange(lx.sum()) - first
    f_of = fpos // 128
    row = fpos % 128
    col[lx] = ((suboff_of[lx] + (Rs_of[lx] + f_of) * 8 + i_of[lx]) * 128
               + row)
    # oh block index for flex edges
    ohmap = np.full((nbuck, 32), -1, np.int64)
    for gr in groups:
        for (i, f), ob in gr["oh_idx"].items():
            ohmap[gr["bks"][i], f] = ob
    oh_of = np.zeros(E, np.int64)
    oh_of[lx] = ohmap[buck[lx], f_of]
    assert (oh_of[lx] >= 0).all()
    flexrow = np.zeros(E, np.int64)
    flexrow[lx] = row

    xsT = xs.astype(np.float32)
    xdT = xd.astype(np.float32)
    per_core = []
    cb = np.searchsorted(core, np.arange(NCORES + 1))
    for m in range(NCORES):
        s0, s1 = cb[m], cb[m + 1]
        c_src, c_dst = src_s[s0:s1], dst_s[s0:s1]
        c_ea, c_col = ea_s[s0:s1], col[s0:s1]
        c_lx = lx[s0:s1]
        c_oh = oh_of[s0:s1]
        c_fr = flexrow[s0:s1]
        c_slot = slot[s0:s1]
        xt = np.zeros((128, XC * 2), np.float32)
        xt[0:16, c_col] = xsT[c_src].T
        xt[16, c_col] = c_ea
        xt[17, c_col] = 1.0
        xt[18:34, c_col] = xdT[c_dst].T
        # flex one-hots: edge at (oh block, row) -> slot
        oh = np.zeros((128, OC), np.float32)
        oh[c_fr[c_lx], c_oh[c_lx] * 128 + c_slot[c_lx]] = 1.0
        # pa (skip lhsT, [32, nbuck*128]) + ohg (pooling one-hot)
        PC = ngroups * GRP * 128
        GC = ngroups * GRP * 128
        pa = np.zeros((32, PC), np.float32)
        ohg = np.zeros((128, GC), np.float32)
        for g, gr in enumerate(groups):
            for i, k in enumerate(gr["bks"]):
                base = m * D + k * 128
                w = min(128, D - k * 128)
                nodes = np.arange(base, base + w)
                cblk = (g * GRP + i) * 128
                pa[0:16, cblk:cblk + w] = xdT[nodes].T
                pa[16, cblk:cblk + w] = 1.0
                bt = batch_dst[nodes]
                ohg[np.arange(w), cblk + bt] = 1.0
        f8 = np.float16 if os.environ.get("KF16") else F8NP
        per_core.append({
            "xt": xt.astype(f8),
            "oh": oh.astype(f8),
            "pa": pa.astype(np.float16),
            "ohg": ohg.astype(f8),
        })

    w2 = np.zeros((128, 128), np.float32)
    w2[0:35] = _waug(ii, rel)
    ws4 = np.zeros((32, 64), np.float32)
    ws4[0:16] = ii[f"Wskip_{rel}"]
    ws4[16] = ii[f"bconv_{rel}"]
    nslabs = sum(gr["R"] + gr["F"] for gr in groups)
    return {
        "groups": groups, "XC": XC, "OC": OC, "ngroups": ngroups,
        "per_core": per_core, "w2": w2.astype(np.float16),
        "ws4": ws4.astype(np.float16), "nslabs": nslabs, "D": D,
        "nbuck": nbuck,
    }


def pack_all(ii):
    Dc, Db = NC_N // NCORES, NB_N // NCORES
    rel_c = pack_relation(ii["x_x"], ii["x_c"], ii["src_ac"].astype(np.int64),
                          ii["dst_ac"].astype(np.int64),
                          np.asarray(ii["ea_ac"])[:, 0], Dc, ii, "ac",
                          ii["batch_c"].astype(np.int64))
    rel_b = pack_relation(ii["x_c"], ii["x_b"], ii["src_cb"].astype(np.int64),
                          ii["dst_cb"].astype(np.int64),
                          np.asarray(ii["ea_cb"])[:, 0], Db, ii, "cb",
                          ii["batch_b"].astype(np.int64))

    cnt_c = np.bincount(ii["batch_c"].astype(np.int64), minlength=G)
    cnt_b = np.bincount(ii["batch_b"].astype(np.int64), minlength=G)
    recip2 = np.zeros((G, 128), np.float32)
    recip2[:, 0:64] = (1.0 / np.maximum(cnt_c, 1))[:, None]
    recip2[:, 64:128] = (1.0 / np.maximum(cnt_b, 1))[:, None]

    mlp = {
        "W1": ii["W1"].astype(np.float16), "W2": ii["W2"].astype(np.float16),
        "W3": ii["W3"].astype(np.float16),
        "Wout": ii["Wout"].astype(np.float16),
        "b1": np.asarray(ii["b1"], np.float32).reshape(64, 1),
        "b2": np.asarray(ii["b2"], np.float32).reshape(64, 1),
        "b3": np.asarray(ii["b3"], np.float32).reshape(64, 1),
        "bout": np.asarray(ii["bout"], np.float32).reshape(1, 1),
    }
    f8 = np.float16 if os.environ.get("KF16") else F8NP
    ident8 = np.eye(128, dtype=f8)
    ident16 = np.eye(128, dtype=np.float16)
    zl = np.zeros((1, 128), np.float16)
    zr = np.zeros((1, 512), np.float16)
    z128 = np.zeros((128, 128), np.float32)
    return {"c": rel_c, "b": rel_b, "recip2": recip2.astype(np.float16),
            "mlp": mlp, "ident8": ident8, "ident16": ident16,
            "zl": zl, "zr": zr, "z128": z128}


def emulate(ii):
    """Numpy emulation of the device program (for packing validation)."""
    pk = pack_all(ii)
    pooled = np.zeros((G, 128), np.float64)
    for tag in ("c", "b"):
        rl = pk[tag]
        w2 = rl["w2"].astype(np.float32)
        ws4 = rl["ws4"].astype(np.float32)
        for m in range(NCORES):
            pc = rl["per_core"][m]
            xt = pc["xt"].astype(np.float32)
            oh = pc["oh"].astype(np.float32)
            pa = pc["pa"].astype(np.float32)
            ohg = pc["ohg"].astype(np.float32)
            for g, gr in enumerate(rl["groups"]):
                n_b, Rs, Fs = gr["n_b"], gr["R"], gr["F"]
                agg = np.zeros((128, n_b, 64), np.float32)
                for i in range(n_b):
                    cblk = (g * GRP + i) * 128
                    lhs = pa[0:17, cblk:cblk + 128]
                    agg[:, i, :] += lhs.T @ ws4[0:17]
                for s in range(Rs + Fs):
                    c0 = gr["xt_off"] * 2 + s * 1024
                    blkx = xt[:, c0:c0 + 1024]
                    sv = np.zeros((128, 8, 128), np.float32)
                    for i in range(8):
                        sv[:, i, :] = (blkx[:, i * 128:(i + 1) * 128].T @ w2)
                    gt = (1.0 / (1.0 + np.exp(-sv[:, :, 0:64]))) \
                        .astype(np.float16).astype(np.float32)
                    msg = (gt * sv[:, :, 64:128]).astype(np.float16) \
                        .astype(np.float32)
                    if s < Rs:
                        agg += msg[:, :n_b, :]
                    else:
                        for (i, ob) in gr["flex"][s - Rs]:
                            ohb = oh[:, ob * 128:(ob + 1) * 128]
                            agg[:, i, :] += ohb.T @ msg[:, i, :]
                h = np.maximum(agg, 0.0).astype(np.float16).astype(np.float32)
                off = 0 if tag == "c" else 64
                for i in range(n_b):
                    ohgb = ohg[:, (g * GRP + i) * 128:(g * GRP + i + 1) * 128]
                    pooled[:, off:off + 64] += ohgb.T @ h[:, i, :]
    mean = pooled * pk["recip2"].astype(np.float64)
    hcur = mean.T.astype(np.float32)          # [2H, G]
    mlp = pk["mlp"]
    for wk, bk in (("W1", "b1"), ("W2", "b2"), ("W3", "b3")):
        hcur = np.maximum(mlp[wk].astype(np.float32).T @ hcur + mlp[bk], 0.0)
    out = mlp["Wout"].astype(np.float32).T @ hcur + mlp["bout"]
    return out.reshape(G)


def kernel(**inputs):
    _install_ntff_shim()
    import concourse.bass as bass  # noqa: F401
    import concourse.bacc as bacc
    import concourse.mybir as mybir
    import concourse.tile as tile
    from concourse.bass_utils import run_bass_kernel_spmd

    F32 = mybir.dt.float32
    F16 = mybir.dt.float16
    FP8 = F16 if os.environ.get("KF16") else mybir.dt.float8e4
    AF = mybir.ActivationFunctionType
    OP = mybir.AluOpType

    ii = {k: np.asarray(v) for k, v in inputs.items()}
    pk = pack_all(ii)

    nc = bacc.Bacc("TRN2", target_bir_lowering=False, debug=False,
                   num_devices=NCORES)

    def din(name, arr0):
        return nc.dram_tensor(name, list(arr0.shape),
                              mybir.dt.from_np(arr0.dtype),
                              kind="ExternalInput")

    h = {}
    for tag in ("c", "b"):
        rl = pk[tag]
        pc0 = rl["per_core"][0]
        h[f"xt_{tag}"] = din(f"xt_{tag}", pc0["xt"])
        h[f"oh_{tag}"] = din(f"oh_{tag}", pc0["oh"])
        h[f"pa_{tag}"] = din(f"pa_{tag}", pc0["pa"])
        h[f"ohg_{tag}"] = din(f"ohg_{tag}", pc0["ohg"])
        h[f"w2_{tag}"] = din(f"w2_{tag}", rl["w2"])
        h[f"ws4_{tag}"] = din(f"ws4_{tag}", rl["ws4"])
    h["i8"] = din("i8", pk["ident8"])
    h["i16"] = din("i16", pk["ident16"])
    h["recip2"] = din("recip2", pk["recip2"])
    h["zl"] = din("zl", pk["zl"])
    h["zr"] = din("zr", pk["zr"])
    h["z128"] = din("z128", pk["z128"])
    for k, v in pk["mlp"].items():
        h["mlp_" + k] = din("mlp_" + k, v)
    out_h = nc.dram_tensor("out", [1, G], F32, kind="ExternalOutput")

    with tile.TileContext(nc) as tc:
        with tc.tile_pool(name="const", bufs=1) as cp, \
             tc.tile_pool(name="stream", bufs=3) as sp, \
             tc.tile_pool(name="work", bufs=3) as wp, \
             tc.tile_pool(name="svp", bufs=3, space="PSUM") as svp, \
             tc.tile_pool(name="aggp", bufs=2, space="PSUM") as aggp, \
             tc.tile_pool(name="dram", bufs=1, space="DRAM") as dp:

            i8_t = cp.tile([128, 128], FP8, tag="i8")
            nc.sync.dma_start(i8_t[:], h["i8"].ap())
            i16_t = cp.tile([128, 128], F16, tag="i16")
            nc.sync.dma_start(i16_t[:], h["i16"].ap())
            recip_t = cp.tile([128, 128], F16, tag="recip2")
            nc.sync.dma_start(recip_t[:], h["recip2"].ap())
            zl = cp.tile([1, 128], F16, tag="zl")
            nc.sync.dma_start(zl[:], h["zl"].ap())
            zr = cp.tile([1, 512], F16, tag="zr")
            nc.sync.dma_start(zr[:], h["zr"].ap())

            pooled_sb = wp.tile([128, 128], F32, tag="pooled_sb")
            nc.sync.dma_start(pooled_sb[:], h["z128"].ap())

            slab_ctr = [0]

            def relation(tag, col_off):
                rl = pk[tag]
                w2_t = cp.tile([128, 128], F16, tag=f"w2{tag}")
                nc.sync.dma_start(w2_t[:], h[f"w2_{tag}"].ap())
                ws4_t = cp.tile([32, 64], F16, tag=f"ws4{tag}")
                nc.sync.dma_start(ws4_t[:], h[f"ws4_{tag}"].ap())
                xt_v = h[f"xt_{tag}"].ap()
                oh_v = h[f"oh_{tag}"].ap()
                pa_v = h[f"pa_{tag}"].ap()
                ohg_v = h[f"ohg_{tag}"].ap()
                first_pool = [True]
                ngroups = rl["ngroups"]
                pa_w = GRP * 128
                jobs = []
                for g, gr in enumerate(rl["groups"]):
                    if gr["n_b"] == 0:
                        continue
                    for s in range(gr["R"] + gr["F"]):
                        jobs.append((g, gr, s))
                last_g = jobs[-1][0]
                # software pipeline: scatter(slab k) emits after MM1s(k+2)
                # (2-deep: the sigmoid+mult chain is longer than one slab);
                # relu(group) emits with its last scatter; pooling(group)
                # defers one more slab so PE never waits on ACT.
                pend_sc = []
                pend_pool = []     # list of [due_idx, fn]
                cur = [0]

                def flush_sc(depth=1):
                    while len(pend_sc) > depth:
                        pend_sc.pop(0)()

                def flush_pool():
                    while pend_pool and pend_pool[0][0] <= cur[0]:
                        pend_pool.pop(0)[1]()

                def mk_pool(g, gr, agg, ohg_t):
                    n_b = gr["n_b"]
                    h_sb = wp.tile([128, GRP, 64], F16, name=f"h{tag}{g}",
                                   tag="hsb")
                    nc.scalar.activation(h_sb[:, 0:n_b, :],
                                         agg[:, 0:n_b, :], AF.Relu)

                    def pool():
                        gpool = svp.tile([128, 64], F32, name=f"gp{tag}{g}",
                                         tag="sv")
                        for i in range(n_b):
                            nc.tensor.matmul(
                                gpool[:],
                                ohg_t[:, i * 128:(i + 1) * 128],
                                h_sb[:, i, :],
                                start=(i == 0), stop=(i == n_b - 1),
                                skip_group_check=True)
                        nc.vector.tensor_tensor(
                            pooled_sb[:, col_off:col_off + 64],
                            pooled_sb[:, col_off:col_off + 64],
                            gpool[:], op=OP.add)
                    pend_pool.append([cur[0] + 1, pool])

                st = {}
                xt2 = [None, 0]
                for idx, (g, gr, s) in enumerate(jobs):
                    cur[0] = idx
                    n_b, Rs, Fs = gr["n_b"], gr["R"], gr["F"]
                    nsl = Rs + Fs
                    if s == 0:
                        pa_t = sp.tile([32, pa_w], F16, name=f"pa{tag}{g}",
                                       tag="pa")
                        nc.sync.dma_start(pa_t[:],
                                          pa_v[:, g * pa_w:(g + 1) * pa_w])
                        ohg_t = sp.tile([128, GRP * 128], FP8,
                                        name=f"ohg{tag}{g}", tag="ohg")
                        nc.sync.dma_start(
                            ohg_t[:, 0:n_b * 128],
                            ohg_v[:, g * GRP * 128:
                                  g * GRP * 128 + n_b * 128])
                        agg = aggp.tile([128, GRP, 64], F32,
                                        name=f"agg{tag}{g}", tag="agg")
                        nc.tensor.matmul(agg[:, 0:n_b, :], zl[:],
                                         zr[:, 0:n_b * 64], start=True,
                                         stop=False, skip_group_check=True)
                        for i in range(n_b):
                            nc.tensor.matmul(
                                agg[:, i, :],
                                pa_t[0:17, i * 128:i * 128 + 128],
                                ws4_t[0:17, :],
                                start=False, stop=False,
                                skip_group_check=True)
                        nsc = Rs + sum(len(p) for p in gr["flex"])
                        st[g] = {"agg": agg, "ohg_t": ohg_t, "sci": [0],
                                 "nsc": nsc}
                    sg = st[g]
                    agg, ohg_t = sg["agg"], sg["ohg_t"]

                    def sc_flags(sg=sg):
                        sg["sci"][0] += 1
                        return {"start": False,
                                "stop": sg["sci"][0] == sg["nsc"],
                                "skip_group_check": True}

                    # xt DMA batched over slab pairs (within the group)
                    if s % 2 == 0:
                        wcols = min(2, nsl - s) * 1024
                        c0 = gr["xt_off"] * 2 + s * 1024
                        xt2[0] = sp.tile([128, 2048], FP8,
                                         name=f"xt{tag}{g}_{s}", tag="xt")
                        nc.sync.dma_start(xt2[0][:, 0:wcols],
                                          xt_v[:, c0:c0 + wcols])
                    xtsl = xt2[0][:, (s % 2) * 1024:(s % 2) * 1024 + 1024]
                    flex = None
                    if s >= Rs:
                        flex = gr["flex"][s - Rs]
                        ob0 = flex[0][1]
                        obn = len(flex)
                        oh_t = sp.tile([128, GRP * 128], FP8,
                                       name=f"oh{tag}{g}_{s}", tag="ohf")
                        nc.sync.dma_start(
                            oh_t[:, 0:obn * 128],
                            oh_v[:, ob0 * 128:(ob0 + obn) * 128])
                        present = set(i for i, _ in flex)
                    svk = svp.tile([128, 8, 128], F32,
                                   name=f"sv{tag}{g}_{s}", tag="sv")
                    nact = 8 if flex is None else len(flex)
                    for i in range(8):
                        if flex is not None and i not in present:
                            continue
                        nc.tensor.matmul(
                            svk[:, i, :],
                            xtsl[:, i * 128:(i + 1) * 128],
                            w2_t[:],
                            start=True, stop=True)
                    flush_sc(1)  # scatter of slab idx-2 runs behind our MM1s
                    flush_pool()
                    gt = wp.tile([128, 8, 64], F16,
                                 name=f"gt{tag}{g}_{s}", tag="gt")
                    nc.scalar.activation(gt[:, 0:nact, :],
                                         svk[:, 0:nact, 0:64], AF.Sigmoid)
                    msg = wp.tile([128, 8, 64], F16,
                                  name=f"msg{tag}{g}_{s}", tag="msg")
                    nc.vector.tensor_tensor(msg[:, 0:nact, :],
                                            gt[:, 0:nact, :],
                                            svk[:, 0:nact, 64:128],
                                            op=OP.mult)

                    def mk(flex, oh_t, msg, g=g, gr=gr, agg=agg,
                           ohg_t=ohg_t, is_last=(s == nsl - 1),
                           sc_flags=sc_flags):
                        def emit():
                            if flex is None:
                                nc.tensor.matmul(agg[:, 0:gr["n_b"], :],
                                                 i8_t[:],
                                                 msg[:, 0:gr["n_b"], :],
                                                 **sc_flags())
                            else:
                                for j, (i, _) in enumerate(flex):
                                    nc.tensor.matmul(
                                        agg[:, i, :],
                                        oh_t[:, j * 128:(j + 1) * 128],
                                        msg[:, i, :], **sc_flags())
                            if is_last:
                                mk_pool(g, gr, agg, ohg_t)
                        return emit

                    pend_sc.append(mk(flex,
                                      oh_t if flex is not None else None,
                                      msg))
                cur[0] += 1
                flush_sc(0)
                cur[0] += 1
                flush_pool()
                while pend_pool:
                    pend_pool.pop(0)[1]()

            if os.environ.get("KONLYREL") != "b":
                relation("c", 0)
            if os.environ.get("KONLYREL") != "c":
                relation("b", 64)

            # --- head: AllReduce pooled, divide by counts, transpose, MLP ---
            bounce_in = dp.tile([128, 128], F32, tag="bounce_in")
            bounce_out = dp.tile([128, 128], F32, tag="bounce_out")
            nc.sync.dma_start(bounce_in[:], pooled_sb[:])
            if not os.environ.get("KNOCOLL"):
                nc.gpsimd.collective_compute(
                    "AllReduce", OP.add,
                    replica_groups=[list(range(NCORES))],
                    ins=[bounce_in.opt()], outs=[bounce_out.opt()])
                nc.sync.dma_start(pooled_sb[:], bounce_out[:])
            mean16 = wp.tile([128, 128], F16, tag="mean16")
            nc.vector.tensor_tensor(mean16[:], pooled_sb[:], recip_t[:],
                                    op=OP.mult)
            tps = aggp.tile([128, 128], F16, tag="agg")
            nc.tensor.transpose(tps[:], mean16[:], i16_t[:])
            mean_sb = wp.tile([128, 128], F16, tag="mean_sb")
            nc.vector.tensor_copy(mean_sb[:], tps[:])

            mw, mb = {}, {}
            for k in ("W1", "W2", "W3", "Wout"):
                mw[k] = cp.tile(list(pk["mlp"][k].shape), F16, name=f"mw{k}",
                                tag=f"mw{k}")
                nc.sync.dma_start(mw[k][:], h["mlp_" + k].ap())
            for k in ("b1", "b2", "b3", "bout"):
                mb[k] = cp.tile(list(pk["mlp"][k].shape), F32, name=f"mb{k}",
                                tag=f"mb{k}")
                nc.sync.dma_start(mb[k][:], h["mlp_" + k].ap())

            hcur = mean_sb
            for li, (wk, bk) in enumerate((("W1", "b1"), ("W2", "b2"),
                                           ("W3", "b3"))):
                ps = aggp.tile([64, G], F32, name=f"mlp{li}", tag="agg")
                nc.tensor.matmul(ps[:], mw[wk][:], hcur[:],
                                 start=True, stop=True)
                hn = wp.tile([64, G], F16, name=f"hn{li}", tag=f"hn{li}")
                nc.scalar.activation(hn[:], ps[:], AF.Relu, bias=mb[bk][:])
                hcur = hn
            ps_o = aggp.tile([1, G], F32, tag="agg")
            nc.tensor.matmul(ps_o[:], mw["Wout"][:], hcur[:],
                             start=True, stop=True)
            osb = wp.tile([1, G], F32, tag="osb")
            nc.scalar.activation(osb[:], ps_o[:], AF.Identity,
                                 bias=mb["bout"][:])
            nc.sync.dma_start(out_h.ap(), osb[:])

    nc.compile()

    in_maps = []
    for m in range(NCORES):
        im = {}
        for tag in ("c", "b"):
            rl = pk[tag]
            pc = rl["per_core"][m]
            im[f"xt_{tag}"] = pc["xt"]
            im[f"oh_{tag}"] = pc["oh"]
            im[f"pa_{tag}"] = pc["pa"]
            im[f"ohg_{tag}"] = pc["ohg"]
            im[f"w2_{tag}"] = rl["w2"]
            im[f"ws4_{tag}"] = rl["ws4"]
        im["i8"] = pk["ident8"]
        im["i16"] = pk["ident16"]
        im["recip2"] = pk["recip2"]
        im["zl"] = pk["zl"]
        im["zr"] = pk["zr"]
        im["z128"] = pk["z128"]
        for k, v in pk["mlp"].items():
            im["mlp_" + k] = v
        in_maps.append(im)

    trace = bool(os.environ.get("KERNEL_TRACE"))
    res = run_bass_kernel_spmd(nc, in_maps, core_ids=list(range(NCORES)),
                               trace=trace)
    global LAST_EXEC_NS
    LAST_EXEC_NS = res.exec_time_ns
    return res.results[0]["out"].reshape(G).astype(np.float32)


# revision 52
# speedup vs baseline: 3.5787x; 1.0230x over previous
"""Trainium2 Bass kernel for hetero-GNN (2x ResGatedGraphConv + segment-mean pooling + MLP).

Sharding: destination-node range per core; each core processes the edges whose
dst falls in its range. Host does index marshalling only; all model arithmetic
runs on device.

Device strategy ("degree rounds"):
  - dst nodes are grouped into 128-slot buckets; 8 buckets form a PSUM group
    whose aggregate [128 slots, 8*64] lives in one PSUM bank.
  - edges of a bucket are packed into R identity rounds (the j-th edge of
    slot p sits at row p of round j) plus <=F flex rounds (leftovers, with a
    host-built fp8 one-hot scatter matrix).
  - a slab = one round of each of the 8 buckets = 8 subtiles of 128 edges.
    Per slab: 8 fused matmuls [xt.T @ W_aug] (row-tiled concurrent pairs),
    one sigmoid (ACT), one gated multiply (DVE), and ONE identity-stationary
    matmul that scatter-adds all 8 subtiles into the group aggregate.
  - skip connection is folded into the same PSUM accumulation (a dummy zero
    matmul opens the accumulation group for the whole bank).
  - relu + pooling via per-bucket one-hot matmul into a [G, 2H] transposed
    pooled PSUM; AllReduce across 8 cores; small MLP head on device.
"""
import os
import sys
import types
import numpy as np
import ml_dtypes

F8NP = ml_dtypes.float8_e4m3fn

NCORES = 8
G = 128
H = 64
F = 16
NC_N = 100000
NB_N = 200000
GRP = 8          # buckets per PSUM group
LAST_EXEC_NS = None


def _install_ntff_shim():
    if 'antenv.axon_hooks' in sys.modules:
        return
    try:
        mod = types.ModuleType('antenv.axon_hooks')
        _h = [None]
        mod.set_axon_ntff_profile_hook = lambda h: _h.__setitem__(0, h)
        mod.get_axon_ntff_profile_hook = lambda: _h[0]
        sys.modules['antenv.axon_hooks'] = mod
        import antenv
        antenv.axon_hooks = mod
        from trn_agent_boot.trn_boot import _ntff_profile_via_ctypes
        mod.set_axon_ntff_profile_hook(
            _ntff_profile_via_ctypes('/opt/axon/libaxon_pjrt.so'))
    except Exception:
        pass


def _waug(ii, rel):
    Wq, Wv, Wk = ii[f"Wq_{rel}"], ii[f"Wv_{rel}"], ii[f"Wk_{rel}"]
    We = ii[f"We_{rel}"][0]
    bq, bv, bk, be = (ii[f"bq_{rel}"], ii[f"bv_{rel}"],
                      ii[f"bk_{rel}"], ii[f"be_{rel}"])
    w = np.zeros((35, 128), np.float32)
    w[0:16, 0:64] = Wq
    w[0:16, 64:128] = Wv
    w[16, 0:64] = 2 * We
    w[16, 64:128] = We
    w[17, 0:64] = bq + bk + 2 * be
    w[17, 64:128] = bv + be
    w[18:34, 0:64] = Wk
    return w


def pack_relation(xs, xd, src, dst, ea, D, ii, rel, batch_dst):
    """Host marshalling for one relation.

    Returns common schedule + per-core device arrays."""
    E = len(src)
    nbuck = (D + 127) // 128
    order = np.argsort(dst, kind="stable")
    src_s, dst_s, ea_s = src[order], dst[order], ea[order]
    core = dst_s // D
    loc = dst_s % D
    buck = loc // 128
    slot = loc % 128
    lin = (core * nbuck + buck) * 128 + slot
    deg = np.bincount(lin, minlength=NCORES * nbuck * 128) \
            .reshape(NCORES, nbuck, 128)
    starts = np.searchsorted(dst_s, dst_s, side="left")
    rank = np.arange(E) - starts

    # common per-bucket-position R (identity rounds): minimize
    # R + w*max_core(F).  Flex subtiles cost more than identity rounds
    # (extra one-hot DMA + per-subtile scatter matmul), so weight them and
    # prefer the larger R on ties.
    maxd = int(deg.max())
    bestT = np.full(nbuck, np.inf)
    bestR = np.zeros(nbuck, np.int64)
    for R in range(0, maxd + 1):
        lo = np.maximum(deg - R, 0).sum(-1)            # [NCORES, nbuck]
        Fk = (-(-lo // 128)).max(0)                    # [nbuck]
        T = R + 1.8 * Fk
        upd = T <= bestT
        bestT[upd] = T[upd]
        bestR[upd] = R

    # group buckets (sorted by R desc) into chunks of GRP
    border = np.argsort(-bestR, kind="stable")
    ngroups = (nbuck + GRP - 1) // GRP
    groups = []
    bucket_group = np.zeros(nbuck, np.int64)   # bucket -> group
    bucket_pos = np.zeros(nbuck, np.int64)     # bucket -> index in group
    bucket_Rs = np.zeros(nbuck, np.int64)      # bucket -> group R*
    xt_col = 0
    oh_blk = 0
    for g in range(ngroups):
        bks = border[g * GRP:(g + 1) * GRP]
        n_b = len(bks)
        Rs = int(bestR[bks].max()) if n_b else 0
        # leftovers recomputed at group R*; order buckets by flex count so
        # each flex slab's present subtiles form a prefix
        lo2 = np.maximum(deg[:, bks, :] - Rs, 0).sum(-1)   # [NCORES, n_b]
        Fk = (-(-lo2 // 128)).max(0)                       # [n_b]
        perm = np.argsort(-Fk, kind="stable")
        bks = bks[perm]
        Fk = Fk[perm]
        Fs = int(Fk.max()) if n_b else 0
        flex = []
        oh_idx = {}
        for f in range(Fs):
            present = [(i, 0) for i in range(n_b) if Fk[i] > f]
            present = [(i, oh_blk + j) for j, (i, _) in enumerate(present)]
            for i, ob in present:
                oh_idx[(i, f)] = ob
            oh_blk += len(present)
            flex.append(present)
        bucket_group[bks] = g
        bucket_pos[bks] = np.arange(n_b)
        bucket_Rs[bks] = Rs
        groups.append({
            "n_b": n_b, "R": Rs, "F": Fs, "bks": bks, "Fk": Fk,
            "flex": flex, "oh_idx": oh_idx, "xt_off": xt_col,
        })
        xt_col += (Rs + Fs) * 512
    XC = max(xt_col, 512)
    OC = max(oh_blk * 128, 128)

    # per-edge destination column in xt (per core arrays share the schedule)
    # xt layout: [64, nsub*128]; subtile (group g, slab s, pos i) at column
    # block (xt_off/512)*8 + s*8 + i  (xt_off counts 512-col slab units).
    g_of = bucket_group[buck]
    i_of = bucket_pos[buck]
    Rs_of = bucket_Rs[buck]
    xoff_of = np.array([gr["xt_off"] for gr in groups], np.int64)[g_of]
    suboff_of = xoff_of // 512 * 8
    is_id = rank < Rs_of
    col = np.zeros(E, np.int64)
    col[is_id] = ((suboff_of[is_id] + rank[is_id] * 8 + i_of[is_id]) * 128
                  + slot[is_id])
    # flex: position among the bucket's leftover edges (dst-sorted order)
    lx = ~is_id
    lin_lx = lin[lx] // 128      # (core,bucket) linear id of leftover edges
    first = np.searchsorted(lin_lx, lin_lx, side="left")
    fpos = np.arange(lx.sum()) - first
    f_of = fpos // 128
    row = fpos % 128
    col[lx] = ((suboff_of[lx] + (Rs_of[lx] + f_of) * 8 + i_of[lx]) * 128
               + row)
    # oh block index for flex edges
    ohmap = np.full((nbuck, 32), -1, np.int64)
    for gr in groups:
        for (i, f), ob in gr["oh_idx"].items():
            ohmap[gr["bks"][i], f] = ob
    oh_of = np.zeros(E, np.int64)
    oh_of[lx] = ohmap[buck[lx], f_of]
    assert (oh_of[lx] >= 0).all()
    flexrow = np.zeros(E, np.int64)
    flexrow[lx] = row

    xsT = xs.astype(np.float32)
    xdT = xd.astype(np.float32)
    per_core = []
    cb = np.searchsorted(core, np.arange(NCORES + 1))
    for m in range(NCORES):
        s0, s1 = cb[m], cb[m + 1]
        c_src, c_dst = src_s[s0:s1], dst_s[s0:s1]
        c_ea, c_col = ea_s[s0:s1], col[s0:s1]
        c_lx = lx[s0:s1]
        c_oh = oh_of[s0:s1]
        c_fr = flexrow[s0:s1]
        c_slot = slot[s0:s1]
        xt = np.zeros((128, XC * 2), np.float32)
        xt[0:16, c_col] = xsT[c_src].T
        xt[16, c_col] = c_ea
        xt[17, c_col] = 1.0
        xt[18:34, c_col] = xdT[c_dst].T
        # flex one-hots: edge at (oh block, row) -> slot
        oh = np.zeros((128, OC), np.float32)
        oh[c_fr[c_lx], c_oh[c_lx] * 128 + c_slot[c_lx]] = 1.0
        # pa (skip lhsT, [32, nbuck*128]) + ohg (pooling one-hot)
        PC = ngroups * GRP * 128
        GC = ngroups * GRP * 128
        pa = np.zeros((32, PC), np.float32)
        ohg = np.zeros((128, GC), np.float32)
        for g, gr in enumerate(groups):
            for i, k in enumerate(gr["bks"]):
                base = m * D + k * 128
                w = min(128, D - k * 128)
                nodes = np.arange(base, base + w)
                cblk = (g * GRP + i) * 128
                pa[0:16, cblk:cblk + w] = xdT[nodes].T
                pa[16, cblk:cblk + w] = 1.0
                bt = batch_dst[nodes]
                ohg[np.arange(w), cblk + bt] = 1.0
        f8 = np.float16 if os.environ.get("KF16") else F8NP
        per_core.append({
            "xt": xt.astype(f8),
            "oh": oh.astype(f8),
            "pa": pa.astype(np.float16),
            "ohg": ohg.astype(f8),
        })

    w2 = np.zeros((128, 128), np.float32)
    w2[0:35] = _waug(ii, rel)
    ws4 = np.zeros((32, 64), np.float32)
    ws4[0:16] = ii[f"Wskip_{rel}"]
    ws4[16] = ii[f"bconv_{rel}"]
    nslabs = sum(gr["R"] + gr["F"] for gr in groups)
    return {
        "groups": groups, "XC": XC, "OC": OC, "ngroups": ngroups,
        "per_core": per_core, "w2": w2.astype(np.float16),
        "ws4": ws4.astype(np.float16), "nslabs": nslabs, "D": D,
        "nbuck": nbuck,
    }


def pack_all(ii):
    Dc, Db = NC_N // NCORES, NB_N // NCORES
    rel_c = pack_relation(ii["x_x"], ii["x_c"], ii["src_ac"].astype(np.int64),
                          ii["dst_ac"].astype(np.int64),
                          np.asarray(ii["ea_ac"])[:, 0], Dc, ii, "ac",
                          ii["batch_c"].astype(np.int64))
    rel_b = pack_relation(ii["x_c"], ii["x_b"], ii["src_cb"].astype(np.int64),
                          ii["dst_cb"].astype(np.int64),
                          np.asarray(ii["ea_cb"])[:, 0], Db, ii, "cb",
                          ii["batch_b"].astype(np.int64))

    cnt_c = np.bincount(ii["batch_c"].astype(np.int64), minlength=G)
    cnt_b = np.bincount(ii["batch_b"].astype(np.int64), minlength=G)
    recip2 = np.zeros((G, 128), np.float32)
    recip2[:, 0:64] = (1.0 / np.maximum(cnt_c, 1))[:, None]
    recip2[:, 64:128] = (1.0 / np.maximum(cnt_b, 1))[:, None]

    mlp = {
        "W1": ii["W1"].astype(np.float16), "W2": ii["W2"].astype(np.float16),
        "W3": ii["W3"].astype(np.float16),
        "Wout": ii["Wout"].astype(np.float16),
        "b1": np.asarray(ii["b1"], np.float32).reshape(64, 1),
        "b2": np.asarray(ii["b2"], np.float32).reshape(64, 1),
        "b3": np.asarray(ii["b3"], np.float32).reshape(64, 1),
        "bout": np.asarray(ii["bout"], np.float32).reshape(1, 1),
    }
    f8 = np.float16 if os.environ.get("KF16") else F8NP
    ident8 = np.eye(128, dtype=f8)
    ident16 = np.eye(128, dtype=np.float16)
    zl = np.zeros((1, 128), np.float16)
    zr = np.zeros((1, 512), np.float16)
    z128 = np.zeros((128, 128), np.float32)
    return {"c": rel_c, "b": rel_b, "recip2": recip2.astype(np.float16),
            "mlp": mlp, "ident8": ident8, "ident16": ident16,
            "zl": zl, "zr": zr, "z128": z128}


def emulate(ii):
    """Numpy emulation of the device program (for packing validation)."""
    pk = pack_all(ii)
    pooled = np.zeros((G, 128), np.float64)
    for tag in ("c", "b"):
        rl = pk[tag]
        w2 = rl["w2"].astype(np.float32)
        ws4 = rl["ws4"].astype(np.float32)
        for m in range(NCORES):
            pc = rl["per_core"][m]
            xt = pc["xt"].astype(np.float32)
            oh = pc["oh"].astype(np.float32)
            pa = pc["pa"].astype(np.float32)
            ohg = pc["ohg"].astype(np.float32)
            for g, gr in enumerate(rl["groups"]):
                n_b, Rs, Fs = gr["n_b"], gr["R"], gr["F"]
                agg = np.zeros((128, n_b, 64), np.float32)
                for i in range(n_b):
                    cblk = (g * GRP + i) * 128
                    lhs = pa[0:17, cblk:cblk + 128]
                    agg[:, i, :] += lhs.T @ ws4[0:17]
                for s in range(Rs + Fs):
                    c0 = gr["xt_off"] * 2 + s * 1024
                    blkx = xt[:, c0:c0 + 1024]
                    sv = np.zeros((128, 8, 128), np.float32)
                    for i in range(8):
                        sv[:, i, :] = (blkx[:, i * 128:(i + 1) * 128].T @ w2)
                    gt = (1.0 / (1.0 + np.exp(-sv[:, :, 0:64]))) \
                        .astype(np.float16).astype(np.float32)
                    msg = (gt * sv[:, :, 64:128]).astype(np.float16) \
                        .astype(np.float32)
                    if s < Rs:
                        agg += msg[:, :n_b, :]
                    else:
                        for (i, ob) in gr["flex"][s - Rs]:
                            ohb = oh[:, ob * 128:(ob + 1) * 128]
                            agg[:, i, :] += ohb.T @ msg[:, i, :]
                h = np.maximum(agg, 0.0).astype(np.float16).astype(np.float32)
                off = 0 if tag == "c" else 64
                for i in range(n_b):
                    ohgb = ohg[:, (g * GRP + i) * 128:(g * GRP + i + 1) * 128]
                    pooled[:, off:off + 64] += ohgb.T @ h[:, i, :]
    mean = pooled * pk["recip2"].astype(np.float64)
    hcur = mean.T.astype(np.float32)          # [2H, G]
    mlp = pk["mlp"]
    for wk, bk in (("W1", "b1"), ("W2", "b2"), ("W3", "b3")):
        hcur = np.maximum(mlp[wk].astype(np.float32).T @ hcur + mlp[bk], 0.0)
    out = mlp["Wout"].astype(np.float32).T @ hcur + mlp["bout"]
    return out.reshape(G)


def kernel(**inputs):
    _install_ntff_shim()
    import concourse.bass as bass  # noqa: F401
    import concourse.bacc as bacc
    import concourse.mybir as mybir
    import concourse.tile as tile
    from concourse.bass_utils import run_bass_kernel_spmd

    F32 = mybir.dt.float32
    F16 = mybir.dt.float16
    FP8 = F16 if os.environ.get("KF16") else mybir.dt.float8e4
    AF = mybir.ActivationFunctionType
    OP = mybir.AluOpType

    ii = {k: np.asarray(v) for k, v in inputs.items()}
    pk = pack_all(ii)

    nc = bacc.Bacc("TRN2", target_bir_lowering=False, debug=False,
                   num_devices=NCORES)

    def din(name, arr0):
        return nc.dram_tensor(name, list(arr0.shape),
                              mybir.dt.from_np(arr0.dtype),
                              kind="ExternalInput")

    h = {}
    for tag in ("c", "b"):
        rl = pk[tag]
        pc0 = rl["per_core"][0]
        h[f"xt_{tag}"] = din(f"xt_{tag}", pc0["xt"])
        h[f"oh_{tag}"] = din(f"oh_{tag}", pc0["oh"])
        h[f"pa_{tag}"] = din(f"pa_{tag}", pc0["pa"])
        h[f"ohg_{tag}"] = din(f"ohg_{tag}", pc0["ohg"])
        h[f"w2_{tag}"] = din(f"w2_{tag}", rl["w2"])
        h[f"ws4_{tag}"] = din(f"ws4_{tag}", rl["ws4"])
    h["i8"] = din("i8", pk["ident8"])
    h["i16"] = din("i16", pk["ident16"])
    h["recip2"] = din("recip2", pk["recip2"])
    h["zl"] = din("zl", pk["zl"])
    h["zr"] = din("zr", pk["zr"])
    h["z128"] = din("z128", pk["z128"])
    for k, v in pk["mlp"].items():
        h["mlp_" + k] = din("mlp_" + k, v)
    out_h = nc.dram_tensor("out", [1, G], F32, kind="ExternalOutput")

    with tile.TileContext(nc) as tc:
        with tc.tile_pool(name="const", bufs=1) as cp, \
             tc.tile_pool(name="stream", bufs=3) as sp, \
             tc.tile_pool(name="work", bufs=3) as wp, \
             tc.tile_pool(name="svp", bufs=3, space="PSUM") as svp, \
             tc.tile_pool(name="aggp", bufs=2, space="PSUM") as aggp, \
             tc.tile_pool(name="dram", bufs=1, space="DRAM") as dp:

            i8_t = cp.tile([128, 128], FP8, tag="i8")
            nc.sync.dma_start(i8_t[:], h["i8"].ap())
            i16_t = cp.tile([128, 128], F16, tag="i16")
            nc.sync.dma_start(i16_t[:], h["i16"].ap())
            recip_t = cp.tile([128, 128], F16, tag="recip2")
            nc.sync.dma_start(recip_t[:], h["recip2"].ap())
            zl = cp.tile([1, 128], F16, tag="zl")
            nc.sync.dma_start(zl[:], h["zl"].ap())
            zr = cp.tile([1, 512], F16, tag="zr")
            nc.sync.dma_start(zr[:], h["zr"].ap())

            pooled_r = {}
            for rtag in ("c", "b"):
                pooled_r[rtag] = wp.tile([128, 64], F32, name=f"pool{rtag}",
                                         tag=f"pool{rtag}")
                nc.sync.dma_start(pooled_r[rtag][:], h["z128"].ap()[:, 0:64])

            slab_ctr = [0]

            def relation(tag, col_off):
                rl = pk[tag]
                w2_t = cp.tile([128, 128], F16, tag=f"w2{tag}")
                nc.sync.dma_start(w2_t[:], h[f"w2_{tag}"].ap())
                ws4_t = cp.tile([32, 64], F16, tag=f"ws4{tag}")
                nc.sync.dma_start(ws4_t[:], h[f"ws4_{tag}"].ap())
                xt_v = h[f"xt_{tag}"].ap()
                oh_v = h[f"oh_{tag}"].ap()
                pa_v = h[f"pa_{tag}"].ap()
                ohg_v = h[f"ohg_{tag}"].ap()
                first_pool = [True]
                ngroups = rl["ngroups"]
                pa_w = GRP * 128
                jobs = []
                for g, gr in enumerate(rl["groups"]):
                    if gr["n_b"] == 0:
                        continue
                    for s in range(gr["R"] + gr["F"]):
                        jobs.append((g, gr, s))
                last_g = jobs[-1][0]
                # software pipeline: scatter(slab k) emits after MM1s(k+2)
                # (2-deep: the sigmoid+mult chain is longer than one slab);
                # relu(group) emits with its last scatter; pooling(group)
                # defers one more slab so PE never waits on ACT.
                pend_sc = []
                pend_pool = []     # list of [due_idx, fn]
                cur = [0]

                def flush_sc(depth=1):
                    while len(pend_sc) > depth:
                        pend_sc.pop(0)()

                def flush_pool():
                    while pend_pool and pend_pool[0][0] <= cur[0]:
                        pend_pool.pop(0)[1]()

                def mk_pool(g, gr, agg, ohg_t):
                    n_b = gr["n_b"]
                    h_sb = wp.tile([128, GRP, 64], F16, name=f"h{tag}{g}",
                                   tag="hsb")
                    nc.scalar.activation(h_sb[:, 0:n_b, :],
                                         agg[:, 0:n_b, :], AF.Relu)

                    def pool():
                        gpool = svp.tile([128, 64], F32, name=f"gp{tag}{g}",
                                         tag="sv")
                        for i in range(n_b):
                            nc.tensor.matmul(
                                gpool[:],
                                ohg_t[:, i * 128:(i + 1) * 128],
                                h_sb[:, i, :],
                                start=(i == 0), stop=(i == n_b - 1),
                                skip_group_check=True)
                        nc.vector.tensor_tensor(
                            pooled_r[tag][:], pooled_r[tag][:],
                            gpool[:], op=OP.add)
                    pend_pool.append([cur[0] + 1, pool])

                st = {}
                xt2 = [None, 0]
                for idx, (g, gr, s) in enumerate(jobs):
                    cur[0] = idx
                    n_b, Rs, Fs = gr["n_b"], gr["R"], gr["F"]
                    nsl = Rs + Fs
                    if s == 0:
                        pa_t = sp.tile([32, pa_w], F16, name=f"pa{tag}{g}",
                                       tag="pa")
                        nc.sync.dma_start(pa_t[:],
                                          pa_v[:, g * pa_w:(g + 1) * pa_w])
                        ohg_t = sp.tile([128, GRP * 128], FP8,
                                        name=f"ohg{tag}{g}", tag="ohg")
                        nc.sync.dma_start(
                            ohg_t[:, 0:n_b * 128],
                            ohg_v[:, g * GRP * 128:
                                  g * GRP * 128 + n_b * 128])
                        agg = aggp.tile([128, GRP, 64], F32,
                                        name=f"agg{tag}{g}", tag="agg")
                        nc.tensor.matmul(agg[:, 0:n_b, :], zl[:],
                                         zr[:, 0:n_b * 64], start=True,
                                         stop=False, skip_group_check=True)
                        for i in range(n_b):
                            nc.tensor.matmul(
                                agg[:, i, :],
                                pa_t[0:17, i * 128:i * 128 + 128],
                                ws4_t[0:17, :],
                                start=False, stop=False,
                                skip_group_check=True)
                        nsc = Rs + sum(len(p) for p in gr["flex"])
                        st[g] = {"agg": agg, "ohg_t": ohg_t, "sci": [0],
                                 "nsc": nsc}
                    sg = st[g]
                    agg, ohg_t = sg["agg"], sg["ohg_t"]

                    def sc_flags(sg=sg):
                        sg["sci"][0] += 1
                        return {"start": False,
                                "stop": sg["sci"][0] == sg["nsc"],
                                "skip_group_check": True}

                    # xt DMA batched over slab pairs (within the group)
                    if s % 2 == 0:
                        wcols = min(2, nsl - s) * 1024
                        c0 = gr["xt_off"] * 2 + s * 1024
                        xt2[0] = sp.tile([128, 2048], FP8,
                                         name=f"xt{tag}{g}_{s}", tag="xt")
                        nc.sync.dma_start(xt2[0][:, 0:wcols],
                                          xt_v[:, c0:c0 + wcols])
                    xtsl = xt2[0][:, (s % 2) * 1024:(s % 2) * 1024 + 1024]
                    flex = None
                    if s >= Rs:
                        flex = gr["flex"][s - Rs]
                        ob0 = flex[0][1]
                        obn = len(flex)
                        oh_t = sp.tile([128, GRP * 128], FP8,
                                       name=f"oh{tag}{g}_{s}", tag="ohf")
                        nc.sync.dma_start(
                            oh_t[:, 0:obn * 128],
                            oh_v[:, ob0 * 128:(ob0 + obn) * 128])
                        present = set(i for i, _ in flex)
                    svk = svp.tile([128, 8, 128], F32,
                                   name=f"sv{tag}{g}_{s}", tag="sv")
                    nact = 8 if flex is None else len(flex)
                    for i in range(8):
                        if flex is not None and i not in present:
                            continue
                        nc.tensor.matmul(
                            svk[:, i, :],
                            xtsl[:, i * 128:(i + 1) * 128],
                            w2_t[:],
                            start=True, stop=True)
                    flush_sc(1)  # scatter of slab idx-2 runs behind our MM1s
                    flush_pool()
                    gt = wp.tile([128, 8, 64], F16,
                                 name=f"gt{tag}{g}_{s}", tag="gt")
                    nc.scalar.activation(gt[:, 0:nact, :],
                                         svk[:, 0:nact, 0:64], AF.Sigmoid)
                    msg = wp.tile([128, 8, 64], F16,
                                  name=f"msg{tag}{g}_{s}", tag="msg")
                    nc.vector.tensor_tensor(msg[:, 0:nact, :],
                                            gt[:, 0:nact, :],
                                            svk[:, 0:nact, 64:128],
                                            op=OP.mult)

                    def mk(flex, oh_t, msg, g=g, gr=gr, agg=agg,
                           ohg_t=ohg_t, is_last=(s == nsl - 1),
                           sc_flags=sc_flags):
                        def emit():
                            if flex is None:
                                nc.tensor.matmul(agg[:, 0:gr["n_b"], :],
                                                 i8_t[:],
                                                 msg[:, 0:gr["n_b"], :],
                                                 **sc_flags())
                            else:
                                for j, (i, _) in enumerate(flex):
                                    nc.tensor.matmul(
                                        agg[:, i, :],
                                        oh_t[:, j * 128:(j + 1) * 128],
                                        msg[:, i, :], **sc_flags())
                            if is_last:
                                mk_pool(g, gr, agg, ohg_t)
                        return emit

                    pend_sc.append(mk(flex,
                                      oh_t if flex is not None else None,
                                      msg))
                cur[0] += 1
                flush_sc(0)
                cur[0] += 1
                flush_pool()
                while pend_pool:
                    pend_pool.pop(0)[1]()

            def do_collective(rtag):
                bi = dp.tile([128, 64], F32, name=f"bi{rtag}",
                             tag=f"bi{rtag}")
                bo = dp.tile([128, 64], F32, name=f"bo{rtag}",
                             tag=f"bo{rtag}")
                nc.sync.dma_start(bi[:], pooled_r[rtag][:])
                nc.gpsimd.collective_compute(
                    "AllReduce", OP.add,
                    replica_groups=[list(range(NCORES))],
                    ins=[bi.opt()], outs=[bo.opt()])
                nc.sync.dma_start(pooled_r[rtag][:], bo[:])

            relation("c", 0)
            do_collective("c")   # overlaps relation b's compute
            relation("b", 64)
            do_collective("b")

            # --- head: divide by counts, transpose, MLP ---
            mean16 = wp.tile([128, 128], F16, tag="mean16")
            nc.vector.tensor_tensor(mean16[:, 0:64], pooled_r["c"][:],
                                    recip_t[:, 0:64], op=OP.mult)
            nc.vector.tensor_tensor(mean16[:, 64:128], pooled_r["b"][:],
                                    recip_t[:, 64:128], op=OP.mult)
            tps = aggp.tile([128, 128], F16, tag="agg")
            nc.tensor.transpose(tps[:], mean16[:], i16_t[:])
            mean_sb = wp.tile([128, 128], F16, tag="mean_sb")
            nc.vector.tensor_copy(mean_sb[:], tps[:])

            mw, mb = {}, {}
            for k in ("W1", "W2", "W3", "Wout"):
                mw[k] = cp.tile(list(pk["mlp"][k].shape), F16, name=f"mw{k}",
                                tag=f"mw{k}")
                nc.sync.dma_start(mw[k][:], h["mlp_" + k].ap())
            for k in ("b1", "b2", "b3", "bout"):
                mb[k] = cp.tile(list(pk["mlp"][k].shape), F32, name=f"mb{k}",
                                tag=f"mb{k}")
                nc.sync.dma_start(mb[k][:], h["mlp_" + k].ap())

            hcur = mean_sb
            for li, (wk, bk) in enumerate((("W1", "b1"), ("W2", "b2"),
                                           ("W3", "b3"))):
                ps = aggp.tile([64, G], F32, name=f"mlp{li}", tag="agg")
                nc.tensor.matmul(ps[:], mw[wk][:], hcur[:],
                                 start=True, stop=True)
                hn = wp.tile([64, G], F16, name=f"hn{li}", tag=f"hn{li}")
                nc.scalar.activation(hn[:], ps[:], AF.Relu, bias=mb[bk][:])
                hcur = hn
            ps_o = aggp.tile([1, G], F32, tag="agg")
            nc.tensor.matmul(ps_o[:], mw["Wout"][:], hcur[:],
                             start=True, stop=True)
            osb = wp.tile([1, G], F32, tag="osb")
            nc.scalar.activation(osb[:], ps_o[:], AF.Identity,
                                 bias=mb["bout"][:])
            nc.sync.dma_start(out_h.ap(), osb[:])

    nc.compile()

    in_maps = []
    for m in range(NCORES):
        im = {}
        for tag in ("c", "b"):
            rl = pk[tag]
            pc = rl["per_core"][m]
            im[f"xt_{tag}"] = pc["xt"]
            im[f"oh_{tag}"] = pc["oh"]
            im[f"pa_{tag}"] = pc["pa"]
            im[f"ohg_{tag}"] = pc["ohg"]
            im[f"w2_{tag}"] = rl["w2"]
            im[f"ws4_{tag}"] = rl["ws4"]
        im["i8"] = pk["ident8"]
        im["i16"] = pk["ident16"]
        im["recip2"] = pk["recip2"]
        im["zl"] = pk["zl"]
        im["zr"] = pk["zr"]
        im["z128"] = pk["z128"]
        for k, v in pk["mlp"].items():
            im["mlp_" + k] = v
        in_maps.append(im)

    trace = bool(os.environ.get("KERNEL_TRACE"))
    res = run_bass_kernel_spmd(nc, in_maps, core_ids=list(range(NCORES)),
                               trace=trace)
    global LAST_EXEC_NS
    LAST_EXEC_NS = res.exec_time_ns
    return res.results[0]["out"].reshape(G).astype(np.float32)


# revision 55
# speedup vs baseline: 4.0374x; 1.1282x over previous
"""Trainium2 Bass kernel for hetero-GNN (2x ResGatedGraphConv + segment-mean pooling + MLP).

Sharding: destination-node range per core; each core processes the edges whose
dst falls in its range. Host does index marshalling only; all model arithmetic
runs on device.

Device strategy ("degree rounds"):
  - dst nodes are grouped into 128-slot buckets; 8 buckets form a PSUM group
    whose aggregate [128 slots, 8*64] lives in one PSUM bank.
  - edges of a bucket are packed into R identity rounds (the j-th edge of
    slot p sits at row p of round j) plus <=F flex rounds (leftovers, with a
    host-built fp8 one-hot scatter matrix).
  - a slab = one round of each of the 8 buckets = 8 subtiles of 128 edges.
    Per slab: 8 fused matmuls [xt.T @ W_aug] (row-tiled concurrent pairs),
    one sigmoid (ACT), one gated multiply (DVE), and ONE identity-stationary
    matmul that scatter-adds all 8 subtiles into the group aggregate.
  - skip connection is folded into the same PSUM accumulation (a dummy zero
    matmul opens the accumulation group for the whole bank).
  - relu + pooling via per-bucket one-hot matmul into a [G, 2H] transposed
    pooled PSUM; AllReduce across 8 cores; small MLP head on device.
"""
import os
import sys
import types
import numpy as np
import ml_dtypes

F8NP = ml_dtypes.float8_e4m3fn

NCORES = 8
G = 128
H = 64
F = 16
NC_N = 100000
NB_N = 200000
GRP = 8          # buckets per PSUM group
LAST_EXEC_NS = None


def _install_ntff_shim():
    if 'antenv.axon_hooks' in sys.modules:
        return
    try:
        mod = types.ModuleType('antenv.axon_hooks')
        _h = [None]
        mod.set_axon_ntff_profile_hook = lambda h: _h.__setitem__(0, h)
        mod.get_axon_ntff_profile_hook = lambda: _h[0]
        sys.modules['antenv.axon_hooks'] = mod
        import antenv
        antenv.axon_hooks = mod
        from trn_agent_boot.trn_boot import _ntff_profile_via_ctypes
        mod.set_axon_ntff_profile_hook(
            _ntff_profile_via_ctypes('/opt/axon/libaxon_pjrt.so'))
    except Exception:
        pass


def _waug(ii, rel):
    Wq, Wv, Wk = ii[f"Wq_{rel}"], ii[f"Wv_{rel}"], ii[f"Wk_{rel}"]
    We = ii[f"We_{rel}"][0]
    bq, bv, bk, be = (ii[f"bq_{rel}"], ii[f"bv_{rel}"],
                      ii[f"bk_{rel}"], ii[f"be_{rel}"])
    w = np.zeros((35, 128), np.float32)
    w[0:16, 0:64] = Wq
    w[0:16, 64:128] = Wv
    w[16, 0:64] = 2 * We
    w[16, 64:128] = We
    w[17, 0:64] = bq + bk + 2 * be
    w[17, 64:128] = bv + be
    w[18:34, 0:64] = Wk
    return w


def pack_relation(xs, xd, src, dst, ea, D, ii, rel, batch_dst):
    """Host marshalling for one relation.

    Returns common schedule + per-core device arrays."""
    E = len(src)
    nbuck = (D + 127) // 128
    # per-core degree-sorted slot permutation: rank dst nodes by degree so
    # each 128-slot bucket holds near-equal degrees (minimal round padding).
    core_all = dst // D
    loc_all = dst % D
    deg_node = np.bincount(core_all * D + loc_all,
                           minlength=NCORES * D).reshape(NCORES, D)
    order_nodes = np.argsort(-deg_node, axis=1, kind="stable")  # rank->loc
    rank_of_loc = np.empty((NCORES, D), np.int64)
    np.put_along_axis(rank_of_loc, order_nodes,
                      np.broadcast_to(np.arange(D), (NCORES, D)), axis=1)
    slot_all = rank_of_loc[core_all, loc_all]
    key = core_all * D + slot_all
    order = np.argsort(key, kind="stable")
    src_s, dst_s, ea_s = src[order], dst[order], ea[order]
    key_s = key[order]
    core = core_all[order]
    buck = (key_s % D) // 128
    slot = (key_s % D) % 128
    lin = (core * nbuck + buck) * 128 + slot
    deg = np.bincount(lin, minlength=NCORES * nbuck * 128) \
            .reshape(NCORES, nbuck, 128)
    starts = np.searchsorted(key_s, key_s, side="left")
    rank = np.arange(E) - starts

    # common per-bucket-position R (identity rounds): minimize
    # R + w*max_core(F).  Flex subtiles cost more than identity rounds
    # (extra one-hot DMA + per-subtile scatter matmul), so weight them and
    # prefer the larger R on ties.
    maxd = int(deg.max())
    bestT = np.full(nbuck, np.inf)
    bestR = np.zeros(nbuck, np.int64)
    for R in range(0, maxd + 1):
        lo = np.maximum(deg - R, 0).sum(-1)            # [NCORES, nbuck]
        Fk = (-(-lo // 128)).max(0)                    # [nbuck]
        T = R + 1.8 * Fk
        upd = T <= bestT
        bestT[upd] = T[upd]
        bestR[upd] = R

    # group buckets (sorted by R desc) into chunks of GRP
    border = np.argsort(-bestR, kind="stable")
    ngroups = (nbuck + GRP - 1) // GRP
    groups = []
    bucket_group = np.zeros(nbuck, np.int64)   # bucket -> group
    bucket_pos = np.zeros(nbuck, np.int64)     # bucket -> index in group
    bucket_Rs = np.zeros(nbuck, np.int64)      # bucket -> group R*
    xt_col = 0
    oh_blk = 0
    for g in range(ngroups):
        bks = border[g * GRP:(g + 1) * GRP]
        n_b = len(bks)
        Rs = int(bestR[bks].max()) if n_b else 0
        # leftovers recomputed at group R*; order buckets by flex count so
        # each flex slab's present subtiles form a prefix
        lo2 = np.maximum(deg[:, bks, :] - Rs, 0).sum(-1)   # [NCORES, n_b]
        Fk = (-(-lo2 // 128)).max(0)                       # [n_b]
        perm = np.argsort(-Fk, kind="stable")
        bks = bks[perm]
        Fk = Fk[perm]
        Fs = int(Fk.max()) if n_b else 0
        flex = []
        oh_idx = {}
        for f in range(Fs):
            present = [(i, 0) for i in range(n_b) if Fk[i] > f]
            present = [(i, oh_blk + j) for j, (i, _) in enumerate(present)]
            for i, ob in present:
                oh_idx[(i, f)] = ob
            oh_blk += len(present)
            flex.append(present)
        bucket_group[bks] = g
        bucket_pos[bks] = np.arange(n_b)
        bucket_Rs[bks] = Rs
        groups.append({
            "n_b": n_b, "R": Rs, "F": Fs, "bks": bks, "Fk": Fk,
            "flex": flex, "oh_idx": oh_idx, "xt_off": xt_col,
        })
        xt_col += (Rs + Fs) * 512
    XC = max(xt_col, 512)
    OC = max(oh_blk * 128, 128)

    # per-edge destination column in xt (per core arrays share the schedule)
    # xt layout: [64, nsub*128]; subtile (group g, slab s, pos i) at column
    # block (xt_off/512)*8 + s*8 + i  (xt_off counts 512-col slab units).
    g_of = bucket_group[buck]
    i_of = bucket_pos[buck]
    Rs_of = bucket_Rs[buck]
    xoff_of = np.array([gr["xt_off"] for gr in groups], np.int64)[g_of]
    suboff_of = xoff_of // 512 * 8
    is_id = rank < Rs_of
    col = np.zeros(E, np.int64)
    col[is_id] = ((suboff_of[is_id] + rank[is_id] * 8 + i_of[is_id]) * 128
                  + slot[is_id])
    # flex: position among the bucket's leftover edges (dst-sorted order)
    lx = ~is_id
    lin_lx = lin[lx] // 128      # (core,bucket) linear id of leftover edges
    first = np.searchsorted(lin_lx, lin_lx, side="left")
    fpos = np.arange(lx.sum()) - first
    f_of = fpos // 128
    row = fpos % 128
    col[lx] = ((suboff_of[lx] + (Rs_of[lx] + f_of) * 8 + i_of[lx]) * 128
               + row)
    # oh block index for flex edges
    ohmap = np.full((nbuck, 32), -1, np.int64)
    for gr in groups:
        for (i, f), ob in gr["oh_idx"].items():
            ohmap[gr["bks"][i], f] = ob
    oh_of = np.zeros(E, np.int64)
    oh_of[lx] = ohmap[buck[lx], f_of]
    assert (oh_of[lx] >= 0).all()
    flexrow = np.zeros(E, np.int64)
    flexrow[lx] = row

    xsT = xs.astype(np.float32)
    xdT = xd.astype(np.float32)
    per_core = []
    cb = np.searchsorted(core, np.arange(NCORES + 1))
    for m in range(NCORES):
        s0, s1 = cb[m], cb[m + 1]
        c_src, c_dst = src_s[s0:s1], dst_s[s0:s1]
        c_ea, c_col = ea_s[s0:s1], col[s0:s1]
        c_lx = lx[s0:s1]
        c_oh = oh_of[s0:s1]
        c_fr = flexrow[s0:s1]
        c_slot = slot[s0:s1]
        xt = np.zeros((128, XC * 2), np.float32)
        xt[0:16, c_col] = xsT[c_src].T
        xt[16, c_col] = c_ea
        xt[17, c_col] = 1.0
        xt[18:34, c_col] = xdT[c_dst].T
        # flex one-hots: edge at (oh block, row) -> slot
        oh = np.zeros((128, OC), np.float32)
        oh[c_fr[c_lx], c_oh[c_lx] * 128 + c_slot[c_lx]] = 1.0
        # pa (skip lhsT, [32, nbuck*128]) + ohg (pooling one-hot)
        PC = ngroups * GRP * 128
        GC = ngroups * GRP * 128
        pa = np.zeros((32, PC), np.float32)
        ohg = np.zeros((128, GC), np.float32)
        for g, gr in enumerate(groups):
            for i, k in enumerate(gr["bks"]):
                base = m * D + k * 128
                w = min(128, D - k * 128)
                nodes = np.arange(base, base + w)
                cblk = (g * GRP + i) * 128
                pa[0:16, cblk:cblk + w] = xdT[nodes].T
                pa[16, cblk:cblk + w] = 1.0
                bt = batch_dst[nodes]
                ohg[np.arange(w), cblk + bt] = 1.0
        f8 = np.float16 if os.environ.get("KF16") else F8NP
        per_core.append({
            "xt": xt.astype(f8),
            "oh": oh.astype(f8),
            "pa": pa.astype(np.float16),
            "ohg": ohg.astype(f8),
        })

    w2 = np.zeros((128, 128), np.float32)
    w2[0:35] = _waug(ii, rel)
    ws4 = np.zeros((32, 64), np.float32)
    ws4[0:16] = ii[f"Wskip_{rel}"]
    ws4[16] = ii[f"bconv_{rel}"]
    nslabs = sum(gr["R"] + gr["F"] for gr in groups)
    return {
        "groups": groups, "XC": XC, "OC": OC, "ngroups": ngroups,
        "per_core": per_core, "w2": w2.astype(np.float16),
        "ws4": ws4.astype(np.float16), "nslabs": nslabs, "D": D,
        "nbuck": nbuck,
    }


def pack_all(ii):
    Dc, Db = NC_N // NCORES, NB_N // NCORES
    rel_c = pack_relation(ii["x_x"], ii["x_c"], ii["src_ac"].astype(np.int64),
                          ii["dst_ac"].astype(np.int64),
                          np.asarray(ii["ea_ac"])[:, 0], Dc, ii, "ac",
                          ii["batch_c"].astype(np.int64))
    rel_b = pack_relation(ii["x_c"], ii["x_b"], ii["src_cb"].astype(np.int64),
                          ii["dst_cb"].astype(np.int64),
                          np.asarray(ii["ea_cb"])[:, 0], Db, ii, "cb",
                          ii["batch_b"].astype(np.int64))

    cnt_c = np.bincount(ii["batch_c"].astype(np.int64), minlength=G)
    cnt_b = np.bincount(ii["batch_b"].astype(np.int64), minlength=G)
    recip2 = np.zeros((G, 128), np.float32)
    recip2[:, 0:64] = (1.0 / np.maximum(cnt_c, 1))[:, None]
    recip2[:, 64:128] = (1.0 / np.maximum(cnt_b, 1))[:, None]

    mlp = {
        "W1": ii["W1"].astype(np.float16), "W2": ii["W2"].astype(np.float16),
        "W3": ii["W3"].astype(np.float16),
        "Wout": ii["Wout"].astype(np.float16),
        "b1": np.asarray(ii["b1"], np.float32).reshape(64, 1),
        "b2": np.asarray(ii["b2"], np.float32).reshape(64, 1),
        "b3": np.asarray(ii["b3"], np.float32).reshape(64, 1),
        "bout": np.asarray(ii["bout"], np.float32).reshape(1, 1),
    }
    f8 = np.float16 if os.environ.get("KF16") else F8NP
    ident8 = np.eye(128, dtype=f8)
    ident16 = np.eye(128, dtype=np.float16)
    zl = np.zeros((1, 128), np.float16)
    zr = np.zeros((1, 512), np.float16)
    z128 = np.zeros((128, 128), np.float32)
    return {"c": rel_c, "b": rel_b, "recip2": recip2.astype(np.float16),
            "mlp": mlp, "ident8": ident8, "ident16": ident16,
            "zl": zl, "zr": zr, "z128": z128}


def emulate(ii):
    """Numpy emulation of the device program (for packing validation)."""
    pk = pack_all(ii)
    pooled = np.zeros((G, 128), np.float64)
    for tag in ("c", "b"):
        rl = pk[tag]
        w2 = rl["w2"].astype(np.float32)
        ws4 = rl["ws4"].astype(np.float32)
        for m in range(NCORES):
            pc = rl["per_core"][m]
            xt = pc["xt"].astype(np.float32)
            oh = pc["oh"].astype(np.float32)
            pa = pc["pa"].astype(np.float32)
            ohg = pc["ohg"].astype(np.float32)
            for g, gr in enumerate(rl["groups"]):
                n_b, Rs, Fs = gr["n_b"], gr["R"], gr["F"]
                agg = np.zeros((128, n_b, 64), np.float32)
                for i in range(n_b):
                    cblk = (g * GRP + i) * 128
                    lhs = pa[0:17, cblk:cblk + 128]
                    agg[:, i, :] += lhs.T @ ws4[0:17]
                for s in range(Rs + Fs):
                    c0 = gr["xt_off"] * 2 + s * 1024
                    blkx = xt[:, c0:c0 + 1024]
                    sv = np.zeros((128, 8, 128), np.float32)
                    for i in range(8):
                        sv[:, i, :] = (blkx[:, i * 128:(i + 1) * 128].T @ w2)
                    gt = (1.0 / (1.0 + np.exp(-sv[:, :, 0:64]))) \
                        .astype(np.float16).astype(np.float32)
                    msg = (gt * sv[:, :, 64:128]).astype(np.float16) \
                        .astype(np.float32)
                    if s < Rs:
                        agg += msg[:, :n_b, :]
                    else:
                        for (i, ob) in gr["flex"][s - Rs]:
                            ohb = oh[:, ob * 128:(ob + 1) * 128]
                            agg[:, i, :] += ohb.T @ msg[:, i, :]
                h = np.maximum(agg, 0.0).astype(np.float16).astype(np.float32)
                off = 0 if tag == "c" else 64
                for i in range(n_b):
                    ohgb = ohg[:, (g * GRP + i) * 128:(g * GRP + i + 1) * 128]
                    pooled[:, off:off + 64] += ohgb.T @ h[:, i, :]
    mean = pooled * pk["recip2"].astype(np.float64)
    hcur = mean.T.astype(np.float32)          # [2H, G]
    mlp = pk["mlp"]
    for wk, bk in (("W1", "b1"), ("W2", "b2"), ("W3", "b3")):
        hcur = np.maximum(mlp[wk].astype(np.float32).T @ hcur + mlp[bk], 0.0)
    out = mlp["Wout"].astype(np.float32).T @ hcur + mlp["bout"]
    return out.reshape(G)


def kernel(**inputs):
    _install_ntff_shim()
    import concourse.bass as bass  # noqa: F401
    import concourse.bacc as bacc
    import concourse.mybir as mybir
    import concourse.tile as tile
    from concourse.bass_utils import run_bass_kernel_spmd

    F32 = mybir.dt.float32
    F16 = mybir.dt.float16
    FP8 = F16 if os.environ.get("KF16") else mybir.dt.float8e4
    AF = mybir.ActivationFunctionType
    OP = mybir.AluOpType

    ii = {k: np.asarray(v) for k, v in inputs.items()}
    pk = pack_all(ii)

    nc = bacc.Bacc("TRN2", target_bir_lowering=False, debug=False,
                   num_devices=NCORES)

    def din(name, arr0):
        return nc.dram_tensor(name, list(arr0.shape),
                              mybir.dt.from_np(arr0.dtype),
                              kind="ExternalInput")

    h = {}
    for tag in ("c", "b"):
        rl = pk[tag]
        pc0 = rl["per_core"][0]
        h[f"xt_{tag}"] = din(f"xt_{tag}", pc0["xt"])
        h[f"oh_{tag}"] = din(f"oh_{tag}", pc0["oh"])
        h[f"pa_{tag}"] = din(f"pa_{tag}", pc0["pa"])
        h[f"ohg_{tag}"] = din(f"ohg_{tag}", pc0["ohg"])
        h[f"w2_{tag}"] = din(f"w2_{tag}", rl["w2"])
        h[f"ws4_{tag}"] = din(f"ws4_{tag}", rl["ws4"])
    h["i8"] = din("i8", pk["ident8"])
    h["i16"] = din("i16", pk["ident16"])
    h["recip2"] = din("recip2", pk["recip2"])
    h["zl"] = din("zl", pk["zl"])
    h["zr"] = din("zr", pk["zr"])
    h["z128"] = din("z128", pk["z128"])
    for k, v in pk["mlp"].items():
        h["mlp_" + k] = din("mlp_" + k, v)
    out_h = nc.dram_tensor("out", [1, G], F32, kind="ExternalOutput")

    with tile.TileContext(nc) as tc:
        with tc.tile_pool(name="const", bufs=1) as cp, \
             tc.tile_pool(name="stream", bufs=4) as sp, \
             tc.tile_pool(name="work", bufs=3) as wp, \
             tc.tile_pool(name="svp", bufs=3, space="PSUM") as svp, \
             tc.tile_pool(name="aggp", bufs=2, space="PSUM") as aggp, \
             tc.tile_pool(name="dram", bufs=1, space="DRAM") as dp:

            i8_t = cp.tile([128, 128], FP8, tag="i8")
            nc.sync.dma_start(i8_t[:], h["i8"].ap())
            i16_t = cp.tile([128, 128], F16, tag="i16")
            nc.sync.dma_start(i16_t[:], h["i16"].ap())
            recip_t = cp.tile([128, 128], F16, tag="recip2")
            nc.sync.dma_start(recip_t[:], h["recip2"].ap())
            zl = cp.tile([1, 128], F16, tag="zl")
            nc.sync.dma_start(zl[:], h["zl"].ap())
            zr = cp.tile([1, 512], F16, tag="zr")
            nc.sync.dma_start(zr[:], h["zr"].ap())

            pooled_r = {}
            for rtag in ("c", "b"):
                pooled_r[rtag] = wp.tile([128, 64], F32, name=f"pool{rtag}",
                                         tag=f"pool{rtag}")
                nc.sync.dma_start(pooled_r[rtag][:], h["z128"].ap()[:, 0:64])

            # warm the PE clock (HAM) while the first stream DMAs land
            warm = svp.tile([128, 8, 128], F32, tag="sv")
            for _wi in range(56):
                nc.tensor.matmul(warm[:, _wi % 8, :], i16_t[:], i16_t[:],
                                 start=True, stop=True)

            slab_ctr = [0]

            def relation(tag, col_off):
                rl = pk[tag]
                w2_t = cp.tile([128, 128], F16, tag=f"w2{tag}")
                nc.sync.dma_start(w2_t[:], h[f"w2_{tag}"].ap())
                ws4_t = cp.tile([32, 64], F16, tag=f"ws4{tag}")
                nc.sync.dma_start(ws4_t[:], h[f"ws4_{tag}"].ap())
                xt_v = h[f"xt_{tag}"].ap()
                oh_v = h[f"oh_{tag}"].ap()
                pa_v = h[f"pa_{tag}"].ap()
                ohg_v = h[f"ohg_{tag}"].ap()
                first_pool = [True]
                ngroups = rl["ngroups"]
                pa_w = GRP * 128
                jobs = []
                for g, gr in enumerate(rl["groups"]):
                    if gr["n_b"] == 0:
                        continue
                    for s in range(gr["R"] + gr["F"]):
                        jobs.append((g, gr, s))
                last_g = jobs[-1][0]
                # software pipeline: scatter(slab k) emits after MM1s(k+2)
                # (2-deep: the sigmoid+mult chain is longer than one slab);
                # relu(group) emits with its last scatter; pooling(group)
                # defers one more slab so PE never waits on ACT.
                pend_sc = []
                pend_pool = []     # list of [due_idx, fn]
                cur = [0]

                def flush_sc(depth=1):
                    while len(pend_sc) > depth:
                        pend_sc.pop(0)()

                def flush_pool():
                    while pend_pool and pend_pool[0][0] <= cur[0]:
                        pend_pool.pop(0)[1]()

                def mk_pool(g, gr, agg, ohg_t):
                    n_b = gr["n_b"]
                    h_sb = wp.tile([128, GRP, 64], F16, name=f"h{tag}{g}",
                                   tag="hsb")
                    nc.scalar.activation(h_sb[:, 0:n_b, :],
                                         agg[:, 0:n_b, :], AF.Relu)

                    def pool():
                        gpool = svp.tile([128, 64], F32, name=f"gp{tag}{g}",
                                         tag="sv")
                        for i in range(n_b):
                            nc.tensor.matmul(
                                gpool[:],
                                ohg_t[:, i * 128:(i + 1) * 128],
                                h_sb[:, i, :],
                                start=(i == 0), stop=(i == n_b - 1),
                                skip_group_check=True)
                        nc.vector.tensor_tensor(
                            pooled_r[tag][:], pooled_r[tag][:],
                            gpool[:], op=OP.add)
                    pend_pool.append([cur[0] + 1, pool])

                st = {}
                xt2 = [None, 0]
                for idx, (g, gr, s) in enumerate(jobs):
                    cur[0] = idx
                    n_b, Rs, Fs = gr["n_b"], gr["R"], gr["F"]
                    nsl = Rs + Fs
                    if s == 0:
                        pa_t = sp.tile([32, pa_w], F16, name=f"pa{tag}{g}",
                                       tag="pa")
                        nc.sync.dma_start(pa_t[:],
                                          pa_v[:, g * pa_w:(g + 1) * pa_w])
                        ohg_t = sp.tile([128, GRP * 128], FP8,
                                        name=f"ohg{tag}{g}", tag="ohg")
                        nc.sync.dma_start(
                            ohg_t[:, 0:n_b * 128],
                            ohg_v[:, g * GRP * 128:
                                  g * GRP * 128 + n_b * 128])
                        agg = aggp.tile([128, GRP, 64], F32,
                                        name=f"agg{tag}{g}", tag="agg")
                        nc.tensor.matmul(agg[:, 0:n_b, :], zl[:],
                                         zr[:, 0:n_b * 64], start=True,
                                         stop=False, skip_group_check=True)
                        for i in range(n_b):
                            nc.tensor.matmul(
                                agg[:, i, :],
                                pa_t[0:17, i * 128:i * 128 + 128],
                                ws4_t[0:17, :],
                                start=False, stop=False,
                                skip_group_check=True)
                        nsc = Rs + sum(len(p) for p in gr["flex"])
                        st[g] = {"agg": agg, "ohg_t": ohg_t, "sci": [0],
                                 "nsc": nsc}
                    sg = st[g]
                    agg, ohg_t = sg["agg"], sg["ohg_t"]

                    def sc_flags(sg=sg):
                        sg["sci"][0] += 1
                        return {"start": False,
                                "stop": sg["sci"][0] == sg["nsc"],
                                "skip_group_check": True}

                    # xt DMA batched over slab pairs (within the group)
                    if s % 2 == 0:
                        wcols = min(2, nsl - s) * 1024
                        c0 = gr["xt_off"] * 2 + s * 1024
                        xt2[0] = sp.tile([128, 2048], FP8,
                                         name=f"xt{tag}{g}_{s}", tag="xt")
                        nc.sync.dma_start(xt2[0][:, 0:wcols],
                                          xt_v[:, c0:c0 + wcols])
                    xtsl = xt2[0][:, (s % 2) * 1024:(s % 2) * 1024 + 1024]
                    flex = None
                    if s >= Rs:
                        flex = gr["flex"][s - Rs]
                        ob0 = flex[0][1]
                        obn = len(flex)
                        oh_t = sp.tile([128, GRP * 128], FP8,
                                       name=f"oh{tag}{g}_{s}", tag="ohf")
                        nc.sync.dma_start(
                            oh_t[:, 0:obn * 128],
                            oh_v[:, ob0 * 128:(ob0 + obn) * 128])
                        present = set(i for i, _ in flex)
                    svk = svp.tile([128, 8, 128], F32,
                                   name=f"sv{tag}{g}_{s}", tag="sv")
                    nact = 8 if flex is None else len(flex)
                    for i in range(8):
                        if flex is not None and i not in present:
                            continue
                        nc.tensor.matmul(
                            svk[:, i, :],
                            xtsl[:, i * 128:(i + 1) * 128],
                            w2_t[:],
                            start=True, stop=True)
                    flush_sc(1)  # scatter of slab idx-2 runs behind our MM1s
                    flush_pool()
                    gt = wp.tile([128, 8, 64], F16,
                                 name=f"gt{tag}{g}_{s}", tag="gt")
                    nc.scalar.activation(gt[:, 0:nact, :],
                                         svk[:, 0:nact, 0:64], AF.Sigmoid)
                    msg = wp.tile([128, 8, 64], F16,
                                  name=f"msg{tag}{g}_{s}", tag="msg")
                    nc.vector.tensor_tensor(msg[:, 0:nact, :],
                                            gt[:, 0:nact, :],
                                            svk[:, 0:nact, 64:128],
                                            op=OP.mult)

                    def mk(flex, oh_t, msg, g=g, gr=gr, agg=agg,
                           ohg_t=ohg_t, is_last=(s == nsl - 1),
                           sc_flags=sc_flags):
                        def emit():
                            if flex is None:
                                nc.tensor.matmul(agg[:, 0:gr["n_b"], :],
                                                 i8_t[:],
                                                 msg[:, 0:gr["n_b"], :],
                                                 **sc_flags())
                            else:
                                for j, (i, _) in enumerate(flex):
                                    nc.tensor.matmul(
                                        agg[:, i, :],
                                        oh_t[:, j * 128:(j + 1) * 128],
                                        msg[:, i, :], **sc_flags())
                            if is_last:
                                mk_pool(g, gr, agg, ohg_t)
                        return emit

                    pend_sc.append(mk(flex,
                                      oh_t if flex is not None else None,
                                      msg))
                cur[0] += 1
                flush_sc(0)
                cur[0] += 1
                flush_pool()
                while pend_pool:
                    pend_pool.pop(0)[1]()

            def do_collective(rtag):
                bi = dp.tile([128, 64], F32, name=f"bi{rtag}",
                             tag=f"bi{rtag}")
                bo = dp.tile([128, 64], F32, name=f"bo{rtag}",
                             tag=f"bo{rtag}")
                nc.sync.dma_start(bi[:], pooled_r[rtag][:])
                nc.gpsimd.collective_compute(
                    "AllReduce", OP.add,
                    replica_groups=[list(range(NCORES))],
                    ins=[bi.opt()], outs=[bo.opt()])
                nc.sync.dma_start(pooled_r[rtag][:], bo[:])

            relation("c", 0)
            do_collective("c")   # overlaps relation b's compute
            relation("b", 64)
            do_collective("b")

            # --- head: divide by counts, transpose, MLP ---
            mean16 = wp.tile([128, 128], F16, tag="mean16")
            nc.vector.tensor_tensor(mean16[:, 0:64], pooled_r["c"][:],
                                    recip_t[:, 0:64], op=OP.mult)
            nc.vector.tensor_tensor(mean16[:, 64:128], pooled_r["b"][:],
                                    recip_t[:, 64:128], op=OP.mult)
            tps = aggp.tile([128, 128], F16, tag="agg")
            nc.tensor.transpose(tps[:], mean16[:], i16_t[:])
            mean_sb = wp.tile([128, 128], F16, tag="mean_sb")
            nc.vector.tensor_copy(mean_sb[:], tps[:])

            mw, mb = {}, {}
            for k in ("W1", "W2", "W3", "Wout"):
                mw[k] = cp.tile(list(pk["mlp"][k].shape), F16, name=f"mw{k}",
                                tag=f"mw{k}")
                nc.sync.dma_start(mw[k][:], h["mlp_" + k].ap())
            for k in ("b1", "b2", "b3", "bout"):
                mb[k] = cp.tile(list(pk["mlp"][k].shape), F32, name=f"mb{k}",
                                tag=f"mb{k}")
                nc.sync.dma_start(mb[k][:], h["mlp_" + k].ap())

            hcur = mean_sb
            for li, (wk, bk) in enumerate((("W1", "b1"), ("W2", "b2"),
                                           ("W3", "b3"))):
                ps = aggp.tile([64, G], F32, name=f"mlp{li}", tag="agg")
                nc.tensor.matmul(ps[:], mw[wk][:], hcur[:],
                                 start=True, stop=True)
                hn = wp.tile([64, G], F16, name=f"hn{li}", tag=f"hn{li}")
                nc.scalar.activation(hn[:], ps[:], AF.Relu, bias=mb[bk][:])
                hcur = hn
            ps_o = aggp.tile([1, G], F32, tag="agg")
            nc.tensor.matmul(ps_o[:], mw["Wout"][:], hcur[:],
                             start=True, stop=True)
            osb = wp.tile([1, G], F32, tag="osb")
            nc.scalar.activation(osb[:], ps_o[:], AF.Identity,
                                 bias=mb["bout"][:])
            nc.sync.dma_start(out_h.ap(), osb[:])

    nc.compile()

    in_maps = []
    for m in range(NCORES):
        im = {}
        for tag in ("c", "b"):
            rl = pk[tag]
            pc = rl["per_core"][m]
            im[f"xt_{tag}"] = pc["xt"]
            im[f"oh_{tag}"] = pc["oh"]
            im[f"pa_{tag}"] = pc["pa"]
            im[f"ohg_{tag}"] = pc["ohg"]
            im[f"w2_{tag}"] = rl["w2"]
            im[f"ws4_{tag}"] = rl["ws4"]
        im["i8"] = pk["ident8"]
        im["i16"] = pk["ident16"]
        im["recip2"] = pk["recip2"]
        im["zl"] = pk["zl"]
        im["zr"] = pk["zr"]
        im["z128"] = pk["z128"]
        for k, v in pk["mlp"].items():
            im["mlp_" + k] = v
        in_maps.append(im)

    trace = bool(os.environ.get("KERNEL_TRACE"))
    res = run_bass_kernel_spmd(nc, in_maps, core_ids=list(range(NCORES)),
                               trace=trace)
    global LAST_EXEC_NS
    LAST_EXEC_NS = res.exec_time_ns
    return res.results[0]["out"].reshape(G).astype(np.float32)
